# revision 7
# baseline (speedup 1.0000x reference)
"""Trainium2 Bass kernel for nn_ADSCDConv (dense_cnn), 8-core data parallel.

Per core (2 samples = 384 (b,c) channel-images of 96x96), groups of 128
partitions: g0=(b0,c0:128), g1=(b1,c0:128), g2=(b0,c128:192)||(b1,c128:192).

v3 schedule (vs v2):
  - center tap on PE for all windows except (0,0) (9-tap PSUM
    accumulation, center last); drains are pure ScalarE ACT copies.
  - band maxes + most window sums via single tensor_tensor_reduce ops
    (op0=max/add + accum reduce) on DVE, replacing the TT trees.
  - bulk window sums split DVE/ScalarE so neither FIFO gates the
    g0-conv start; diag for g1/g2 on ScalarE.
  - DMA priority: xA g0/g2low chunk-interleaved, then g2high, g1 on the
    sync queue; xB kicks held on the ACT queue until the critical xA
    has landed.
  - PE rows 65/70/70, DVE tail rows 31/26/26.
"""

from contextlib import ExitStack

import numpy as np
import ml_dtypes

BF16 = ml_dtypes.bfloat16

B, C, H, W = 16, 192, 96, 96
G = 4
R = C // 4  # 48
BN_EPS = 1e-5
N_CORES = 8
HP, WP = H + 2, 100  # padded rows 98, padded cols 100 (x payload at col 2)
XB_R0 = 63           # padB covers padded rows 63..97
XB_NR = 35

# conv windows: banks per window, pool A(4)/B(3) alternating
WIN_SEQ = {0: [4, 3, 4, 3], 1: [4, 3, 4, 3], 2: [4, 3, 4, 3]}
ROWS_PE = {g: 5 * sum(WIN_SEQ[g]) for g in range(3)}  # 70, 70, 70

TAIL_CHUNKS = {0: [(0, 7), (7, 13), (13, 26)],
               1: [(0, 13), (13, 26)],
               2: [(0, 13), (13, 26)]}

# tap order: center (tap 4, the only theta-dependent tap) last
TAP_ORDER = [0, 1, 2, 3, 5, 6, 7, 8, 4]
NC8 = TAP_ORDER[:8]

_COMPILED = None


def _build():
    import concourse.tile as tile
    from concourse import bacc, mybir

    f32 = mybir.dt.float32
    bf16 = mybir.dt.bfloat16
    ALU = mybir.AluOpType
    ACTF = mybir.ActivationFunctionType

    nc = bacc.Bacc("TRN2", target_bir_lowering=False, debug=False, num_devices=N_CORES)

    # ---- DRAM tensors ----
    xA_d = nc.dram_tensor("xA", [384, HP, WP], bf16, kind="ExternalInput").ap()
    xB_d = nc.dram_tensor("xB", [384, XB_NR, WP], bf16, kind="ExternalInput").ap()
    out_d = nc.dram_tensor("out", [384, H, W], bf16, kind="ExternalOutput").ap()
    warm_d = nc.dram_tensor("warm", [128, 1], bf16, kind="ExternalOutput").ap()
    eye_d = nc.dram_tensor("eye", [128, 128], bf16, kind="ExternalInput").ap()
    w1avg_a_d = nc.dram_tensor("w1avg_a", [128, R], f32, kind="ExternalInput").ap()
    w1avg_b_d = nc.dram_tensor("w1avg_b", [128, R], f32, kind="ExternalInput").ap()
    w1mx_a_d = nc.dram_tensor("w1mx_a", [128, R], f32, kind="ExternalInput").ap()
    w1mx_b_d = nc.dram_tensor("w1mx_b", [128, R], f32, kind="ExternalInput").ap()
    w2t_d = nc.dram_tensor("w2t", [R, C], f32, kind="ExternalInput").ap()
    p1a_d = nc.dram_tensor("p1a", [128, R], f32, kind="ExternalInput").ap()
    p1b_d = nc.dram_tensor("p1b", [128, R], f32, kind="ExternalInput").ap()
    bns_d = nc.dram_tensor("bn_scale", [R, 1], f32, kind="ExternalInput").ap()
    bnb_d = nc.dram_tensor("bn_beta", [R, 1], f32, kind="ExternalInput").ap()
    w2s_d = nc.dram_tensor("w2s", [R, G * C], f32, kind="ExternalInput").ap()
    adkT_d = nc.dram_tensor("adkT", [384, 36], f32, kind="ExternalInput").ap()

    with tile.TileContext(nc) as tc, ExitStack() as ctx:
        def sb(name, shape, dt):
            return nc.alloc_sbuf_tensor(name, shape, dt).ap()

        padA = [sb(f"padA{g}", [128, HP, WP], bf16) for g in range(3)]
        padB = [sb(f"padB{g}", [128, XB_NR, WP], bf16) for g in range(3)]
        tailb = [sb(f"tail{g}", [128, 31, W], bf16) for g in range(3)]
        diag = [sb(f"diag{g}", [128, 9, 128], bf16) for g in range(3)]
        pooled = [sb(f"pooled{g}", [128, 9], f32) for g in range(3)]
        avgs = [sb(f"avgs{g}", [128, 1], f32) for g in range(3)]
        mx = [sb(f"mx{g}", [128, 1], f32) for g in range(3)]
        th = [sb(f"theta{g}", [128, 1], f32) for g in range(3)]
        w9 = [sb(f"w9_{g}", [128, 9], f32) for g in range(3)]
        w4p = [sb(f"w4p{g}", [128, 1], f32) for g in range(3)]
        wsum9 = [sb(f"wsum9_{g}", [128, 1], f32) for g in range(3)]
        adkT = [sb(f"adkT{g}_sb", [128, 36], f32) for g in range(3)]

        eye = sb("eye_sb", [128, 128], bf16)
        w1avg_a = sb("w1avg_a_sb", [128, R], f32)
        w1avg_b = sb("w1avg_b_sb", [128, R], f32)
        w1mx_a = sb("w1mx_a_sb", [128, R], f32)
        w1mx_b = sb("w1mx_b_sb", [128, R], f32)
        w2t = sb("w2t_sb", [R, C], f32)
        p1a = sb("p1a_sb", [128, R], f32)
        p1b = sb("p1b_sb", [128, R], f32)
        bns = sb("bns_sb", [R, 1], f32)
        bnb = sb("bnb_sb", [R, 1], f32)
        w2s = sb("w2s_sb", [R, G * C], f32)

        h_adk = [sb(f"h_adk{b}", [R, 9], f32) for b in range(2)]
        hsum = [sb(f"hsum{b}", [R, 1], f32) for b in range(2)]

        scr = ctx.enter_context(tc.tile_pool(name="scr", bufs=4))
        treep = ctx.enter_context(tc.tile_pool(name="treep", bufs=2))
        term_pool = ctx.enter_context(tc.tile_pool(name="terms", bufs=3))
        osb_pool = ctx.enter_context(tc.tile_pool(name="osbp", bufs=6))
        ct_pool = ctx.enter_context(tc.tile_pool(name="ctp", bufs=1))
        psA = ctx.enter_context(tc.tile_pool(name="psA", bufs=1, space="PSUM"))
        psB = ctx.enter_context(tc.tile_pool(name="psB", bufs=1, space="PSUM"))
        # stats bank: single-shot matmul groups only may share a bank
        stpa = nc.alloc_psum_tensor("statps", [128, 512], f32).ap()

        # ---------------- DMA emission ----------------
        row_chunks = [(0, 33), (33, 65), (65, HP)]

        def xA_chunk(g, ci, q0=0, q1=128):
            r0, r1 = row_chunks[ci]
            nc.sync.dma_start(
                out=padA[g][q0:q1, r0:r1, :],
                in_=xA_d[g * 128 + q0:g * 128 + q1, r0:r1, :],
            )

        def emit_xB_dma(g):
            nc.scalar.dma_start(
                out=padB[g][:, :, :],
                in_=xB_d[g * 128:(g + 1) * 128, :, :],
            )

        # priority order on the sync queue: g0/g2low interleaved, then
        # g2high, then g1
        xA_chunk(0, 0)
        xA_chunk(0, 1)
        xA_chunk(2, 0, 0, 64)
        xA_chunk(2, 1, 0, 64)
        xA_chunk(0, 2)
        xA_chunk(2, 2, 0, 64)
        for ci in range(3):
            xA_chunk(2, ci, 64, 128)
        for ci in range(3):
            xA_chunk(1, ci)
        wloads = [
            (eye, eye_d), (w1avg_a, w1avg_a_d), (w1avg_b, w1avg_b_d),
            (w1mx_a, w1mx_a_d), (w1mx_b, w1mx_b_d), (w2t, w2t_d),
            (p1a, p1a_d), (p1b, p1b_d), (bns, bns_d), (bnb, bnb_d),
            (w2s, w2s_d),
            (adkT[0], adkT_d[0:128, :]), (adkT[1], adkT_d[128:256, :]),
            (adkT[2], adkT_d[256:384, :]),
        ]
        for (dst, src) in wloads:
            nc.gpsimd.dma_start(out=dst, in_=src)

        # ---------------- stats ----------------
        def sums_scl(g, k, q0=0, q1=128):
            # 3 window sums of band k via ScalarE ACT accumulate
            for j in range(3):
                win = padA[g][q0:q1, 1 + 32 * k:33 + 32 * k, 2 + 32 * j:34 + 32 * j]
                acc = pooled[g][q0:q1, 3 * k + j:3 * k + j + 1]
                s = treep.tile([128, 32, 32], bf16, tag="wscr", name=f"w{g}_{k}_{j}")
                nc.scalar.activation(out=s[q0:q1, :, :], in_=win,
                                     func=ACTF.Copy, accum_out=acc)

        mxb = [sb(f"mxb{g}", [128, 3], f32) for g in range(3)]

        def sums_ts(g, k, q0=0, q1=128):
            # 3 window sums of band k via one TS-accum op each on DVE
            r0 = 1 + 32 * k
            for j in range(3):
                c0 = 2 + 32 * j
                s = treep.tile([128, 32, 32], bf16, tag="smo", name=f"ws{g}_{k}_{j}")
                nc.vector.tensor_scalar(
                    s[q0:q1, :, :], padA[g][q0:q1, r0:r0 + 32, c0:c0 + 32],
                    1.0, None, op0=ALU.mult, op1=ALU.add,
                    accum_out=pooled[g][q0:q1, 3 * k + j:3 * k + j + 1],
                )

        def max_ts(g, k, q0=0, q1=128):
            # band max via one TS-accum(max) op into mxb[g][:, k]
            r0 = 1 + 32 * k
            s = treep.tile([128, 32, WP], bf16, tag="mxo", name=f"mx{g}_{k}_{q0}")
            nc.vector.tensor_scalar(
                s[q0:q1, :, :], padA[g][q0:q1, r0:r0 + 32, :],
                1.0, None, op0=ALU.mult, op1=ALU.max,
                accum_out=mxb[g][q0:q1, k:k + 1],
            )

        def max_fin(g, q0=0, q1=128):
            nc.vector.tensor_reduce(out=mx[g][q0:q1, :], in_=mxb[g][q0:q1, :],
                                    axis=mybir.AxisListType.X, op=ALU.max)

        def emit_avg_fin(g, q0=0, q1=128):
            asc = scr.tile([128, 9], bf16, tag="ascr", name=f"avg{g}_{q0}")
            nc.scalar.activation(out=asc[q0:q1, :], in_=pooled[g][q0:q1, :],
                                 func=ACTF.Copy, accum_out=avgs[g][q0:q1, :])

        # ---------------- per-sample algebra ----------------
        def emit_sample(b, part):
            if b == 0:
                chunks = [
                    (w1avg_a[:, :], w1mx_a[:, :], p1a[:, :], (0, 0, 128)),
                    (w1avg_b[0:64, :], w1mx_b[0:64, :], p1b[0:64, :], (2, 0, 64)),
                ]
            else:
                chunks = [
                    (w1avg_a[:, :], w1mx_a[:, :], p1a[:, :], (1, 0, 128)),
                    (w1avg_b[64:128, :], w1mx_b[64:128, :], p1b[64:128, :], (2, 64, 128)),
                ]
            base = 8 + b * 22
            for i, (wa, wm, wp, (sg, q0, q1)) in enumerate(chunks):
                o = base + 11 * i
                if part == "pool":
                    nc.tensor.matmul(stpa[0:R, o + 2:o + 11], lhsT=wp, rhs=pooled[sg][q0:q1, :], start=True, stop=True)
                else:
                    nc.tensor.matmul(stpa[0:R, o:o + 1], lhsT=wa, rhs=avgs[sg][q0:q1, :], start=True, stop=True)
                    nc.tensor.matmul(stpa[0:R, o + 1:o + 2], lhsT=wm, rhs=mx[sg][q0:q1, :], start=True, stop=True)

        def emit_fold_pool(b):
            base = 8 + b * 22
            hc = scr.tile([R, 9], f32, tag="scr48", name=f"hc{b}")
            nc.vector.tensor_copy(hc[:, :], stpa[0:R, base + 2:base + 11])
            hs = scr.tile([R, 9], f32, tag="scr48", name=f"hs{b}")
            nc.vector.tensor_add(hs[:, :], hc[:, :],
                                 stpa[0:R, base + 13:base + 22])
            t1 = scr.tile([R, 9], f32, tag="scr48", name=f"bn{b}")
            nc.vector.tensor_scalar(t1[:, :], hs[:, :], bns[:, :], bnb[:, :],
                                    op0=ALU.mult, op1=ALU.add)
            nc.vector.tensor_scalar_max(h_adk[b][:, :], t1[:, :], 0.0)

        def emit_fold_theta(b):
            base = 8 + b * 22
            hg = scr.tile([R, 2], f32, tag="scr2", name=f"hg{b}")
            nc.vector.tensor_copy(hg[:, :], stpa[0:R, base:base + 2])
            hs = scr.tile([R, 2], f32, tag="scr2", name=f"ht{b}")
            nc.vector.tensor_add(hs[:, :], hg[:, :],
                                 stpa[0:R, base + 11:base + 13])
            ha = scr.tile([R, 1], f32, tag="scr1", name=f"ha{b}")
            hm = scr.tile([R, 1], f32, tag="scr1", name=f"hm{b}")
            nc.vector.tensor_scalar_max(ha[:, :], hs[:, 0:1], 0.0)
            nc.vector.tensor_scalar_max(hm[:, :], hs[:, 1:2], 0.0)
            nc.vector.tensor_add(hsum[b][:, :], ha[:, :], hm[:, :])

        # ---------------- theta ----------------
        ps_t = [stpa[:, i:i + 1] for i in range(3)]

        def emit_theta_mm(b):
            nc.tensor.matmul(ps_t[b], lhsT=w2t[:, 0:128], rhs=hsum[b][:, :], start=True, stop=True)
            q0, q1 = (0, 64) if b == 0 else (64, 128)
            nc.tensor.matmul(ps_t[2][q0:q1], lhsT=w2t[:, 128:192], rhs=hsum[b][:, :], start=True, stop=True)

        def emit_theta_fin(g):
            et = scr.tile([128, 1], f32, tag="scr1", name=f"et{g}")
            nc.scalar.activation(out=et[:, :], in_=ps_t[g], func=ACTF.Exp, scale=-1.0)
            d = scr.tile([128, 1], f32, tag="scr1", name=f"etd{g}")
            nc.vector.tensor_scalar_add(d[:, :], et[:, :], 1.0)
            nc.vector.reciprocal(th[g][:, :], d[:, :])

        # ---------------- dynamic kernels w9 ----------------
        def emit_w9_mm(g):
            ps_s = stpa[:, 64 + g * 36:64 + (g + 1) * 36]
            for gg in range(G):
                sl = slice(gg * 9, gg * 9 + 9)
                if g < 2:
                    nc.tensor.matmul(ps_s[:, sl], lhsT=w2s[:, gg * 192:gg * 192 + 128],
                                     rhs=h_adk[g][:, :], start=True, stop=True)
                else:
                    nc.tensor.matmul(ps_s[0:64, sl], lhsT=w2s[:, gg * 192 + 128:gg * 192 + 192],
                                     rhs=h_adk[0][:, :], start=True, stop=True)
                    nc.tensor.matmul(ps_s[64:128, sl], lhsT=w2s[:, gg * 192 + 128:gg * 192 + 192],
                                     rhs=h_adk[1][:, :], start=True, stop=True)

        def emit_w9_exp(g):
            ps_s = stpa[:, 64 + g * 36:64 + (g + 1) * 36]
            e = scr.tile([128, 36], f32, tag="scr36", name=f"e{g}")
            nc.scalar.activation(out=e[:, :], in_=ps_s, func=ACTF.Exp)
            return e

        def emit_w9_fin(g, e):
            d1 = scr.tile([128, 9], f32, tag="scr9", name=f"d1_{g}")
            d2 = scr.tile([128, 9], f32, tag="scr9", name=f"d2_{g}")
            nc.vector.tensor_add(d1[:, :], e[:, 0:9], e[:, 9:18])
            nc.vector.tensor_add(d2[:, :], e[:, 18:27], e[:, 27:36])
            nc.vector.tensor_add(d1[:, :], d1[:, :], d2[:, :])
            rec = scr.tile([128, 9], f32, tag="scr9", name=f"rec{g}")
            nc.vector.reciprocal(rec[:, :], d1[:, :])
            a = adkT[g]
            m1 = scr.tile([128, 9], f32, tag="scr9", name=f"m1_{g}")
            m2 = scr.tile([128, 9], f32, tag="scr9", name=f"m2_{g}")
            nc.vector.tensor_mul(m1[:, :], e[:, 0:9], a[:, 0:9])
            nc.vector.tensor_mul(m2[:, :], e[:, 9:18], a[:, 9:18])
            nc.vector.tensor_add(m1[:, :], m1[:, :], m2[:, :])
            nc.vector.tensor_mul(m2[:, :], e[:, 18:27], a[:, 18:27])
            nc.vector.tensor_add(m1[:, :], m1[:, :], m2[:, :])
            nc.vector.tensor_mul(m2[:, :], e[:, 27:36], a[:, 27:36])
            nc.vector.tensor_add(m1[:, :], m1[:, :], m2[:, :])
            nc.vector.tensor_mul(w9[g][:, :], m1[:, :], rec[:, :])
            nc.vector.tensor_reduce(out=wsum9[g][:, :], in_=w9[g][:, :],
                                    axis=mybir.AxisListType.X, op=ALU.add)

        def emit_w4p(g):
            t1 = scr.tile([128, 1], f32, tag="scr1", name=f"t1_{g}")
            nc.vector.tensor_mul(t1[:, :], w9[g][:, 4:5], th[g][:, :])
            nc.vector.tensor_add(t1[:, :], t1[:, :], w9[g][:, 4:5])
            nc.vector.tensor_sub(w4p[g][:, :], t1[:, :], wsum9[g][:, :])

        def emit_diag(g, taps, engine):
            for tap in taps:
                scal = w4p[g][:, 0:1] if tap == 4 else w9[g][:, tap:tap + 1]
                if engine == "vector":
                    nc.vector.tensor_scalar_mul(diag[g][:, tap, :], eye[:, :], scal)
                else:
                    nc.scalar.activation(out=diag[g][:, tap, :], in_=eye[:, :],
                                         func=ACTF.Copy, scale=scal)

        # ---------------- conv on PE ----------------
        win_r0 = {}
        for g in range(3):
            r = 0
            for w, nb in enumerate(WIN_SEQ[g]):
                win_r0[(g, w)] = r
                r += 5 * nb

        pools = {0: psA, 1: psB}
        win_tile = {}

        def conv_taps(g, w, taps):
            nb = WIN_SEQ[g][w]
            r0 = win_r0[(g, w)]
            key = (g, w)
            if key not in win_tile:
                pool = pools[w % 2]
                pnb = 4 if w % 2 == 0 else 3
                win_tile[key] = pool.tile(
                    [128, pnb, 512], f32, tag=f"w{pnb}", name=f"ps{g}_{w}")
            ps = win_tile[key]
            for tap in taps:
                dy, dx = divmod(tap, 3)
                for b in range(nb):
                    y0 = r0 + 5 * b + dy
                    nc.tensor.matmul(
                        ps[:, b, 0:480],
                        lhsT=diag[g][:, tap, :],
                        rhs=padA[g][:, y0:y0 + 5, dx + 1:dx + 97],
                        start=(tap == taps[0]), stop=(tap == taps[-1]),
                    )

        def conv_drain(g, w, center=False):
            nb = WIN_SEQ[g][w]
            r0 = win_r0[(g, w)]
            ps = win_tile.pop((g, w))
            nr = 5 * nb
            ot = osb_pool.tile([128, 20, W], bf16, tag="ow", name=f"ow{g}_{w}")
            nc.scalar.activation(
                out=ot[:, 0:nr, :],
                in_=ps[:, 0:nb, 0:480], func=ACTF.Copy)
            if center:
                tm = ct_pool.tile([128, 20, W], bf16, tag="ct", name=f"ct{g}_{w}")
                nc.vector.tensor_scalar_mul(
                    tm[:, 0:nr, :], padA[g][:, r0 + 1:r0 + nr + 1, 2:98], w4p[g][:, 0:1])
                nc.vector.tensor_add(ot[:, 0:nr, :], ot[:, 0:nr, :], tm[:, 0:nr, :])
            nc.sync.dma_start(
                out=out_d[g * 128:(g + 1) * 128, r0:r0 + nr, :],
                in_=ot[:, 0:nr, :])

        # ---------------- conv tail on DVE ----------------
        def emit_conv_dve(g, lo, hi):
            y0 = ROWS_PE[g] + lo
            n = hi - lo
            acc = None
            for i, tap in enumerate(TAP_ORDER):
                dy, dx = divmod(tap, 3)
                scal = w4p[g][:, 0:1] if tap == 4 else w9[g][:, tap:tap + 1]
                if dx == 1:
                    src = padA[g][:, y0 + dy:y0 + n + dy, 2:98]
                else:
                    rb = y0 + dy - XB_R0
                    col = 2 if dx == 0 else 4
                    src = padB[g][:, rb:rb + n, col:col + 96]
                t = term_pool.tile([128, 13, 96], bf16, tag="term",
                                   name=f"t{g}_{lo}_{i}")
                nc.vector.tensor_scalar_mul(t[:, 0:n, :], src, scal)
                if i == 0:
                    acc = t
                elif i < 8:
                    nxt = term_pool.tile([128, 13, 96], bf16, tag="term",
                                         name=f"a{g}_{lo}_{i}")
                    nc.vector.tensor_add(nxt[:, 0:n, :], acc[:, 0:n, :], t[:, 0:n, :])
                    acc = nxt
                else:
                    nc.vector.tensor_add(tailb[g][:, lo:hi, :], acc[:, 0:n, :], t[:, 0:n, :])
            nc.sync.dma_start(out=out_d[g * 128:(g + 1) * 128, y0:y0 + n, :],
                              in_=tailb[g][:, lo:hi, :])

        # ---------------- PE warm-up ----------------
        def emit_warmup(k, gate, lhsT=None, drain=False):
            for j in range(k):
                nc.tensor.matmul(stpa[:, 384:512], lhsT=(lhsT if lhsT is not None else eye[:, :]),
                                 rhs=gate, start=True, stop=True)
            if drain:
                wsc = scr.tile([128, 1], bf16, tag="wscr1", name="wscr")
                nc.scalar.activation(out=wsc[:, :], in_=stpa[:, 384:385], func=ACTF.Copy)
                nc.sync.dma_start(out=warm_d, in_=wsc[:, :])

        # ================ emission order ================
        # -- early stats: SCL takes g0 b0 + g2low b0; DVE takes the rest
        sums_scl(0, 0)
        sums_scl(2, 0, 0, 64)

        max_ts(0, 0)
        sums_ts(0, 1)
        max_ts(0, 1)
        sums_ts(2, 1, 0, 64)
        sums_ts(0, 2)
        sums_ts(2, 2, 0, 64)

        # PE warmups gated on arriving chunks
        emit_warmup(14, padA[0][:, 10:12, 0:64])
        emit_warmup(14, padA[0][:, 40:42, 0:64])
        emit_warmup(12, padA[2][0:64, 40:42, 0:64], lhsT=eye[0:64, :])
        emit_warmup(10, padA[0][:, 70:72, 0:64], drain=True)
        emit_warmup(10, padA[2][0:64, 70:72, 0:64], lhsT=eye[0:64, :])

        # w9(0) chain -> diag(0) -> conv start
        emit_sample(0, "pool")      # PE
        emit_fold_pool(0)           # DVE
        emit_w9_mm(0)               # PE
        e0 = emit_w9_exp(0)         # SCL
        emit_w9_fin(0, e0)          # DVE
        emit_diag(0, NC8, "vector")

        conv_taps(0, 0, NC8)

        # theta(0) chain (center of w00 handled at drain on DVE).
        # DVE order: g0/g2low maxes -> g1-b2 sums -> fold_theta(0) ->
        # g2high sums -> theta fin -> w4p(0); SCL order: avgfins ->
        # g1 b0/b1 sums -> theta exp -> xB kicks -> drain(0,0) copy.
        max_ts(0, 2)
        max_fin(0)
        max_ts(2, 0, 0, 64)
        max_ts(2, 1, 0, 64)
        max_ts(2, 2, 0, 64)
        max_fin(2, 0, 64)
        emit_avg_fin(0)             # SCL
        emit_avg_fin(2, 0, 64)      # SCL
        emit_sample(0, "theta")     # PE
        sums_ts(1, 2)               # DVE (g1 band-2 sums fill the hole)
        emit_fold_theta(0)          # DVE
        emit_theta_mm(0)            # PE
        sums_scl(1, 0)              # SCL
        sums_scl(1, 1)              # SCL
        sums_ts(2, 0, 64, 128)      # DVE filler while PE runs conv00
        sums_ts(2, 1, 64, 128)
        sums_ts(2, 2, 64, 128)
        emit_theta_fin(0)           # SCL exp + DVE
        emit_w4p(0)                 # DVE
        emit_diag(0, [4], "vector")

        conv_taps(0, 1, TAP_ORDER)

        emit_xB_dma(0)
        emit_xB_dma(1)
        emit_xB_dma(2)
        conv_drain(0, 0, center=True)

        max_ts(2, 0, 64, 128)       # DVE: g2high + g1 maxes
        max_ts(2, 1, 64, 128)
        max_ts(2, 2, 64, 128)
        max_fin(2, 64, 128)
        max_ts(1, 0)
        max_ts(1, 1)
        max_ts(1, 2)
        max_fin(1)

        emit_avg_fin(1)             # SCL
        emit_avg_fin(2, 64, 128)

        conv_taps(0, 2, TAP_ORDER)

        emit_sample(1, "pool")      # PE
        emit_fold_pool(1)           # DVE
        emit_conv_dve(0, *TAIL_CHUNKS[0][0])

        conv_drain(0, 1)
        conv_taps(0, 3, TAP_ORDER)

        emit_sample(1, "theta")     # PE
        emit_fold_theta(1)          # DVE
        emit_theta_mm(1)            # PE
        emit_w9_mm(1)               # PE
        emit_w9_mm(2)               # PE
        e1 = emit_w9_exp(1)         # SCL
        e2 = emit_w9_exp(2)         # SCL
        emit_theta_fin(1)           # SCL + DVE
        emit_w9_fin(1, e1)          # DVE
        emit_w4p(1)                 # DVE
        emit_diag(1, TAP_ORDER, "scalar")
        emit_theta_fin(2)           # SCL + DVE
        emit_w9_fin(2, e2)          # DVE
        emit_w4p(2)                 # DVE
        emit_diag(2, TAP_ORDER, "scalar")

        conv_drain(0, 2)
        conv_taps(1, 0, TAP_ORDER)
        emit_conv_dve(0, *TAIL_CHUNKS[0][1])
        conv_drain(0, 3)
        conv_taps(1, 1, TAP_ORDER)
        emit_conv_dve(0, *TAIL_CHUNKS[0][2])
        conv_drain(1, 0)
        conv_taps(1, 2, TAP_ORDER)
        emit_conv_dve(1, *TAIL_CHUNKS[1][0])
        conv_drain(1, 1)
        conv_taps(1, 3, TAP_ORDER)
        emit_conv_dve(1, *TAIL_CHUNKS[1][1])
        conv_drain(1, 2)
        conv_taps(2, 0, TAP_ORDER)
        emit_conv_dve(2, *TAIL_CHUNKS[2][0])
        conv_drain(1, 3)
        conv_taps(2, 1, TAP_ORDER)
        emit_conv_dve(2, *TAIL_CHUNKS[2][1])
        conv_drain(2, 0)
        conv_taps(2, 2, TAP_ORDER)
        conv_drain(2, 1)
        conv_taps(2, 3, TAP_ORDER)
        conv_drain(2, 2)
        conv_drain(2, 3)

    nc.compile()
    return nc


def _host_prep(inputs):
    x = np.ascontiguousarray(inputs["x"], dtype=np.float32)
    cam_w1 = np.asarray(inputs["cam_w1"], dtype=np.float32)
    cam_w2 = np.asarray(inputs["cam_w2"], dtype=np.float32)
    proj_w1 = np.asarray(inputs["proj_w1"], dtype=np.float32)
    bn_gamma = np.asarray(inputs["bn_gamma"], dtype=np.float32)
    bn_beta = np.asarray(inputs["bn_beta"], dtype=np.float32)
    proj_w2 = np.asarray(inputs["proj_w2"], dtype=np.float32)
    adk = np.asarray(inputs["adk_weight"], dtype=np.float32)

    xb16 = x.astype(BF16)
    xpA = np.zeros((B, C, HP, WP), dtype=BF16)
    xpA[:, :, 1:97, 2:98] = xb16
    # padB: x payload at col 3, rows = padded rows 63..96 (x rows 62..95)
    xpB = np.zeros((B, C, XB_NR, WP), dtype=BF16)
    xpB[:, :, 0:34, 3:99] = xb16[:, :, 62:96, :]

    in_maps = []
    w1t = cam_w1.T.astype(np.float32)
    p1t = (proj_w1.T / 1024.0).astype(np.float32)
    cmap = np.concatenate([np.arange(128), np.arange(128),
                           np.arange(128, 192), np.arange(128, 192)])
    consts = {
        "eye": np.eye(128, dtype=BF16),
        "w1avg_a": np.ascontiguousarray(w1t[0:128] / (H * W)),
        "w1avg_b": np.ascontiguousarray(np.concatenate([w1t[128:192] / (H * W)] * 2, axis=0)),
        "w1mx_a": np.ascontiguousarray(w1t[0:128]),
        "w1mx_b": np.ascontiguousarray(np.concatenate([w1t[128:192]] * 2, axis=0)),
        "w2t": np.ascontiguousarray(cam_w2.T.astype(np.float32)),
        "p1a": np.ascontiguousarray(p1t[0:128]),
        "p1b": np.ascontiguousarray(np.concatenate([p1t[128:192]] * 2, axis=0)),
        "bn_scale": np.ascontiguousarray((bn_gamma / np.sqrt(1.0 + BN_EPS)).reshape(R, 1)),
        "bn_beta": np.ascontiguousarray(bn_beta.reshape(R, 1)),
        "w2s": np.ascontiguousarray(proj_w2.T.astype(np.float32)),
        "adkT": np.ascontiguousarray(
            adk.transpose(1, 0, 2, 3).reshape(C, G * 9)[cmap].astype(np.float32)
        ),
    }
    for k in range(N_CORES):
        b0, b1 = 2 * k, 2 * k + 1
        shardA = np.ascontiguousarray(np.concatenate(
            [xpA[b0, 0:128], xpA[b1, 0:128], xpA[b0, 128:192], xpA[b1, 128:192]],
            axis=0))
        shardB = np.ascontiguousarray(np.concatenate(
            [xpB[b0, 0:128], xpB[b1, 0:128], xpB[b0, 128:192], xpB[b1, 128:192]],
            axis=0))
        m = {"xA": shardA, "xB": shardB}
        m.update(consts)
        in_maps.append(m)
    return in_maps


def kernel(**inputs) -> np.ndarray:
    global _COMPILED
    from concourse.bass_utils import run_bass_kernel_spmd

    in_maps = _host_prep(inputs)

    if _COMPILED is None:
        _COMPILED = _build()
    nc = _COMPILED

    res = run_bass_kernel_spmd(nc, in_maps, core_ids=list(range(N_CORES)))
    outs = [r["out"] for r in res.results]

    y = np.empty((B, C, H, W), np.float32)
    for k in range(N_CORES):
        o = np.asarray(outs[k]).reshape(384, H, W).astype(np.float32)
        b0, b1 = 2 * k, 2 * k + 1
        y[b0, 0:128] = o[0:128]
        y[b1, 0:128] = o[128:256]
        y[b0, 128:192] = o[256:320]
        y[b1, 128:192] = o[320:384]
    return y


if __name__ == "__main__":
    import reference

    inputs = {k: np.asarray(v) for k, v in reference.setup_inputs().items()}
    y = kernel(**inputs)
    print("kernel output:", y.shape, y.dtype)


# revision 12
# speedup vs baseline: 1.1356x; 1.1356x over previous
"""Trainium2 Bass kernel for nn_ADSCDConv (dense_cnn), 8-core data parallel.

Per core (2 samples = 384 (b,c) channel-images of 96x96), groups of 128
partitions: g0=(b0,c0:128), g1=(b1,c0:128), g2=(b0,c128:192)||(b1,c128:192).

v4 schedule (vs v2 baseline):
  - center tap on PE for all windows except (0,0)/(0,1); drains split
    into a PSUM-freeing ScalarE copy and a (theta-gated) DVE center
    add + DMA so PSUM recycling never waits on theta.
  - incremental per-band algebra: pooled band k feeds h/w9/diag for
    tap row dy=k only, so windows (0,0)/(0,1) start their dy=0 taps
    as soon as band 0 of g0+g2 has landed.
  - stats via the v2-proven DVE trees (g0) + ScalarE ACT-accum (g2,
    g1); w9 PSUM/adkT laid out band-major.
  - DVE tail taps read flat contiguous strips (full padded rows) so
    the muls hit the 4x DVE mode; only the final add is strided.
  - DMA: g0+g1 serialized on the sync queue, g2 parallel on the
    vector queue, weights on gpsimd, xB kicked mid-stream from the
    ACT queue.
  - PE rows 70/80/70, DVE tail rows 26/16/26.
"""

from contextlib import ExitStack

import numpy as np
import ml_dtypes

BF16 = ml_dtypes.bfloat16

B, C, H, W = 16, 192, 96, 96
G = 4
R = C // 4  # 48
BN_EPS = 1e-5
N_CORES = 8
HP, WP = H + 2, 100  # padded rows 98, padded cols 100 (x payload at col 2)
XB_R0 = 63           # padB covers padded rows 63..97
XB_NR = 35

# conv windows: banks per window; PSUM pool A(4)/B(3) alternates by
# GLOBAL window index across groups
WIN_SEQ = {0: [4, 3, 4, 3], 1: [4, 3, 4, 3, 1], 2: [3, 4, 3, 4]}
ROWS_PE = {g: 5 * sum(WIN_SEQ[g]) for g in range(3)}  # 70, 75, 70

TAIL_CHUNKS = {0: [(0, 13), (13, 26)],
               1: [(0, 11), (11, 21)],
               2: [(0, 13), (13, 26)]}

# tap order: center (tap 4, the only theta-dependent tap) last
TAP_ORDER = [0, 1, 2, 3, 5, 6, 7, 8, 4]
NC8 = TAP_ORDER[:8]
BAND_TAPS = {0: [0, 1, 2], 1: [3, 5], 2: [6, 7, 8]}

_COMPILED = None


def _build():
    import concourse.tile as tile
    from concourse import bacc, mybir

    f32 = mybir.dt.float32
    bf16 = mybir.dt.bfloat16
    ALU = mybir.AluOpType
    ACTF = mybir.ActivationFunctionType

    nc = bacc.Bacc("TRN2", target_bir_lowering=False, debug=False, num_devices=N_CORES)

    # ---- DRAM tensors ----
    xA_d = nc.dram_tensor("xA", [384, HP, WP], bf16, kind="ExternalInput").ap()
    xB_d = nc.dram_tensor("xB", [384, XB_NR, WP], bf16, kind="ExternalInput").ap()
    out_d = nc.dram_tensor("out", [384, H, W], bf16, kind="ExternalOutput").ap()
    warm_d = nc.dram_tensor("warm", [128, 1], bf16, kind="ExternalOutput").ap()
    eye_d = nc.dram_tensor("eye", [128, 128], bf16, kind="ExternalInput").ap()
    pk128_d = nc.dram_tensor("pk128", [128, 6 * R], f32, kind="ExternalInput").ap()
    pk48_d = nc.dram_tensor("pk48", [R, 962], f32, kind="ExternalInput").ap()
    adkT_d = nc.dram_tensor("adkT", [128, 108], f32, kind="ExternalInput").ap()

    with tile.TileContext(nc) as tc, ExitStack() as ctx:
        def sb(name, shape, dt):
            return nc.alloc_sbuf_tensor(name, shape, dt).ap()

        padA = [sb(f"padA{g}", [128, HP, WP], bf16) for g in range(3)]
        padB = [sb(f"padB{g}", [128, XB_NR, WP], bf16) for g in range(3)]
        padAf = [p.rearrange("p a b -> p (a b)") for p in padA]
        padBf = [p.rearrange("p a b -> p (a b)") for p in padB]
        tailb = [sb(f"tail{g}", [128, 26, W], bf16) for g in range(3)]
        diag = [sb(f"diag{g}", [128, 9, 128], bf16) for g in range(3)]
        pooled = [sb(f"pooled{g}", [128, 9], f32) for g in range(3)]
        avgs = [sb(f"avgs{g}", [128, 1], f32) for g in range(3)]
        mx = [sb(f"mx{g}", [128, 1], f32) for g in range(3)]
        th = [sb(f"theta{g}", [128, 1], f32) for g in range(3)]
        w9 = [sb(f"w9_{g}", [128, 9], f32) for g in range(3)]
        w4p = [sb(f"w4p{g}", [128, 1], f32) for g in range(3)]
        wsum9 = [sb(f"wsum9_{g}", [128, 1], f32) for g in range(3)]
        adkT_sb = sb("adkT_sb", [128, 108], f32)
        adkT = [adkT_sb[:, g * 36:(g + 1) * 36] for g in range(3)]
        lvmax = [sb(f"lvmax{g}", [128, 12, WP], bf16) for g in range(3)]

        eye = sb("eye_sb", [128, 128], bf16)
        pk128 = sb("pk128_sb", [128, 6 * R], f32)
        w1avg_a = pk128[:, 0:R]
        w1avg_b = pk128[:, R:2 * R]
        w1mx_a = pk128[:, 2 * R:3 * R]
        w1mx_b = pk128[:, 3 * R:4 * R]
        p1a = pk128[:, 4 * R:5 * R]
        p1b = pk128[:, 5 * R:6 * R]
        pk48 = sb("pk48_sb", [R, 962], f32)
        w2t = pk48[:, 0:C]
        w2s = pk48[:, C:C + G * C]
        bns = pk48[:, 960:961]
        bnb = pk48[:, 961:962]

        h_adk = [sb(f"h_adk{b}", [R, 9], f32) for b in range(2)]
        hsum = [sb(f"hsum{b}", [R, 1], f32) for b in range(2)]

        scr = ctx.enter_context(tc.tile_pool(name="scr", bufs=4))
        treep = ctx.enter_context(tc.tile_pool(name="treep", bufs=2))
        sclp = ctx.enter_context(tc.tile_pool(name="sclp", bufs=2))
        term_pool = ctx.enter_context(tc.tile_pool(name="terms", bufs=3))
        osb_pool = ctx.enter_context(tc.tile_pool(name="osbp", bufs=6))
        ct_pool = ctx.enter_context(tc.tile_pool(name="ctp", bufs=2))
        psA = ctx.enter_context(tc.tile_pool(name="psA", bufs=1, space="PSUM"))
        psB = ctx.enter_context(tc.tile_pool(name="psB", bufs=1, space="PSUM"))
        # stats bank: single-shot matmul groups only may share a bank
        stpa = nc.alloc_psum_tensor("statps", [128, 512], f32).ap()

        # ---------------- DMA emission ----------------
        row_chunks = [(0, 33), (33, 65), (65, HP)]

        def xA_chunk(eng, g, ci):
            r0, r1 = row_chunks[ci]
            eng.dma_start(
                out=padA[g][:, r0:r1, :],
                in_=xA_d[g * 128:(g + 1) * 128, r0:r1, :],
            )

        def emit_xB_dma(g):
            nc.scalar.dma_start(
                out=padB[g][:, :, :],
                in_=xB_d[g * 128:(g + 1) * 128, :, :],
            )

        # g0 then g1 serialized on the sync queue; weights then g2 on the
        # gpsimd queue (parallel to sync)
        for ci in range(3):
            xA_chunk(nc.sync, 0, ci)
        for ci in range(3):
            xA_chunk(nc.gpsimd, 2, ci)
        nc.gpsimd.dma_start(out=pk128, in_=pk128_d)
        nc.gpsimd.dma_start(out=pk48, in_=pk48_d)
        nc.gpsimd.dma_start(out=adkT_sb, in_=adkT_d)
        nc.gpsimd.dma_start(out=eye, in_=eye_d)
        for ci in range(3):
            xA_chunk(nc.sync, 1, ci)

        # ---------------- stats ----------------
        def sums_scl(g, k):
            # 3 window sums of band k via ScalarE ACT accumulate
            for j in range(3):
                win = padA[g][:, 1 + 32 * k:33 + 32 * k, 2 + 32 * j:34 + 32 * j]
                acc = pooled[g][:, 3 * k + j:3 * k + j + 1]
                s = sclp.tile([128, 32, 32], bf16, tag="wscr", name=f"w{g}_{k}_{j}")
                nc.scalar.activation(out=s[:, :, :], in_=win,
                                     func=ACTF.Copy, accum_out=acc)

        def sums_tree(g, k):
            # DVE: bf16 TT add-tree 32->16->8->4 rows (2x mode), then 3
            # cache-reduce window sums over the 4 leaf rows
            p = padA[g]
            r0 = 1 + 32 * k
            t16 = treep.tile([128, 16, WP], bf16, tag="tr16", name=f"s16_{g}_{k}")
            nc.vector.tensor_add(t16[:, :, :], p[:, r0:r0 + 16, :], p[:, r0 + 16:r0 + 32, :])
            t8 = treep.tile([128, 8, WP], bf16, tag="tr8", name=f"s8_{g}_{k}")
            nc.vector.tensor_add(t8[:, :, :], t16[:, 0:8, :], t16[:, 8:16, :])
            t4 = treep.tile([128, 4, WP], bf16, tag="tr4", name=f"s4_{g}_{k}")
            nc.vector.tensor_add(t4[:, :, :], t8[:, 0:4, :], t8[:, 4:8, :])
            for j in range(3):
                acc = pooled[g][:, 3 * k + j:3 * k + j + 1]
                s = treep.tile([128, 4, 32], bf16, tag="wscr4", name=f"w{g}_{k}_{j}")
                nc.vector.tensor_scalar(s[:, :, :],
                                        t4[:, :, 2 + 32 * j:34 + 32 * j],
                                        1.0, None,
                                        op0=ALU.mult, op1=ALU.add, accum_out=acc)

        def band_max(g, k):
            # DVE: bf16 TT max-tree 32->16->8->4 rows into lvmax
            p = padA[g]
            r0 = 1 + 32 * k
            t16 = treep.tile([128, 16, WP], bf16, tag="tr16", name=f"m16_{g}_{k}")
            nc.vector.tensor_tensor(out=t16[:, :, :], in0=p[:, r0:r0 + 16, :],
                                    in1=p[:, r0 + 16:r0 + 32, :], op=ALU.max)
            t8 = treep.tile([128, 8, WP], bf16, tag="tr8", name=f"m8_{g}_{k}")
            nc.vector.tensor_tensor(out=t8[:, :, :], in0=t16[:, 0:8, :],
                                    in1=t16[:, 8:16, :], op=ALU.max)
            nc.vector.tensor_tensor(out=lvmax[g][:, 4 * k:4 * k + 4, :],
                                    in0=t8[:, 0:4, :], in1=t8[:, 4:8, :], op=ALU.max)

        def mx_fin(g):
            t6 = treep.tile([128, 6, WP], bf16, tag="tr6", name=f"mf6_{g}")
            nc.vector.tensor_tensor(out=t6[:, :, :], in0=lvmax[g][:, 0:6, :],
                                    in1=lvmax[g][:, 6:12, :], op=ALU.max)
            t3 = treep.tile([128, 3, WP], bf16, tag="tr3", name=f"mf3_{g}")
            nc.vector.tensor_tensor(out=t3[:, :, :], in0=t6[:, 0:3, :],
                                    in1=t6[:, 3:6, :], op=ALU.max)
            nc.vector.tensor_reduce(out=mx[g][:, :], in_=t3[:, :, :],
                                    axis=mybir.AxisListType.XY, op=ALU.max)

        def emit_avg_fin(g):
            asc = scr.tile([128, 9], bf16, tag="ascr", name=f"avg{g}")
            nc.scalar.activation(out=asc[:, :], in_=pooled[g][:, :],
                                 func=ACTF.Copy, accum_out=avgs[g][:, :])

        # ---------------- per-sample algebra (band-incremental) ----------------
        def sample_chunks(b):
            if b == 0:
                return [
                    (w1avg_a[:, :], w1mx_a[:, :], p1a[:, :], (0, 0, 128)),
                    (w1avg_b[0:64, :], w1mx_b[0:64, :], p1b[0:64, :], (2, 0, 64)),
                ]
            return [
                (w1avg_a[:, :], w1mx_a[:, :], p1a[:, :], (1, 0, 128)),
                (w1avg_b[64:128, :], w1mx_b[64:128, :], p1b[64:128, :], (2, 64, 128)),
            ]

        def emit_sample_pool_band(b, k):
            base = 8 + b * 22
            for i, (wa, wm, wp, (sg, q0, q1)) in enumerate(sample_chunks(b)):
                o = base + 11 * i
                nc.tensor.matmul(stpa[0:R, o + 2 + 3 * k:o + 5 + 3 * k], lhsT=wp,
                                 rhs=pooled[sg][q0:q1, 3 * k:3 * k + 3],
                                 start=True, stop=True)

        def emit_sample_theta(b):
            base = 8 + b * 22
            for i, (wa, wm, wp, (sg, q0, q1)) in enumerate(sample_chunks(b)):
                o = base + 11 * i
                nc.tensor.matmul(stpa[0:R, o:o + 1], lhsT=wa, rhs=avgs[sg][q0:q1, :], start=True, stop=True)
                nc.tensor.matmul(stpa[0:R, o + 1:o + 2], lhsT=wm, rhs=mx[sg][q0:q1, :], start=True, stop=True)

        def emit_fold_pool_band(b, k):
            base = 8 + b * 22
            hc = scr.tile([R, 3], f32, tag="scr3", name=f"hc{b}_{k}")
            nc.vector.tensor_copy(hc[:, :], stpa[0:R, base + 2 + 3 * k:base + 5 + 3 * k])
            hs = scr.tile([R, 3], f32, tag="scr3", name=f"hs{b}_{k}")
            nc.vector.tensor_add(hs[:, :], hc[:, :],
                                 stpa[0:R, base + 13 + 3 * k:base + 16 + 3 * k])
            t1 = scr.tile([R, 3], f32, tag="scr3", name=f"bn{b}_{k}")
            nc.vector.tensor_scalar(t1[:, :], hs[:, :], bns[:, :], bnb[:, :],
                                    op0=ALU.mult, op1=ALU.add)
            nc.vector.tensor_scalar_max(h_adk[b][:, 3 * k:3 * k + 3], t1[:, :], 0.0)

        def emit_fold_theta(b):
            base = 8 + b * 22
            hg = scr.tile([R, 2], f32, tag="scr2", name=f"hg{b}")
            nc.vector.tensor_copy(hg[:, :], stpa[0:R, base:base + 2])
            hs = scr.tile([R, 2], f32, tag="scr2", name=f"ht{b}")
            nc.vector.tensor_add(hs[:, :], hg[:, :],
                                 stpa[0:R, base + 11:base + 13])
            ha = scr.tile([R, 1], f32, tag="scr1", name=f"ha{b}")
            hm = scr.tile([R, 1], f32, tag="scr1", name=f"hm{b}")
            nc.vector.tensor_scalar_max(ha[:, :], hs[:, 0:1], 0.0)
            nc.vector.tensor_scalar_max(hm[:, :], hs[:, 1:2], 0.0)
            nc.vector.tensor_add(hsum[b][:, :], ha[:, :], hm[:, :])

        # ---------------- theta ----------------
        ps_t = [stpa[:, i:i + 1] for i in range(3)]

        def emit_theta_mm(b):
            nc.tensor.matmul(ps_t[b], lhsT=w2t[:, 0:128], rhs=hsum[b][:, :], start=True, stop=True)
            q0, q1 = (0, 64) if b == 0 else (64, 128)
            nc.tensor.matmul(ps_t[2][q0:q1], lhsT=w2t[:, 128:192], rhs=hsum[b][:, :], start=True, stop=True)

        def emit_theta_fin(g):
            et = scr.tile([128, 1], f32, tag="scr1", name=f"et{g}")
            nc.scalar.activation(out=et[:, :], in_=ps_t[g], func=ACTF.Exp, scale=-1.0)
            d = scr.tile([128, 1], f32, tag="scr1", name=f"etd{g}")
            nc.vector.tensor_scalar_add(d[:, :], et[:, :], 1.0)
            nc.vector.reciprocal(th[g][:, :], d[:, :])

        # ---------------- dynamic kernels w9 (band-major layout) ----------------
        # ps_s columns: 12*k + 3*gg + j ; adkT host layout matches.
        def emit_w9_mm_band(g, k):
            ps_s = stpa[:, 64 + g * 36:64 + (g + 1) * 36]
            for gg in range(G):
                sl = slice(12 * k + 3 * gg, 12 * k + 3 * gg + 3)
                if g < 2:
                    nc.tensor.matmul(ps_s[:, sl], lhsT=w2s[:, gg * 192:gg * 192 + 128],
                                     rhs=h_adk[g][:, 3 * k:3 * k + 3], start=True, stop=True)
                else:
                    nc.tensor.matmul(ps_s[0:64, sl], lhsT=w2s[:, gg * 192 + 128:gg * 192 + 192],
                                     rhs=h_adk[0][:, 3 * k:3 * k + 3], start=True, stop=True)
                    nc.tensor.matmul(ps_s[64:128, sl], lhsT=w2s[:, gg * 192 + 128:gg * 192 + 192],
                                     rhs=h_adk[1][:, 3 * k:3 * k + 3], start=True, stop=True)

        def emit_w9_exp_band(g, k):
            ps_s = stpa[:, 64 + g * 36 + 12 * k:64 + g * 36 + 12 * k + 12]
            e = scr.tile([128, 12], f32, tag="scr12", name=f"e{g}_{k}")
            nc.scalar.activation(out=e[:, :], in_=ps_s, func=ACTF.Exp)
            return e

        def emit_w9_exp_all(g):
            ps_s = stpa[:, 64 + g * 36:64 + (g + 1) * 36]
            e = scr.tile([128, 36], f32, tag="scr36", name=f"eall{g}")
            nc.scalar.activation(out=e[:, :], in_=ps_s, func=ACTF.Exp)
            return e

        def emit_w9_fin_band(g, k, e):
            # e: [128, 12] (4 groups x 3 cols) for band k
            d1 = scr.tile([128, 3], f32, tag="scr3b", name=f"d1_{g}_{k}")
            d2 = scr.tile([128, 3], f32, tag="scr3b", name=f"d2_{g}_{k}")
            nc.vector.tensor_add(d1[:, :], e[:, 0:3], e[:, 3:6])
            nc.vector.tensor_add(d2[:, :], e[:, 6:9], e[:, 9:12])
            nc.vector.tensor_add(d1[:, :], d1[:, :], d2[:, :])
            rec = scr.tile([128, 3], f32, tag="scr3b", name=f"rec{g}_{k}")
            nc.vector.reciprocal(rec[:, :], d1[:, :])
            a = adkT[g][:, 12 * k:12 * k + 12]
            m1 = scr.tile([128, 3], f32, tag="scr3b", name=f"m1_{g}_{k}")
            m2 = scr.tile([128, 3], f32, tag="scr3b", name=f"m2_{g}_{k}")
            nc.vector.tensor_mul(m1[:, :], e[:, 0:3], a[:, 0:3])
            nc.vector.tensor_mul(m2[:, :], e[:, 3:6], a[:, 3:6])
            nc.vector.tensor_add(m1[:, :], m1[:, :], m2[:, :])
            nc.vector.tensor_mul(m2[:, :], e[:, 6:9], a[:, 6:9])
            nc.vector.tensor_add(m1[:, :], m1[:, :], m2[:, :])
            nc.vector.tensor_mul(m2[:, :], e[:, 9:12], a[:, 9:12])
            nc.vector.tensor_add(m1[:, :], m1[:, :], m2[:, :])
            nc.vector.tensor_mul(w9[g][:, 3 * k:3 * k + 3], m1[:, :], rec[:, :])

        def emit_wsum9(g):
            nc.vector.tensor_reduce(out=wsum9[g][:, :], in_=w9[g][:, :],
                                    axis=mybir.AxisListType.X, op=ALU.add)

        def emit_w4p(g):
            t1 = scr.tile([128, 1], f32, tag="scr1", name=f"t1_{g}")
            nc.vector.tensor_mul(t1[:, :], w9[g][:, 4:5], th[g][:, :])
            nc.vector.tensor_add(t1[:, :], t1[:, :], w9[g][:, 4:5])
            nc.vector.tensor_sub(w4p[g][:, :], t1[:, :], wsum9[g][:, :])

        def emit_diag(g, taps, engine):
            for tap in taps:
                scal = w4p[g][:, 0:1] if tap == 4 else w9[g][:, tap:tap + 1]
                if engine == "vector":
                    nc.vector.tensor_scalar_mul(diag[g][:, tap, :], eye[:, :], scal)
                else:
                    nc.scalar.activation(out=diag[g][:, tap, :], in_=eye[:, :],
                                         func=ACTF.Copy, scale=scal)

        # ---------------- conv on PE ----------------
        win_r0 = {}
        win_pool = {}
        gidx = 0
        for g in range(3):
            r = 0
            for w, nb in enumerate(WIN_SEQ[g]):
                win_r0[(g, w)] = r
                win_pool[(g, w)] = gidx % 2  # 0 -> psA(4), 1 -> psB(3)
                r += 5 * nb
                gidx += 1

        pools = {0: psA, 1: psB}
        win_tile = {}

        def conv_taps(g, w, taps, first=True, last=True):
            nb = WIN_SEQ[g][w]
            r0 = win_r0[(g, w)]
            key = (g, w)
            if key not in win_tile:
                pi = win_pool[key]
                pnb = 4 if pi == 0 else 3
                win_tile[key] = pools[pi].tile(
                    [128, pnb, 512], f32, tag=f"w{pnb}", name=f"ps{g}_{w}")
            ps = win_tile[key]
            for tap in taps:
                dy, dx = divmod(tap, 3)
                for b in range(nb):
                    y0 = r0 + 5 * b + dy
                    nc.tensor.matmul(
                        ps[:, b, 0:480],
                        lhsT=diag[g][:, tap, :],
                        rhs=padA[g][:, y0:y0 + 5, dx + 1:dx + 97],
                        start=(first and tap == taps[0]),
                        stop=(last and tap == taps[-1]),
                    )

        def conv_drain_copy(g, w):
            nb = WIN_SEQ[g][w]
            ps = win_tile.pop((g, w))
            nr = 5 * nb
            ot = osb_pool.tile([128, 20, W], bf16, tag="ow", name=f"ow{g}_{w}")
            nc.scalar.activation(
                out=ot[:, 0:nr, :],
                in_=ps[:, 0:nb, 0:480], func=ACTF.Copy)
            return ot

        def conv_out_dma(g, w, ot):
            nb = WIN_SEQ[g][w]
            r0 = win_r0[(g, w)]
            nr = 5 * nb
            nc.sync.dma_start(
                out=out_d[g * 128:(g + 1) * 128, r0:r0 + nr, :],
                in_=ot[:, 0:nr, :])

        def conv_center_dma(g, w, ot):
            nb = WIN_SEQ[g][w]
            r0 = win_r0[(g, w)]
            nr = 5 * nb
            tm = ct_pool.tile([128, 20, W], bf16, tag="ct", name=f"ct{g}_{w}")
            nc.vector.tensor_scalar_mul(
                tm[:, 0:nr, :], padA[g][:, r0 + 1:r0 + nr + 1, 2:98], w4p[g][:, 0:1])
            nc.vector.tensor_add(ot[:, 0:nr, :], ot[:, 0:nr, :], tm[:, 0:nr, :])
            conv_out_dma(g, w, ot)

        def conv_drain(g, w):
            conv_out_dma(g, w, conv_drain_copy(g, w))

        # ---------------- conv tail on DVE (flat strips) ----------------
        def emit_conv_dve(g, lo, hi):
            y0 = ROWS_PE[g] + lo
            n = hi - lo
            L = (n - 1) * 100 + 96
            acc = None
            for i, tap in enumerate(TAP_ORDER):
                dy, dx = divmod(tap, 3)
                scal = w4p[g][:, 0:1] if tap == 4 else w9[g][:, tap:tap + 1]
                if dx == 1:
                    src = padAf[g][:, (y0 + dy) * 100 + 2:(y0 + dy) * 100 + 2 + L]
                else:
                    o0 = (y0 + dy - XB_R0) * 100 + (2 if dx == 0 else 4)
                    src = padBf[g][:, o0:o0 + L]
                t = term_pool.tile([128, 16, 100], bf16, tag="term",
                                   name=f"t{g}_{lo}_{i}")
                tf = t.rearrange("p a b -> p (a b)")
                nc.vector.tensor_scalar_mul(tf[:, 0:L], src, scal)
                if i == 0:
                    acc = t
                elif i < 8:
                    nxt = term_pool.tile([128, 16, 100], bf16, tag="term",
                                         name=f"a{g}_{lo}_{i}")
                    nxf = nxt.rearrange("p a b -> p (a b)")
                    nc.vector.tensor_add(nxf[:, 0:L], acc.rearrange("p a b -> p (a b)")[:, 0:L], tf[:, 0:L])
                    acc = nxt
                else:
                    nc.vector.tensor_add(tailb[g][:, lo:hi, :],
                                         acc[:, 0:n, 0:96], t[:, 0:n, 0:96])
            nc.sync.dma_start(out=out_d[g * 128:(g + 1) * 128, y0:y0 + n, :],
                              in_=tailb[g][:, lo:hi, :])

        # ---------------- PE warm-up ----------------
        def emit_warmup(k, gate, drain=False):
            for j in range(k):
                nc.tensor.matmul(stpa[:, 384:512], lhsT=eye[:, :],
                                 rhs=gate, start=True, stop=True)
            if drain:
                wsc = scr.tile([128, 1], bf16, tag="wscr1", name="wscr")
                nc.scalar.activation(out=wsc[:, :], in_=stpa[:, 384:385], func=ACTF.Copy)
                nc.sync.dma_start(out=warm_d, in_=wsc[:, :])

        # ================ emission order ================
        # PE warmups gated on arriving chunks (g0 via sync, g2 via vector)
        emit_warmup(12, padA[0][:, 10:12, 0:64])
        emit_warmup(12, padA[2][:, 10:12, 0:64])
        emit_warmup(12, padA[0][:, 40:42, 0:64])
        emit_warmup(10, padA[2][:, 40:42, 0:64], drain=True)

        # band 0: stats + algebra chain + first conv wave
        sums_scl(2, 0)              # SCL
        sums_tree(0, 0)             # DVE
        emit_sample_pool_band(0, 0)  # PE
        emit_fold_pool_band(0, 0)   # DVE
        emit_w9_mm_band(0, 0)       # PE
        e00 = emit_w9_exp_band(0, 0)  # SCL
        emit_w9_fin_band(0, 0, e00)  # DVE
        emit_diag(0, BAND_TAPS[0], "vector")
        conv_taps(0, 0, BAND_TAPS[0], first=True, last=False)
        conv_taps(0, 1, BAND_TAPS[0], first=True, last=False)

        # band 1
        sums_scl(2, 1)
        sums_tree(0, 1)
        emit_sample_pool_band(0, 1)
        emit_fold_pool_band(0, 1)
        emit_w9_mm_band(0, 1)
        e01 = emit_w9_exp_band(0, 1)
        emit_w9_fin_band(0, 1, e01)
        emit_diag(0, BAND_TAPS[1], "vector")
        conv_taps(0, 0, BAND_TAPS[1], first=False, last=False)
        conv_taps(0, 1, BAND_TAPS[1], first=False, last=False)

        # band 2
        sums_scl(2, 2)
        sums_tree(0, 2)
        emit_sample_pool_band(0, 2)
        emit_fold_pool_band(0, 2)
        emit_w9_mm_band(0, 2)
        e02 = emit_w9_exp_band(0, 2)
        emit_w9_fin_band(0, 2, e02)
        emit_diag(0, BAND_TAPS[2], "vector")
        conv_taps(0, 0, BAND_TAPS[2], first=False, last=True)
        conv_taps(0, 1, BAND_TAPS[2], first=False, last=True)

        # theta(0) path: maxes on DVE, avgfins on SCL
        band_max(0, 0)
        band_max(0, 1)
        band_max(0, 2)
        mx_fin(0)
        band_max(2, 0)
        band_max(2, 1)
        band_max(2, 2)
        mx_fin(2)
        emit_avg_fin(0)             # SCL
        emit_avg_fin(2)             # SCL
        emit_sample_theta(0)        # PE
        emit_fold_theta(0)          # DVE
        emit_theta_mm(0)            # PE
        ot00 = conv_drain_copy(0, 0)  # SCL (frees PSUM A)
        ot01 = conv_drain_copy(0, 1)  # SCL (frees PSUM B)
        emit_theta_fin(0)           # SCL exp + DVE
        emit_wsum9(0)               # DVE
        emit_w4p(0)                 # DVE
        emit_diag(0, [4], "vector")

        conv_taps(0, 2, TAP_ORDER)

        conv_center_dma(0, 0, ot00)  # DVE + DMA
        conv_center_dma(0, 1, ot01)

        # g1 stats on SCL (gated on g1 DMA), xB kicks after theta exp
        sums_scl(1, 0)
        sums_scl(1, 1)
        emit_xB_dma(0)
        emit_xB_dma(1)
        emit_xB_dma(2)
        sums_scl(1, 2)
        emit_avg_fin(1)

        band_max(1, 0)              # DVE
        band_max(1, 1)
        band_max(1, 2)
        mx_fin(1)

        conv_drain(0, 2)
        conv_taps(0, 3, TAP_ORDER)

        # sample-1 algebra
        for k in range(3):
            emit_sample_pool_band(1, k)   # PE
        for k in range(3):
            emit_fold_pool_band(1, k)     # DVE
        emit_sample_theta(1)              # PE
        emit_fold_theta(1)                # DVE
        emit_theta_mm(1)                  # PE
        for k in range(3):
            emit_w9_mm_band(1, k)         # PE
        for k in range(3):
            emit_w9_mm_band(2, k)         # PE
        e1 = emit_w9_exp_all(1)           # SCL
        e2 = emit_w9_exp_all(2)           # SCL
        emit_theta_fin(1)                 # SCL + DVE
        for k in range(3):
            emit_w9_fin_band(1, k, e1[:, 12 * k:12 * k + 12])
        emit_wsum9(1)
        emit_w4p(1)
        emit_diag(1, TAP_ORDER, "scalar")
        emit_theta_fin(2)
        for k in range(3):
            emit_w9_fin_band(2, k, e2[:, 12 * k:12 * k + 12])
        emit_wsum9(2)
        emit_w4p(2)
        emit_diag(2, TAP_ORDER, "scalar")

        conv_drain(0, 3)
        conv_taps(1, 0, TAP_ORDER)
        emit_conv_dve(0, *TAIL_CHUNKS[0][0])
        conv_drain(1, 0)
        conv_taps(1, 1, TAP_ORDER)
        emit_conv_dve(0, *TAIL_CHUNKS[0][1])
        conv_drain(1, 1)
        conv_taps(1, 2, TAP_ORDER)
        emit_conv_dve(1, *TAIL_CHUNKS[1][0])
        conv_drain(1, 2)
        conv_taps(1, 3, TAP_ORDER)
        emit_conv_dve(2, *TAIL_CHUNKS[2][0])
        conv_drain(1, 3)
        conv_taps(1, 4, TAP_ORDER)
        emit_conv_dve(1, *TAIL_CHUNKS[1][1])
        conv_drain(1, 4)
        emit_conv_dve(2, *TAIL_CHUNKS[2][1])
        conv_taps(2, 0, TAP_ORDER)
        conv_drain(2, 0)
        conv_taps(2, 1, TAP_ORDER)
        conv_drain(2, 1)
        conv_taps(2, 2, TAP_ORDER)
        conv_drain(2, 2)
        conv_taps(2, 3, TAP_ORDER)
        conv_drain(2, 3)

    nc.compile()
    return nc


def _host_prep(inputs):
    x = np.ascontiguousarray(inputs["x"], dtype=np.float32)
    cam_w1 = np.asarray(inputs["cam_w1"], dtype=np.float32)
    cam_w2 = np.asarray(inputs["cam_w2"], dtype=np.float32)
    proj_w1 = np.asarray(inputs["proj_w1"], dtype=np.float32)
    bn_gamma = np.asarray(inputs["bn_gamma"], dtype=np.float32)
    bn_beta = np.asarray(inputs["bn_beta"], dtype=np.float32)
    proj_w2 = np.asarray(inputs["proj_w2"], dtype=np.float32)
    adk = np.asarray(inputs["adk_weight"], dtype=np.float32)

    xb16 = x.astype(BF16)
    xpA = np.zeros((B, C, HP, WP), dtype=BF16)
    xpA[:, :, 1:97, 2:98] = xb16
    # padB: x payload at col 3, rows = padded rows 63..96 (x rows 62..95)
    xpB = np.zeros((B, C, XB_NR, WP), dtype=BF16)
    xpB[:, :, 0:34, 3:99] = xb16[:, :, 62:96, :]

    in_maps = []
    w1t = cam_w1.T.astype(np.float32)
    p1t = (proj_w1.T / 1024.0).astype(np.float32)
    cmap = np.concatenate([np.arange(128), np.arange(128),
                           np.arange(128, 192), np.arange(128, 192)])
    # adkT band-major: col = dy*12 + gg*3 + dx  (from adk[gg, c, dy, dx])
    adk_bm = adk.transpose(2, 0, 3, 1).reshape(36, C).T  # [C, 36]
    pk128 = np.concatenate([
        w1t[0:128] / (H * W),
        np.concatenate([w1t[128:192] / (H * W)] * 2, axis=0),
        w1t[0:128],
        np.concatenate([w1t[128:192]] * 2, axis=0),
        p1t[0:128],
        np.concatenate([p1t[128:192]] * 2, axis=0),
    ], axis=1).astype(np.float32)
    pk48 = np.concatenate([
        cam_w2.T,
        proj_w2.T,
        (bn_gamma / np.sqrt(1.0 + BN_EPS)).reshape(R, 1),
        bn_beta.reshape(R, 1),
    ], axis=1).astype(np.float32)
    adkT_full = adk_bm[cmap].astype(np.float32)  # [384, 36]
    adkT_pk = np.concatenate([adkT_full[0:128], adkT_full[128:256],
                              adkT_full[256:384]], axis=1)  # [128, 108]
    consts = {
        "eye": np.eye(128, dtype=BF16),
        "pk128": np.ascontiguousarray(pk128),
        "pk48": np.ascontiguousarray(pk48),
        "adkT": np.ascontiguousarray(adkT_pk),
    }
    for k in range(N_CORES):
        b0, b1 = 2 * k, 2 * k + 1
        shardA = np.ascontiguousarray(np.concatenate(
            [xpA[b0, 0:128], xpA[b1, 0:128], xpA[b0, 128:192], xpA[b1, 128:192]],
            axis=0))
        shardB = np.ascontiguousarray(np.concatenate(
            [xpB[b0, 0:128], xpB[b1, 0:128], xpB[b0, 128:192], xpB[b1, 128:192]],
            axis=0))
        m = {"xA": shardA, "xB": shardB}
        m.update(consts)
        in_maps.append(m)
    return in_maps


def kernel(**inputs) -> np.ndarray:
    global _COMPILED
    from concourse.bass_utils import run_bass_kernel_spmd

    in_maps = _host_prep(inputs)

    if _COMPILED is None:
        _COMPILED = _build()
    nc = _COMPILED

    res = run_bass_kernel_spmd(nc, in_maps, core_ids=list(range(N_CORES)))
    outs = [r["out"] for r in res.results]

    y = np.empty((B, C, H, W), np.float32)
    for k in range(N_CORES):
        o = np.asarray(outs[k]).reshape(384, H, W).astype(np.float32)
        b0, b1 = 2 * k, 2 * k + 1
        y[b0, 0:128] = o[0:128]
        y[b1, 0:128] = o[128:256]
        y[b0, 128:192] = o[256:320]
        y[b1, 128:192] = o[320:384]
    return y


if __name__ == "__main__":
    import reference

    inputs = {k: np.asarray(v) for k, v in reference.setup_inputs().items()}
    y = kernel(**inputs)
    print("kernel output:", y.shape, y.dtype)


# revision 13
# speedup vs baseline: 1.1832x; 1.0419x over previous
"""Trainium2 Bass kernel for nn_ADSCDConv (dense_cnn), 8-core data parallel.

Per core (2 samples = 384 (b,c) channel-images of 96x96), groups of 128
partitions: g0=(b0,c0:128), g1=(b1,c0:128), g2=(b0,c128:192)||(b1,c128:192).

v4 schedule (vs v2 baseline):
  - center tap on PE for all windows except (0,0)/(0,1); drains split
    into a PSUM-freeing ScalarE copy and a (theta-gated) DVE center
    add + DMA so PSUM recycling never waits on theta.
  - incremental per-band algebra: pooled band k feeds h/w9/diag for
    tap row dy=k only, so windows (0,0)/(0,1) start their dy=0 taps
    as soon as band 0 of g0+g2 has landed.
  - stats via the v2-proven DVE trees (g0) + ScalarE ACT-accum (g2,
    g1); w9 PSUM/adkT laid out band-major.
  - DVE tail taps read flat contiguous strips (full padded rows) so
    the muls hit the 4x DVE mode; only the final add is strided.
  - DMA: g0+g1 serialized on the sync queue, g2 parallel on the
    vector queue, weights on gpsimd, xB kicked mid-stream from the
    ACT queue.
  - PE rows 70/80/70, DVE tail rows 26/16/26.
"""

from contextlib import ExitStack

import numpy as np
import ml_dtypes

BF16 = ml_dtypes.bfloat16

B, C, H, W = 16, 192, 96, 96
G = 4
R = C // 4  # 48
BN_EPS = 1e-5
N_CORES = 8
HP, WP = H + 2, 100  # padded rows 98, padded cols 100 (x payload at col 2)
XB_R0 = 63           # padB covers padded rows 63..97
XB_NR = 35

# conv windows: banks per window; PSUM pool A(4)/B(3) alternates by
# GLOBAL window index across groups
WIN_SEQ = {0: [4, 3, 4, 3], 1: [4, 3, 4, 3, 1], 2: [3, 4, 3, 4]}
ROWS_PE = {g: 5 * sum(WIN_SEQ[g]) for g in range(3)}  # 70, 75, 70

TAIL_CHUNKS = {0: [(0, 13), (13, 26)],
               1: [(0, 11), (11, 21)],
               2: [(0, 13), (13, 26)]}

# tap order: center (tap 4, the only theta-dependent tap) last
TAP_ORDER = [0, 1, 2, 3, 5, 6, 7, 8, 4]
NC8 = TAP_ORDER[:8]
BAND_TAPS = {0: [0, 1, 2], 1: [3, 5], 2: [6, 7, 8]}

_COMPILED = None


def _build():
    import concourse.tile as tile
    from concourse import bacc, mybir

    f32 = mybir.dt.float32
    bf16 = mybir.dt.bfloat16
    ALU = mybir.AluOpType
    ACTF = mybir.ActivationFunctionType

    nc = bacc.Bacc("TRN2", target_bir_lowering=False, debug=False, num_devices=N_CORES)

    # ---- DRAM tensors ----
    xA_d = nc.dram_tensor("xA", [384, HP, WP], bf16, kind="ExternalInput").ap()
    xB_d = nc.dram_tensor("xB", [384, XB_NR, WP], bf16, kind="ExternalInput").ap()
    out_d = nc.dram_tensor("out", [384, H, W], bf16, kind="ExternalOutput").ap()
    warm_d = nc.dram_tensor("warm", [128, 1], bf16, kind="ExternalOutput").ap()
    eye_d = nc.dram_tensor("eye", [128, 128], bf16, kind="ExternalInput").ap()
    pk128_d = nc.dram_tensor("pk128", [128, 6 * R], f32, kind="ExternalInput").ap()
    pk48_d = nc.dram_tensor("pk48", [R, 962], f32, kind="ExternalInput").ap()
    adkT_d = nc.dram_tensor("adkT", [128, 108], f32, kind="ExternalInput").ap()

    with tile.TileContext(nc) as tc, ExitStack() as ctx:
        def sb(name, shape, dt):
            return nc.alloc_sbuf_tensor(name, shape, dt).ap()

        padA = [sb(f"padA{g}", [128, HP, WP], bf16) for g in range(3)]
        padB = [sb(f"padB{g}", [128, XB_NR, WP], bf16) for g in range(3)]
        padAf = [p.rearrange("p a b -> p (a b)") for p in padA]
        padBf = [p.rearrange("p a b -> p (a b)") for p in padB]
        tailb = [sb(f"tail{g}", [128, 26, W], bf16) for g in range(3)]
        diag = [sb(f"diag{g}", [128, 9, 128], bf16) for g in range(3)]
        pooled = [sb(f"pooled{g}", [128, 9], f32) for g in range(3)]
        avgs = [sb(f"avgs{g}", [128, 1], f32) for g in range(3)]
        mx = [sb(f"mx{g}", [128, 1], f32) for g in range(3)]
        th = [sb(f"theta{g}", [128, 1], f32) for g in range(3)]
        w9 = [sb(f"w9_{g}", [128, 9], f32) for g in range(3)]
        w4p = [sb(f"w4p{g}", [128, 1], f32) for g in range(3)]
        wsum9 = [sb(f"wsum9_{g}", [128, 1], f32) for g in range(3)]
        adkT_sb = sb("adkT_sb", [128, 108], f32)
        adkT = [adkT_sb[:, g * 36:(g + 1) * 36] for g in range(3)]
        lvmax = [sb(f"lvmax{g}", [128, 12, WP], bf16) for g in range(3)]

        eye = sb("eye_sb", [128, 128], bf16)
        pk128 = sb("pk128_sb", [128, 6 * R], f32)
        w1avg_a = pk128[:, 0:R]
        w1avg_b = pk128[:, R:2 * R]
        w1mx_a = pk128[:, 2 * R:3 * R]
        w1mx_b = pk128[:, 3 * R:4 * R]
        p1a = pk128[:, 4 * R:5 * R]
        p1b = pk128[:, 5 * R:6 * R]
        pk48 = sb("pk48_sb", [R, 962], f32)
        w2t = pk48[:, 0:C]
        w2s = pk48[:, C:C + G * C]
        bns = pk48[:, 960:961]
        bnb = pk48[:, 961:962]

        h_adk = [sb(f"h_adk{b}", [R, 9], f32) for b in range(2)]
        hsum = [sb(f"hsum{b}", [R, 1], f32) for b in range(2)]

        scr = ctx.enter_context(tc.tile_pool(name="scr", bufs=4))
        treep = ctx.enter_context(tc.tile_pool(name="treep", bufs=2))
        sclp = ctx.enter_context(tc.tile_pool(name="sclp", bufs=2))
        term_pool = ctx.enter_context(tc.tile_pool(name="terms", bufs=3))
        osb_pool = ctx.enter_context(tc.tile_pool(name="osbp", bufs=6))
        ct_pool = ctx.enter_context(tc.tile_pool(name="ctp", bufs=2))
        psA = ctx.enter_context(tc.tile_pool(name="psA", bufs=1, space="PSUM"))
        psB = ctx.enter_context(tc.tile_pool(name="psB", bufs=1, space="PSUM"))
        # stats bank: single-shot matmul groups only may share a bank
        stpa = nc.alloc_psum_tensor("statps", [128, 512], f32).ap()

        # ---------------- DMA emission ----------------
        row_chunks = [(0, 33), (33, 65), (65, HP)]

        def xA_chunk(eng, g, ci):
            r0, r1 = row_chunks[ci]
            eng.dma_start(
                out=padA[g][:, r0:r1, :],
                in_=xA_d[g * 128:(g + 1) * 128, r0:r1, :],
            )

        def emit_xB_dma(g):
            # WAW gate: tiny DVE write into padB that depends on g1's last
            # xA chunk keeps Tile from hoisting the xB transfer into the
            # critical input window
            nc.vector.tensor_copy(padB[g][:, 0:1, 0:2], padA[1][:, 97:98, 0:2])
            nc.scalar.dma_start(
                out=padB[g][:, :, :],
                in_=xB_d[g * 128:(g + 1) * 128, :, :],
            )

        # g0 then g1 serialized on the sync queue; weights then g2 on the
        # gpsimd queue (parallel to sync)
        for ci in range(3):
            xA_chunk(nc.sync, 0, ci)
        nc.gpsimd.dma_start(out=pk128, in_=pk128_d)
        nc.gpsimd.dma_start(out=pk48, in_=pk48_d)
        nc.gpsimd.dma_start(out=adkT_sb, in_=adkT_d)
        nc.gpsimd.dma_start(out=eye, in_=eye_d)
        for ci in range(3):
            xA_chunk(nc.gpsimd, 2, ci)
        for ci in range(3):
            xA_chunk(nc.sync, 1, ci)

        # ---------------- stats ----------------
        def sums_scl(g, k):
            # 3 window sums of band k via ScalarE ACT accumulate
            for j in range(3):
                win = padA[g][:, 1 + 32 * k:33 + 32 * k, 2 + 32 * j:34 + 32 * j]
                acc = pooled[g][:, 3 * k + j:3 * k + j + 1]
                s = sclp.tile([128, 32, 32], bf16, tag="wscr", name=f"w{g}_{k}_{j}")
                nc.scalar.activation(out=s[:, :, :], in_=win,
                                     func=ACTF.Copy, accum_out=acc)

        def sums_tree(g, k):
            # DVE: bf16 TT add-tree 32->16->8->4 rows (2x mode), then 3
            # cache-reduce window sums over the 4 leaf rows
            p = padA[g]
            r0 = 1 + 32 * k
            t16 = treep.tile([128, 16, WP], bf16, tag="tr16", name=f"s16_{g}_{k}")
            nc.vector.tensor_add(t16[:, :, :], p[:, r0:r0 + 16, :], p[:, r0 + 16:r0 + 32, :])
            t8 = treep.tile([128, 8, WP], bf16, tag="tr8", name=f"s8_{g}_{k}")
            nc.vector.tensor_add(t8[:, :, :], t16[:, 0:8, :], t16[:, 8:16, :])
            t4 = treep.tile([128, 4, WP], bf16, tag="tr4", name=f"s4_{g}_{k}")
            nc.vector.tensor_add(t4[:, :, :], t8[:, 0:4, :], t8[:, 4:8, :])
            for j in range(3):
                acc = pooled[g][:, 3 * k + j:3 * k + j + 1]
                s = treep.tile([128, 4, 32], bf16, tag="wscr4", name=f"w{g}_{k}_{j}")
                nc.vector.tensor_scalar(s[:, :, :],
                                        t4[:, :, 2 + 32 * j:34 + 32 * j],
                                        1.0, None,
                                        op0=ALU.mult, op1=ALU.add, accum_out=acc)

        def band_max(g, k):
            # DVE: bf16 TT max-tree 32->16->8->4 rows into lvmax
            p = padA[g]
            r0 = 1 + 32 * k
            t16 = treep.tile([128, 16, WP], bf16, tag="tr16", name=f"m16_{g}_{k}")
            nc.vector.tensor_tensor(out=t16[:, :, :], in0=p[:, r0:r0 + 16, :],
                                    in1=p[:, r0 + 16:r0 + 32, :], op=ALU.max)
            t8 = treep.tile([128, 8, WP], bf16, tag="tr8", name=f"m8_{g}_{k}")
            nc.vector.tensor_tensor(out=t8[:, :, :], in0=t16[:, 0:8, :],
                                    in1=t16[:, 8:16, :], op=ALU.max)
            nc.vector.tensor_tensor(out=lvmax[g][:, 4 * k:4 * k + 4, :],
                                    in0=t8[:, 0:4, :], in1=t8[:, 4:8, :], op=ALU.max)

        def mx_fin(g):
            t6 = treep.tile([128, 6, WP], bf16, tag="tr6", name=f"mf6_{g}")
            nc.vector.tensor_tensor(out=t6[:, :, :], in0=lvmax[g][:, 0:6, :],
                                    in1=lvmax[g][:, 6:12, :], op=ALU.max)
            t3 = treep.tile([128, 3, WP], bf16, tag="tr3", name=f"mf3_{g}")
            nc.vector.tensor_tensor(out=t3[:, :, :], in0=t6[:, 0:3, :],
                                    in1=t6[:, 3:6, :], op=ALU.max)
            nc.vector.tensor_reduce(out=mx[g][:, :], in_=t3[:, :, :],
                                    axis=mybir.AxisListType.XY, op=ALU.max)

        def emit_avg_fin(g):
            asc = scr.tile([128, 9], bf16, tag="ascr", name=f"avg{g}")
            nc.scalar.activation(out=asc[:, :], in_=pooled[g][:, :],
                                 func=ACTF.Copy, accum_out=avgs[g][:, :])

        # ---------------- per-sample algebra (band-incremental) ----------------
        def sample_chunks(b):
            if b == 0:
                return [
                    (w1avg_a[:, :], w1mx_a[:, :], p1a[:, :], (0, 0, 128)),
                    (w1avg_b[0:64, :], w1mx_b[0:64, :], p1b[0:64, :], (2, 0, 64)),
                ]
            return [
                (w1avg_a[:, :], w1mx_a[:, :], p1a[:, :], (1, 0, 128)),
                (w1avg_b[64:128, :], w1mx_b[64:128, :], p1b[64:128, :], (2, 64, 128)),
            ]

        def emit_sample_pool_band(b, k):
            base = 8 + b * 22
            for i, (wa, wm, wp, (sg, q0, q1)) in enumerate(sample_chunks(b)):
                o = base + 11 * i
                nc.tensor.matmul(stpa[0:R, o + 2 + 3 * k:o + 5 + 3 * k], lhsT=wp,
                                 rhs=pooled[sg][q0:q1, 3 * k:3 * k + 3],
                                 start=True, stop=True)

        def emit_sample_theta(b):
            base = 8 + b * 22
            for i, (wa, wm, wp, (sg, q0, q1)) in enumerate(sample_chunks(b)):
                o = base + 11 * i
                nc.tensor.matmul(stpa[0:R, o:o + 1], lhsT=wa, rhs=avgs[sg][q0:q1, :], start=True, stop=True)
                nc.tensor.matmul(stpa[0:R, o + 1:o + 2], lhsT=wm, rhs=mx[sg][q0:q1, :], start=True, stop=True)

        def emit_fold_pool_band(b, k):
            base = 8 + b * 22
            hc = scr.tile([R, 3], f32, tag="scr3", name=f"hc{b}_{k}")
            nc.vector.tensor_copy(hc[:, :], stpa[0:R, base + 2 + 3 * k:base + 5 + 3 * k])
            hs = scr.tile([R, 3], f32, tag="scr3", name=f"hs{b}_{k}")
            nc.vector.tensor_add(hs[:, :], hc[:, :],
                                 stpa[0:R, base + 13 + 3 * k:base + 16 + 3 * k])
            t1 = scr.tile([R, 3], f32, tag="scr3", name=f"bn{b}_{k}")
            nc.vector.tensor_scalar(t1[:, :], hs[:, :], bns[:, :], bnb[:, :],
                                    op0=ALU.mult, op1=ALU.add)
            nc.vector.tensor_scalar_max(h_adk[b][:, 3 * k:3 * k + 3], t1[:, :], 0.0)

        def emit_fold_theta(b):
            base = 8 + b * 22
            hg = scr.tile([R, 2], f32, tag="scr2", name=f"hg{b}")
            nc.vector.tensor_copy(hg[:, :], stpa[0:R, base:base + 2])
            hs = scr.tile([R, 2], f32, tag="scr2", name=f"ht{b}")
            nc.vector.tensor_add(hs[:, :], hg[:, :],
                                 stpa[0:R, base + 11:base + 13])
            ha = scr.tile([R, 1], f32, tag="scr1", name=f"ha{b}")
            hm = scr.tile([R, 1], f32, tag="scr1", name=f"hm{b}")
            nc.vector.tensor_scalar_max(ha[:, :], hs[:, 0:1], 0.0)
            nc.vector.tensor_scalar_max(hm[:, :], hs[:, 1:2], 0.0)
            nc.vector.tensor_add(hsum[b][:, :], ha[:, :], hm[:, :])

        # ---------------- theta ----------------
        ps_t = [stpa[:, i:i + 1] for i in range(3)]

        def emit_theta_mm(b):
            nc.tensor.matmul(ps_t[b], lhsT=w2t[:, 0:128], rhs=hsum[b][:, :], start=True, stop=True)
            q0, q1 = (0, 64) if b == 0 else (64, 128)
            nc.tensor.matmul(ps_t[2][q0:q1], lhsT=w2t[:, 128:192], rhs=hsum[b][:, :], start=True, stop=True)

        def emit_theta_fin(g):
            et = scr.tile([128, 1], f32, tag="scr1", name=f"et{g}")
            nc.scalar.activation(out=et[:, :], in_=ps_t[g], func=ACTF.Exp, scale=-1.0)
            d = scr.tile([128, 1], f32, tag="scr1", name=f"etd{g}")
            nc.vector.tensor_scalar_add(d[:, :], et[:, :], 1.0)
            nc.vector.reciprocal(th[g][:, :], d[:, :])

        # ---------------- dynamic kernels w9 (band-major layout) ----------------
        # ps_s columns: 12*k + 3*gg + j ; adkT host layout matches.
        def emit_w9_mm_band(g, k):
            ps_s = stpa[:, 64 + g * 36:64 + (g + 1) * 36]
            for gg in range(G):
                sl = slice(12 * k + 3 * gg, 12 * k + 3 * gg + 3)
                if g < 2:
                    nc.tensor.matmul(ps_s[:, sl], lhsT=w2s[:, gg * 192:gg * 192 + 128],
                                     rhs=h_adk[g][:, 3 * k:3 * k + 3], start=True, stop=True)
                else:
                    nc.tensor.matmul(ps_s[0:64, sl], lhsT=w2s[:, gg * 192 + 128:gg * 192 + 192],
                                     rhs=h_adk[0][:, 3 * k:3 * k + 3], start=True, stop=True)
                    nc.tensor.matmul(ps_s[64:128, sl], lhsT=w2s[:, gg * 192 + 128:gg * 192 + 192],
                                     rhs=h_adk[1][:, 3 * k:3 * k + 3], start=True, stop=True)

        def emit_w9_exp_band(g, k):
            ps_s = stpa[:, 64 + g * 36 + 12 * k:64 + g * 36 + 12 * k + 12]
            e = scr.tile([128, 12], f32, tag="scr12", name=f"e{g}_{k}")
            nc.scalar.activation(out=e[:, :], in_=ps_s, func=ACTF.Exp)
            return e

        def emit_w9_exp_all(g):
            ps_s = stpa[:, 64 + g * 36:64 + (g + 1) * 36]
            e = scr.tile([128, 36], f32, tag="scr36", name=f"eall{g}")
            nc.scalar.activation(out=e[:, :], in_=ps_s, func=ACTF.Exp)
            return e

        def emit_w9_fin_band(g, k, e):
            # e: [128, 12] (4 groups x 3 cols) for band k
            d1 = scr.tile([128, 3], f32, tag="scr3b", name=f"d1_{g}_{k}")
            d2 = scr.tile([128, 3], f32, tag="scr3b", name=f"d2_{g}_{k}")
            nc.vector.tensor_add(d1[:, :], e[:, 0:3], e[:, 3:6])
            nc.vector.tensor_add(d2[:, :], e[:, 6:9], e[:, 9:12])
            nc.vector.tensor_add(d1[:, :], d1[:, :], d2[:, :])
            rec = scr.tile([128, 3], f32, tag="scr3b", name=f"rec{g}_{k}")
            nc.vector.reciprocal(rec[:, :], d1[:, :])
            a = adkT[g][:, 12 * k:12 * k + 12]
            m1 = scr.tile([128, 3], f32, tag="scr3b", name=f"m1_{g}_{k}")
            m2 = scr.tile([128, 3], f32, tag="scr3b", name=f"m2_{g}_{k}")
            nc.vector.tensor_mul(m1[:, :], e[:, 0:3], a[:, 0:3])
            nc.vector.tensor_mul(m2[:, :], e[:, 3:6], a[:, 3:6])
            nc.vector.tensor_add(m1[:, :], m1[:, :], m2[:, :])
            nc.vector.tensor_mul(m2[:, :], e[:, 6:9], a[:, 6:9])
            nc.vector.tensor_add(m1[:, :], m1[:, :], m2[:, :])
            nc.vector.tensor_mul(m2[:, :], e[:, 9:12], a[:, 9:12])
            nc.vector.tensor_add(m1[:, :], m1[:, :], m2[:, :])
            nc.vector.tensor_mul(w9[g][:, 3 * k:3 * k + 3], m1[:, :], rec[:, :])

        def emit_wsum9(g):
            nc.vector.tensor_reduce(out=wsum9[g][:, :], in_=w9[g][:, :],
                                    axis=mybir.AxisListType.X, op=ALU.add)

        def emit_w4p(g):
            t1 = scr.tile([128, 1], f32, tag="scr1", name=f"t1_{g}")
            nc.vector.tensor_mul(t1[:, :], w9[g][:, 4:5], th[g][:, :])
            nc.vector.tensor_add(t1[:, :], t1[:, :], w9[g][:, 4:5])
            nc.vector.tensor_sub(w4p[g][:, :], t1[:, :], wsum9[g][:, :])

        def emit_diag(g, taps, engine):
            for tap in taps:
                scal = w4p[g][:, 0:1] if tap == 4 else w9[g][:, tap:tap + 1]
                if engine == "vector":
                    nc.vector.tensor_scalar_mul(diag[g][:, tap, :], eye[:, :], scal)
                else:
                    nc.scalar.activation(out=diag[g][:, tap, :], in_=eye[:, :],
                                         func=ACTF.Copy, scale=scal)

        # ---------------- conv on PE ----------------
        win_r0 = {}
        win_pool = {}
        gidx = 0
        for g in range(3):
            r = 0
            for w, nb in enumerate(WIN_SEQ[g]):
                win_r0[(g, w)] = r
                win_pool[(g, w)] = gidx % 2  # 0 -> psA(4), 1 -> psB(3)
                r += 5 * nb
                gidx += 1

        pools = {0: psA, 1: psB}
        win_tile = {}

        def conv_taps(g, w, taps, first=True, last=True):
            nb = WIN_SEQ[g][w]
            r0 = win_r0[(g, w)]
            key = (g, w)
            if key not in win_tile:
                pi = win_pool[key]
                pnb = 4 if pi == 0 else 3
                win_tile[key] = pools[pi].tile(
                    [128, pnb, 512], f32, tag=f"w{pnb}", name=f"ps{g}_{w}")
            ps = win_tile[key]
            for tap in taps:
                dy, dx = divmod(tap, 3)
                for b in range(nb):
                    y0 = r0 + 5 * b + dy
                    nc.tensor.matmul(
                        ps[:, b, 0:480],
                        lhsT=diag[g][:, tap, :],
                        rhs=padA[g][:, y0:y0 + 5, dx + 1:dx + 97],
                        start=(first and tap == taps[0]),
                        stop=(last and tap == taps[-1]),
                    )

        def conv_drain_copy(g, w):
            nb = WIN_SEQ[g][w]
            ps = win_tile.pop((g, w))
            nr = 5 * nb
            ot = osb_pool.tile([128, 20, W], bf16, tag="ow", name=f"ow{g}_{w}")
            nc.scalar.activation(
                out=ot[:, 0:nr, :],
                in_=ps[:, 0:nb, 0:480], func=ACTF.Copy)
            return ot

        out_rr = [0]

        def conv_out_dma(g, w, ot):
            nb = WIN_SEQ[g][w]
            r0 = win_r0[(g, w)]
            nr = 5 * nb
            eng = (nc.sync, nc.gpsimd)[out_rr[0] % 2]
            out_rr[0] += 1
            eng.dma_start(
                out=out_d[g * 128:(g + 1) * 128, r0:r0 + nr, :],
                in_=ot[:, 0:nr, :])

        def conv_center_dma(g, w, ot):
            nb = WIN_SEQ[g][w]
            r0 = win_r0[(g, w)]
            nr = 5 * nb
            tm = ct_pool.tile([128, 20, W], bf16, tag="ct", name=f"ct{g}_{w}")
            nc.vector.tensor_scalar_mul(
                tm[:, 0:nr, :], padA[g][:, r0 + 1:r0 + nr + 1, 2:98], w4p[g][:, 0:1])
            nc.vector.tensor_add(ot[:, 0:nr, :], ot[:, 0:nr, :], tm[:, 0:nr, :])
            conv_out_dma(g, w, ot)

        def conv_drain(g, w):
            conv_out_dma(g, w, conv_drain_copy(g, w))

        # ---------------- conv tail on DVE (flat strips) ----------------
        def emit_conv_dve(g, lo, hi):
            y0 = ROWS_PE[g] + lo
            n = hi - lo
            L = (n - 1) * 100 + 96
            acc = None
            for i, tap in enumerate(TAP_ORDER):
                dy, dx = divmod(tap, 3)
                scal = w4p[g][:, 0:1] if tap == 4 else w9[g][:, tap:tap + 1]
                if dx == 1:
                    src = padAf[g][:, (y0 + dy) * 100 + 2:(y0 + dy) * 100 + 2 + L]
                else:
                    o0 = (y0 + dy - XB_R0) * 100 + (2 if dx == 0 else 4)
                    src = padBf[g][:, o0:o0 + L]
                t = term_pool.tile([128, 16, 100], bf16, tag="term",
                                   name=f"t{g}_{lo}_{i}")
                tf = t.rearrange("p a b -> p (a b)")
                nc.vector.tensor_scalar_mul(tf[:, 0:L], src, scal)
                if i == 0:
                    acc = t
                elif i < 8:
                    nxt = term_pool.tile([128, 16, 100], bf16, tag="term",
                                         name=f"a{g}_{lo}_{i}")
                    nxf = nxt.rearrange("p a b -> p (a b)")
                    nc.vector.tensor_add(nxf[:, 0:L], acc.rearrange("p a b -> p (a b)")[:, 0:L], tf[:, 0:L])
                    acc = nxt
                else:
                    nc.vector.tensor_add(tailb[g][:, lo:hi, :],
                                         acc[:, 0:n, 0:96], t[:, 0:n, 0:96])
            eng = (nc.sync, nc.gpsimd)[out_rr[0] % 2]
            out_rr[0] += 1
            eng.dma_start(out=out_d[g * 128:(g + 1) * 128, y0:y0 + n, :],
                          in_=tailb[g][:, lo:hi, :])

        # ---------------- PE warm-up ----------------
        def emit_warmup(k, gate, drain=False):
            for j in range(k):
                nc.tensor.matmul(stpa[:, 384:512], lhsT=eye[:, :],
                                 rhs=gate, start=True, stop=True)
            if drain:
                wsc = scr.tile([128, 1], bf16, tag="wscr1", name="wscr")
                nc.scalar.activation(out=wsc[:, :], in_=stpa[:, 384:385], func=ACTF.Copy)
                nc.sync.dma_start(out=warm_d, in_=wsc[:, :])

        # ================ emission order ================
        # PE warmups gated on arriving chunks (g0 via sync, g2 via vector)
        emit_warmup(12, padA[0][:, 10:12, 0:64])
        emit_warmup(12, padA[2][:, 10:12, 0:64])
        emit_warmup(12, padA[0][:, 40:42, 0:64])
        emit_warmup(10, padA[2][:, 40:42, 0:64], drain=True)

        # band 0: stats + algebra chain + first conv wave
        sums_scl(2, 0)              # SCL
        sums_tree(0, 0)             # DVE
        emit_sample_pool_band(0, 0)  # PE
        emit_fold_pool_band(0, 0)   # DVE
        emit_w9_mm_band(0, 0)       # PE
        e00 = emit_w9_exp_band(0, 0)  # SCL
        emit_w9_fin_band(0, 0, e00)  # DVE
        emit_diag(0, BAND_TAPS[0], "vector")
        conv_taps(0, 0, BAND_TAPS[0], first=True, last=False)
        conv_taps(0, 1, BAND_TAPS[0], first=True, last=False)

        # band 1
        sums_scl(2, 1)
        sums_tree(0, 1)
        emit_sample_pool_band(0, 1)
        emit_fold_pool_band(0, 1)
        emit_w9_mm_band(0, 1)
        e01 = emit_w9_exp_band(0, 1)
        emit_w9_fin_band(0, 1, e01)
        emit_diag(0, BAND_TAPS[1], "vector")
        conv_taps(0, 0, BAND_TAPS[1], first=False, last=False)
        conv_taps(0, 1, BAND_TAPS[1], first=False, last=False)

        # band 2
        sums_scl(2, 2)
        sums_tree(0, 2)
        emit_sample_pool_band(0, 2)
        emit_fold_pool_band(0, 2)
        emit_w9_mm_band(0, 2)
        e02 = emit_w9_exp_band(0, 2)
        emit_w9_fin_band(0, 2, e02)
        emit_diag(0, BAND_TAPS[2], "vector")
        conv_taps(0, 0, BAND_TAPS[2], first=False, last=True)
        conv_taps(0, 1, BAND_TAPS[2], first=False, last=True)

        # theta(0) path: maxes on DVE, avgfins on SCL
        band_max(0, 0)
        band_max(0, 1)
        band_max(0, 2)
        mx_fin(0)
        band_max(2, 0)
        band_max(2, 1)
        band_max(2, 2)
        mx_fin(2)
        emit_avg_fin(0)             # SCL
        emit_avg_fin(2)             # SCL
        emit_sample_theta(0)        # PE
        emit_fold_theta(0)          # DVE
        emit_theta_mm(0)            # PE
        ot00 = conv_drain_copy(0, 0)  # SCL (frees PSUM A)
        ot01 = conv_drain_copy(0, 1)  # SCL (frees PSUM B)
        emit_theta_fin(0)           # SCL exp + DVE
        emit_wsum9(0)               # DVE
        emit_w4p(0)                 # DVE
        emit_diag(0, [4], "vector")

        conv_taps(0, 2, TAP_ORDER)

        conv_center_dma(0, 0, ot00)  # DVE + DMA
        conv_center_dma(0, 1, ot01)

        # g1 stats on SCL (gated on g1 DMA), xB kicks after theta exp
        sums_scl(1, 0)
        sums_scl(1, 1)
        emit_xB_dma(0)
        emit_xB_dma(1)
        emit_xB_dma(2)
        sums_scl(1, 2)
        emit_avg_fin(1)

        band_max(1, 0)              # DVE
        band_max(1, 1)
        band_max(1, 2)
        mx_fin(1)

        conv_drain(0, 2)
        conv_taps(0, 3, TAP_ORDER)

        # sample-1 algebra
        for k in range(3):
            emit_sample_pool_band(1, k)   # PE
        for k in range(3):
            emit_fold_pool_band(1, k)     # DVE
        emit_sample_theta(1)              # PE
        emit_fold_theta(1)                # DVE
        emit_theta_mm(1)                  # PE
        for k in range(3):
            emit_w9_mm_band(1, k)         # PE
        for k in range(3):
            emit_w9_mm_band(2, k)         # PE
        e1 = emit_w9_exp_all(1)           # SCL
        e2 = emit_w9_exp_all(2)           # SCL
        emit_theta_fin(1)                 # SCL + DVE
        for k in range(3):
            emit_w9_fin_band(1, k, e1[:, 12 * k:12 * k + 12])
        emit_wsum9(1)
        emit_w4p(1)
        emit_diag(1, TAP_ORDER, "scalar")
        emit_theta_fin(2)
        for k in range(3):
            emit_w9_fin_band(2, k, e2[:, 12 * k:12 * k + 12])
        emit_wsum9(2)
        emit_w4p(2)
        emit_diag(2, TAP_ORDER, "scalar")

        conv_drain(0, 3)
        conv_taps(1, 0, TAP_ORDER)
        emit_conv_dve(0, *TAIL_CHUNKS[0][0])
        conv_drain(1, 0)
        conv_taps(1, 1, TAP_ORDER)
        emit_conv_dve(0, *TAIL_CHUNKS[0][1])
        conv_drain(1, 1)
        conv_taps(1, 2, TAP_ORDER)
        emit_conv_dve(1, *TAIL_CHUNKS[1][0])
        conv_drain(1, 2)
        conv_taps(1, 3, TAP_ORDER)
        emit_conv_dve(2, *TAIL_CHUNKS[2][0])
        conv_drain(1, 3)
        conv_taps(1, 4, TAP_ORDER)
        emit_conv_dve(1, *TAIL_CHUNKS[1][1])
        conv_drain(1, 4)
        emit_conv_dve(2, *TAIL_CHUNKS[2][1])
        conv_taps(2, 0, TAP_ORDER)
        conv_drain(2, 0)
        conv_taps(2, 1, TAP_ORDER)
        conv_drain(2, 1)
        conv_taps(2, 2, TAP_ORDER)
        conv_drain(2, 2)
        conv_taps(2, 3, TAP_ORDER)
        conv_drain(2, 3)

    nc.compile()
    return nc


def _host_prep(inputs):
    x = np.ascontiguousarray(inputs["x"], dtype=np.float32)
    cam_w1 = np.asarray(inputs["cam_w1"], dtype=np.float32)
    cam_w2 = np.asarray(inputs["cam_w2"], dtype=np.float32)
    proj_w1 = np.asarray(inputs["proj_w1"], dtype=np.float32)
    bn_gamma = np.asarray(inputs["bn_gamma"], dtype=np.float32)
    bn_beta = np.asarray(inputs["bn_beta"], dtype=np.float32)
    proj_w2 = np.asarray(inputs["proj_w2"], dtype=np.float32)
    adk = np.asarray(inputs["adk_weight"], dtype=np.float32)

    xb16 = x.astype(BF16)
    xpA = np.zeros((B, C, HP, WP), dtype=BF16)
    xpA[:, :, 1:97, 2:98] = xb16
    # padB: x payload at col 3, rows = padded rows 63..96 (x rows 62..95)
    xpB = np.zeros((B, C, XB_NR, WP), dtype=BF16)
    xpB[:, :, 0:34, 3:99] = xb16[:, :, 62:96, :]

    in_maps = []
    w1t = cam_w1.T.astype(np.float32)
    p1t = (proj_w1.T / 1024.0).astype(np.float32)
    cmap = np.concatenate([np.arange(128), np.arange(128),
                           np.arange(128, 192), np.arange(128, 192)])
    # adkT band-major: col = dy*12 + gg*3 + dx  (from adk[gg, c, dy, dx])
    adk_bm = adk.transpose(2, 0, 3, 1).reshape(36, C).T  # [C, 36]
    pk128 = np.concatenate([
        w1t[0:128] / (H * W),
        np.concatenate([w1t[128:192] / (H * W)] * 2, axis=0),
        w1t[0:128],
        np.concatenate([w1t[128:192]] * 2, axis=0),
        p1t[0:128],
        np.concatenate([p1t[128:192]] * 2, axis=0),
    ], axis=1).astype(np.float32)
    pk48 = np.concatenate([
        cam_w2.T,
        proj_w2.T,
        (bn_gamma / np.sqrt(1.0 + BN_EPS)).reshape(R, 1),
        bn_beta.reshape(R, 1),
    ], axis=1).astype(np.float32)
    adkT_full = adk_bm[cmap].astype(np.float32)  # [384, 36]
    adkT_pk = np.concatenate([adkT_full[0:128], adkT_full[128:256],
                              adkT_full[256:384]], axis=1)  # [128, 108]
    consts = {
        "eye": np.eye(128, dtype=BF16),
        "pk128": np.ascontiguousarray(pk128),
        "pk48": np.ascontiguousarray(pk48),
        "adkT": np.ascontiguousarray(adkT_pk),
    }
    for k in range(N_CORES):
        b0, b1 = 2 * k, 2 * k + 1
        shardA = np.ascontiguousarray(np.concatenate(
            [xpA[b0, 0:128], xpA[b1, 0:128], xpA[b0, 128:192], xpA[b1, 128:192]],
            axis=0))
        shardB = np.ascontiguousarray(np.concatenate(
            [xpB[b0, 0:128], xpB[b1, 0:128], xpB[b0, 128:192], xpB[b1, 128:192]],
            axis=0))
        m = {"xA": shardA, "xB": shardB}
        m.update(consts)
        in_maps.append(m)
    return in_maps


def kernel(**inputs) -> np.ndarray:
    global _COMPILED
    from concourse.bass_utils import run_bass_kernel_spmd

    in_maps = _host_prep(inputs)

    if _COMPILED is None:
        _COMPILED = _build()
    nc = _COMPILED

    res = run_bass_kernel_spmd(nc, in_maps, core_ids=list(range(N_CORES)))
    outs = [r["out"] for r in res.results]

    y = np.empty((B, C, H, W), np.float32)
    for k in range(N_CORES):
        o = np.asarray(outs[k]).reshape(384, H, W).astype(np.float32)
        b0, b1 = 2 * k, 2 * k + 1
        y[b0, 0:128] = o[0:128]
        y[b1, 0:128] = o[128:256]
        y[b0, 128:192] = o[256:320]
        y[b1, 128:192] = o[320:384]
    return y


if __name__ == "__main__":
    import reference

    inputs = {k: np.asarray(v) for k, v in reference.setup_inputs().items()}
    y = kernel(**inputs)
    print("kernel output:", y.shape, y.dtype)


# revision 14
# speedup vs baseline: 1.2815x; 1.0831x over previous
"""Trainium2 Bass kernel for nn_ADSCDConv (dense_cnn), 8-core data parallel.

Per core (2 samples = 384 (b,c) channel-images of 96x96), groups of 128
partitions: g0=(b0,c0:128), g1=(b1,c0:128), g2=(b0,c128:192)||(b1,c128:192).

v2 schedule (vs v1):
  - tap-outer conv matmuls: LDWEIGHTS amortized over a multi-bank PSUM
    window; PE runs at the 202ns/FD480 streaming roofline.
  - center tap (the only theta-dependent one) is emitted LAST per window
    and deferred for the first two windows of g0, so the conv starts on
    the pooled-only dependency chain while the image-max/theta chain
    finishes.
  - windows/bandmax stats are banded (32 rows) and pipelined with the
    input DMA; x lands padded to width 100 with the payload at col 2 so
    window sums hit the DVE 4x mode.
  - DVE conv tail uses fused scalar_tensor_tensor (mul+add in one op);
    a second x copy shifted by one column (padB) keeps all taps 4B
    aligned for the 2x bf16 mode.
  - PSUM: two conv window pools (4 banks + 3 banks) alternate A,B,A,...
    globally across groups; 1 stats bank.
"""

from contextlib import ExitStack

import numpy as np
import ml_dtypes

BF16 = ml_dtypes.bfloat16

B, C, H, W = 16, 192, 96, 96
G = 4
R = C // 4  # 48
BN_EPS = 1e-5
N_CORES = 8
HP, WP = H + 2, 100  # padded rows 98, padded cols 100 (x payload at col 2)
XB_R0 = 50           # padB covers padded rows 50..97
XB_NR = 48

# conv windows: banks per window, alternating pool A(4)/B(3) globally
# (sequence across groups must alternate 4,3,4,3,... for PSUM pool reuse)
WIN_SEQ = {0: [4, 3, 4, 3], 1: [4, 3, 4, 3], 2: [4, 3, 4, 3, 4]}
ROWS_PE = {g: 5 * sum(WIN_SEQ[g]) for g in range(3)}  # 70, 70, 90

# tap order: center (tap 4, the only theta-dependent tap) last
TAP_ORDER = [0, 1, 2, 3, 5, 6, 7, 8, 4]

_COMPILED = None


def _build():
    import concourse.tile as tile
    from concourse import bacc, mybir

    f32 = mybir.dt.float32
    bf16 = mybir.dt.bfloat16
    ALU = mybir.AluOpType
    ACTF = mybir.ActivationFunctionType

    nc = bacc.Bacc("TRN2", target_bir_lowering=False, debug=False, num_devices=N_CORES)

    # ---- DRAM tensors ----
    xA_d = nc.dram_tensor("xA", [384, HP, WP], bf16, kind="ExternalInput").ap()
    xB_d = nc.dram_tensor("xB", [384, XB_NR, WP], bf16, kind="ExternalInput").ap()
    out_d = nc.dram_tensor("out", [384, H, W], bf16, kind="ExternalOutput").ap()
    warm_d = nc.dram_tensor("warm", [128, 1], bf16, kind="ExternalOutput").ap()
    eye_d = nc.dram_tensor("eye", [128, 128], bf16, kind="ExternalInput").ap()
    w1avg_a_d = nc.dram_tensor("w1avg_a", [128, R], f32, kind="ExternalInput").ap()
    w1avg_b_d = nc.dram_tensor("w1avg_b", [128, R], f32, kind="ExternalInput").ap()
    w1mx_a_d = nc.dram_tensor("w1mx_a", [128, R], f32, kind="ExternalInput").ap()
    w1mx_b_d = nc.dram_tensor("w1mx_b", [128, R], f32, kind="ExternalInput").ap()
    w2t_d = nc.dram_tensor("w2t", [R, C], f32, kind="ExternalInput").ap()
    p1a_d = nc.dram_tensor("p1a", [128, R], f32, kind="ExternalInput").ap()
    p1b_d = nc.dram_tensor("p1b", [128, R], f32, kind="ExternalInput").ap()
    bns_d = nc.dram_tensor("bn_scale", [R, 1], f32, kind="ExternalInput").ap()
    bnb_d = nc.dram_tensor("bn_beta", [R, 1], f32, kind="ExternalInput").ap()
    w2s_d = nc.dram_tensor("w2s", [R, G * C], f32, kind="ExternalInput").ap()
    adkT_d = nc.dram_tensor("adkT", [384, 36], f32, kind="ExternalInput").ap()

    with tile.TileContext(nc) as tc, ExitStack() as ctx:
        def sb(name, shape, dt):
            return nc.alloc_sbuf_tensor(name, shape, dt).ap()

        padA = [sb(f"padA{g}", [128, HP, WP], bf16) for g in range(3)]
        padB = [sb(f"padB{g}", [128, XB_NR, WP], bf16) for g in range(3)]
        tailb = [sb(f"tail{g}", [128, 26, W], bf16) for g in range(3)]
        diag = [sb(f"diag{g}", [128, 9, 128], bf16) for g in range(3)]
        pooled = [sb(f"pooled{g}", [128, 9], f32) for g in range(3)]
        avgs = [sb(f"avgs{g}", [128, 1], f32) for g in range(3)]
        mx = [sb(f"mx{g}", [128, 1], f32) for g in range(3)]
        th = [sb(f"theta{g}", [128, 1], f32) for g in range(3)]
        w9 = [sb(f"w9_{g}", [128, 9], f32) for g in range(3)]
        w4p = [sb(f"w4p{g}", [128, 1], f32) for g in range(3)]
        wsum9 = [sb(f"wsum9_{g}", [128, 1], f32) for g in range(3)]
        adkT = [sb(f"adkT{g}_sb", [128, 36], f32) for g in range(3)]

        eye = sb("eye_sb", [128, 128], bf16)
        w1avg_a = sb("w1avg_a_sb", [128, R], f32)
        w1avg_b = sb("w1avg_b_sb", [128, R], f32)
        w1mx_a = sb("w1mx_a_sb", [128, R], f32)
        w1mx_b = sb("w1mx_b_sb", [128, R], f32)
        w2t = sb("w2t_sb", [R, C], f32)
        p1a = sb("p1a_sb", [128, R], f32)
        p1b = sb("p1b_sb", [128, R], f32)
        bns = sb("bns_sb", [R, 1], f32)
        bnb = sb("bnb_sb", [R, 1], f32)
        w2s = sb("w2s_sb", [R, G * C], f32)

        h_adk = [sb(f"h_adk{b}", [R, 9], f32) for b in range(2)]
        hsum = [sb(f"hsum{b}", [R, 1], f32) for b in range(2)]

        scr = ctx.enter_context(tc.tile_pool(name="scr", bufs=4))
        treep = ctx.enter_context(tc.tile_pool(name="treep", bufs=2))
        term_pool = ctx.enter_context(tc.tile_pool(name="terms", bufs=3))
        osb_pool = ctx.enter_context(tc.tile_pool(name="osbp", bufs=8))
        ct_pool = ctx.enter_context(tc.tile_pool(name="ctp", bufs=3))
        psA = ctx.enter_context(tc.tile_pool(name="psA", bufs=1, space="PSUM"))
        psB = ctx.enter_context(tc.tile_pool(name="psB", bufs=1, space="PSUM"))
        # stats bank: single-shot matmul groups only may share a bank
        stpa = nc.alloc_psum_tensor("statps", [128, 512], f32).ap()

        # ---------------- DMA emission ----------------
        row_chunks = [(0, 33), (33, 65), (65, HP)]

        def emit_xA_dma(g):
            for (r0, r1) in row_chunks:
                nc.sync.dma_start(
                    out=padA[g][:, r0:r1, :],
                    in_=xA_d[g * 128:(g + 1) * 128, r0:r1, :],
                )

        def emit_xB_dma(g):
            # WAW gate: tiny DVE write into padB dependent on g1's last xA
            # chunk keeps the xB transfer out of the critical input window
            nc.vector.tensor_copy(padB[g][:, 0:1, 0:2], padA[1][:, 97:98, 0:2])
            nc.scalar.dma_start(
                out=padB[g][:, :, :],
                in_=xB_d[g * 128:(g + 1) * 128, :, :],
            )

        emit_xA_dma(0)
        wloads = [
            (eye, eye_d), (w1avg_a, w1avg_a_d), (w1avg_b, w1avg_b_d),
            (w1mx_a, w1mx_a_d), (w1mx_b, w1mx_b_d), (w2t, w2t_d),
            (p1a, p1a_d), (p1b, p1b_d), (bns, bns_d), (bnb, bnb_d),
            (w2s, w2s_d),
            (adkT[0], adkT_d[0:128, :]), (adkT[1], adkT_d[128:256, :]),
            (adkT[2], adkT_d[256:384, :]),
        ]
        for (dst, src) in wloads:
            nc.gpsimd.dma_start(out=dst, in_=src)
        emit_xA_dma(2)
        emit_xA_dma(1)

        # ---------------- stats ----------------
        def emit_band_windows(g, k, engine="vector"):
            # 3 col-window sums of the 32-row band k -> pooled[g][:, 3k+j]
            if engine == "scalar":
                for j in range(3):
                    win = padA[g][:, 1 + 32 * k:33 + 32 * k, 2 + 32 * j:34 + 32 * j]
                    acc = pooled[g][:, 3 * k + j:3 * k + j + 1]
                    s = treep.tile([128, 32, 32], bf16, tag="wscr", name=f"w{g}_{k}_{j}")
                    nc.scalar.activation(out=s[:, :, :], in_=win,
                                         func=ACTF.Copy, accum_out=acc)
                return
            # DVE: bf16 TT add-tree 32->16->8->4 rows (2x mode), then 3
            # cache-reduce window sums over the 4 leaf rows
            p = padA[g]
            r0 = 1 + 32 * k
            t16 = treep.tile([128, 16, WP], bf16, tag="tr16", name=f"s16_{g}_{k}")
            nc.vector.tensor_add(t16[:, :, :], p[:, r0:r0 + 16, :], p[:, r0 + 16:r0 + 32, :])
            t8 = treep.tile([128, 8, WP], bf16, tag="tr8", name=f"s8_{g}_{k}")
            nc.vector.tensor_add(t8[:, :, :], t16[:, 0:8, :], t16[:, 8:16, :])
            t4 = treep.tile([128, 4, WP], bf16, tag="tr4", name=f"s4_{g}_{k}")
            nc.vector.tensor_add(t4[:, :, :], t8[:, 0:4, :], t8[:, 4:8, :])
            for j in range(3):
                acc = pooled[g][:, 3 * k + j:3 * k + j + 1]
                s = treep.tile([128, 4, 32], bf16, tag="wscr4", name=f"w{g}_{k}_{j}")
                nc.vector.tensor_scalar(s[:, :, :],
                                        t4[:, :, 2 + 32 * j:34 + 32 * j],
                                        1.0, None,
                                        op0=ALU.mult, op1=ALU.add, accum_out=acc)

        lvmax = [sb(f"lvmax{g}", [128, 12, WP], bf16) for g in range(3)]

        def emit_band_max(g, k):
            # bf16 TT max-tree 32->16->8->4 rows into lvmax[g][:, 4k:4k+4]
            p = padA[g]
            r0 = 1 + 32 * k
            t16 = treep.tile([128, 16, WP], bf16, tag="tr16", name=f"m16_{g}_{k}")
            nc.vector.tensor_tensor(out=t16[:, :, :], in0=p[:, r0:r0 + 16, :],
                                    in1=p[:, r0 + 16:r0 + 32, :], op=ALU.max)
            t8 = treep.tile([128, 8, WP], bf16, tag="tr8", name=f"m8_{g}_{k}")
            nc.vector.tensor_tensor(out=t8[:, :, :], in0=t16[:, 0:8, :],
                                    in1=t16[:, 8:16, :], op=ALU.max)
            nc.vector.tensor_tensor(out=lvmax[g][:, 4 * k:4 * k + 4, :],
                                    in0=t8[:, 0:4, :], in1=t8[:, 4:8, :], op=ALU.max)

        def emit_mx_fin(g):
            t6 = treep.tile([128, 6, WP], bf16, tag="tr6", name=f"mf6_{g}")
            nc.vector.tensor_tensor(out=t6[:, :, :], in0=lvmax[g][:, 0:6, :],
                                    in1=lvmax[g][:, 6:12, :], op=ALU.max)
            t3 = treep.tile([128, 3, WP], bf16, tag="tr3", name=f"mf3_{g}")
            nc.vector.tensor_tensor(out=t3[:, :, :], in0=t6[:, 0:3, :],
                                    in1=t6[:, 3:6, :], op=ALU.max)
            nc.vector.tensor_reduce(out=mx[g][:, :], in_=t3[:, :, :],
                                    axis=mybir.AxisListType.XY, op=ALU.max)

        def emit_avg_fin(g):
            asc = scr.tile([128, 9], bf16, tag="ascr", name=f"avg{g}")
            nc.scalar.activation(out=asc[:, :], in_=pooled[g][:, :],
                                 func=ACTF.Copy, accum_out=avgs[g][:, :])

        # ---------------- per-sample algebra ----------------
        def emit_sample(b, part):
            if b == 0:
                chunks = [
                    (w1avg_a[:, :], w1mx_a[:, :], p1a[:, :], (0, 0, 128)),
                    (w1avg_b[0:64, :], w1mx_b[0:64, :], p1b[0:64, :], (2, 0, 64)),
                ]
            else:
                chunks = [
                    (w1avg_a[:, :], w1mx_a[:, :], p1a[:, :], (1, 0, 128)),
                    (w1avg_b[64:128, :], w1mx_b[64:128, :], p1b[64:128, :], (2, 64, 128)),
                ]
            base = 8 + b * 22
            for i, (wa, wm, wp, (sg, q0, q1)) in enumerate(chunks):
                o = base + 11 * i
                if part == "pool":
                    nc.tensor.matmul(stpa[0:R, o + 2:o + 11], lhsT=wp, rhs=pooled[sg][q0:q1, :], start=True, stop=True)
                else:
                    nc.tensor.matmul(stpa[0:R, o:o + 1], lhsT=wa, rhs=avgs[sg][q0:q1, :], start=True, stop=True)
                    nc.tensor.matmul(stpa[0:R, o + 1:o + 2], lhsT=wm, rhs=mx[sg][q0:q1, :], start=True, stop=True)

        def emit_fold_pool(b):
            base = 8 + b * 22
            hc = scr.tile([R, 9], f32, tag="scr48", name=f"hc{b}")
            nc.vector.tensor_copy(hc[:, :], stpa[0:R, base + 2:base + 11])
            hs = scr.tile([R, 9], f32, tag="scr48", name=f"hs{b}")
            nc.vector.tensor_add(hs[:, :], hc[:, :],
                                 stpa[0:R, base + 13:base + 22])
            t1 = scr.tile([R, 9], f32, tag="scr48", name=f"bn{b}")
            nc.vector.tensor_scalar(t1[:, :], hs[:, :], bns[:, :], bnb[:, :],
                                    op0=ALU.mult, op1=ALU.add)
            nc.vector.tensor_scalar_max(h_adk[b][:, :], t1[:, :], 0.0)

        def emit_fold_theta(b):
            base = 8 + b * 22
            hg = scr.tile([R, 2], f32, tag="scr2", name=f"hg{b}")
            nc.vector.tensor_copy(hg[:, :], stpa[0:R, base:base + 2])
            hs = scr.tile([R, 2], f32, tag="scr2", name=f"ht{b}")
            nc.vector.tensor_add(hs[:, :], hg[:, :],
                                 stpa[0:R, base + 11:base + 13])
            ha = scr.tile([R, 1], f32, tag="scr1", name=f"ha{b}")
            hm = scr.tile([R, 1], f32, tag="scr1", name=f"hm{b}")
            nc.vector.tensor_scalar_max(ha[:, :], hs[:, 0:1], 0.0)
            nc.vector.tensor_scalar_max(hm[:, :], hs[:, 1:2], 0.0)
            nc.vector.tensor_add(hsum[b][:, :], ha[:, :], hm[:, :])

        # ---------------- theta ----------------
        ps_t = [stpa[:, i:i + 1] for i in range(3)]

        def emit_theta_mm(b):
            nc.tensor.matmul(ps_t[b], lhsT=w2t[:, 0:128], rhs=hsum[b][:, :], start=True, stop=True)
            q0, q1 = (0, 64) if b == 0 else (64, 128)
            nc.tensor.matmul(ps_t[2][q0:q1], lhsT=w2t[:, 128:192], rhs=hsum[b][:, :], start=True, stop=True)

        def emit_theta_fin(g):
            et = scr.tile([128, 1], f32, tag="scr1", name=f"et{g}")
            nc.scalar.activation(out=et[:, :], in_=ps_t[g], func=ACTF.Exp, scale=-1.0)
            d = scr.tile([128, 1], f32, tag="scr1", name=f"etd{g}")
            nc.vector.tensor_scalar_add(d[:, :], et[:, :], 1.0)
            nc.vector.reciprocal(th[g][:, :], d[:, :])

        # ---------------- dynamic kernels w9 ----------------
        def emit_w9_mm(g):
            ps_s = stpa[:, 64 + g * 36:64 + (g + 1) * 36]
            for gg in range(G):
                sl = slice(gg * 9, gg * 9 + 9)
                if g < 2:
                    nc.tensor.matmul(ps_s[:, sl], lhsT=w2s[:, gg * 192:gg * 192 + 128],
                                     rhs=h_adk[g][:, :], start=True, stop=True)
                else:
                    nc.tensor.matmul(ps_s[0:64, sl], lhsT=w2s[:, gg * 192 + 128:gg * 192 + 192],
                                     rhs=h_adk[0][:, :], start=True, stop=True)
                    nc.tensor.matmul(ps_s[64:128, sl], lhsT=w2s[:, gg * 192 + 128:gg * 192 + 192],
                                     rhs=h_adk[1][:, :], start=True, stop=True)

        def emit_w9_exp(g):
            ps_s = stpa[:, 64 + g * 36:64 + (g + 1) * 36]
            e = scr.tile([128, 36], f32, tag="scr36", name=f"e{g}")
            nc.scalar.activation(out=e[:, :], in_=ps_s, func=ACTF.Exp)
            return e

        def emit_w9_fin(g, e):
            d1 = scr.tile([128, 9], f32, tag="scr9", name=f"d1_{g}")
            d2 = scr.tile([128, 9], f32, tag="scr9", name=f"d2_{g}")
            nc.vector.tensor_add(d1[:, :], e[:, 0:9], e[:, 9:18])
            nc.vector.tensor_add(d2[:, :], e[:, 18:27], e[:, 27:36])
            nc.vector.tensor_add(d1[:, :], d1[:, :], d2[:, :])
            rec = scr.tile([128, 9], f32, tag="scr9", name=f"rec{g}")
            nc.vector.reciprocal(rec[:, :], d1[:, :])
            a = adkT[g]
            m1 = scr.tile([128, 9], f32, tag="scr9", name=f"m1_{g}")
            m2 = scr.tile([128, 9], f32, tag="scr9", name=f"m2_{g}")
            nc.vector.tensor_mul(m1[:, :], e[:, 0:9], a[:, 0:9])
            nc.vector.tensor_mul(m2[:, :], e[:, 9:18], a[:, 9:18])
            nc.vector.tensor_add(m1[:, :], m1[:, :], m2[:, :])
            nc.vector.tensor_mul(m2[:, :], e[:, 18:27], a[:, 18:27])
            nc.vector.tensor_add(m1[:, :], m1[:, :], m2[:, :])
            nc.vector.tensor_mul(m2[:, :], e[:, 27:36], a[:, 27:36])
            nc.vector.tensor_add(m1[:, :], m1[:, :], m2[:, :])
            nc.vector.tensor_mul(w9[g][:, :], m1[:, :], rec[:, :])
            nc.vector.tensor_reduce(out=wsum9[g][:, :], in_=w9[g][:, :],
                                    axis=mybir.AxisListType.X, op=ALU.add)

        def emit_w4p(g):
            t1 = scr.tile([128, 1], f32, tag="scr1", name=f"t1_{g}")
            nc.vector.tensor_mul(t1[:, :], w9[g][:, 4:5], th[g][:, :])
            nc.vector.tensor_add(t1[:, :], t1[:, :], w9[g][:, 4:5])
            nc.vector.tensor_sub(w4p[g][:, :], t1[:, :], wsum9[g][:, :])

        def emit_diag(g, taps, engine):
            for tap in taps:
                scal = w4p[g][:, 0:1] if tap == 4 else w9[g][:, tap:tap + 1]
                if engine == "vector":
                    nc.vector.tensor_scalar_mul(diag[g][:, tap, :], eye[:, :], scal)
                else:
                    nc.scalar.activation(out=diag[g][:, tap, :], in_=eye[:, :],
                                         func=ACTF.Copy, scale=scal)

        # ---------------- conv on PE ----------------
        # window w of group g covers rows win_r0 .. win_r0+5*banks
        win_r0 = {}
        for g in range(3):
            r = 0
            for w, nb in enumerate(WIN_SEQ[g]):
                win_r0[(g, w)] = r
                r += 5 * nb

        pools = {4: psA, 3: psB}
        win_tile = {}
        out_rr = [0]

        def conv_taps(g, w, taps):
            nb = WIN_SEQ[g][w]
            r0 = win_r0[(g, w)]
            key = (g, w)
            if key not in win_tile:
                win_tile[key] = pools[nb].tile(
                    [128, nb, 512], f32, tag=f"w{nb}", name=f"ps{g}_{w}")
            ps = win_tile[key]
            for tap in taps:
                dy, dx = divmod(tap, 3)
                for b in range(nb):
                    y0 = r0 + 5 * b + dy
                    nc.tensor.matmul(
                        ps[:, b, 0:480],
                        lhsT=diag[g][:, tap, :],
                        rhs=padA[g][:, y0:y0 + 5, dx + 1:dx + 97],
                        start=(tap == 0), stop=(tap == 8),
                    )

        def conv_drain(g, w):
            nb = WIN_SEQ[g][w]
            r0 = win_r0[(g, w)]
            ps = win_tile.pop((g, w))
            nr = 5 * nb
            ot = osb_pool.tile([128, 20, W], bf16, tag="ow", name=f"ow{g}_{w}")
            nc.scalar.activation(
                out=ot[:, 0:nr, :],
                in_=ps[:, 0:nb, 0:480], func=ACTF.Copy)
            tm = ct_pool.tile([128, 20, W], bf16, tag="ct", name=f"ct{g}_{w}")
            nc.vector.tensor_scalar_mul(
                tm[:, 0:nr, :], padA[g][:, r0 + 1:r0 + nr + 1, 2:98], w4p[g][:, 0:1])
            nc.vector.tensor_add(ot[:, 0:nr, :], ot[:, 0:nr, :], tm[:, 0:nr, :])
            eng = (nc.sync, nc.gpsimd)[out_rr[0] % 2]
            out_rr[0] += 1
            eng.dma_start(
                out=out_d[g * 128:(g + 1) * 128, r0:r0 + nr, :],
                in_=ot[:, 0:nr, :])

        # ---------------- conv tail on DVE ----------------
        def emit_conv_dve(g, lo, hi):
            # DVE tail rows [ROWS_PE+lo, ROWS_PE+hi): per tap a 4x mul then a
            # 2x add (STT is 1x on this HW, so mul+add pairs are faster)
            y0 = ROWS_PE[g] + lo
            n = hi - lo
            acc = None
            for i, tap in enumerate(TAP_ORDER):
                dy, dx = divmod(tap, 3)
                scal = w4p[g][:, 0:1] if tap == 4 else w9[g][:, tap:tap + 1]
                if dx == 1:
                    src = padA[g][:, y0 + dy:y0 + n + dy, 2:98]
                else:
                    rb = y0 + dy - XB_R0
                    col = 2 if dx == 0 else 4
                    src = padB[g][:, rb:rb + n, col:col + 96]
                t = term_pool.tile([128, 13, 96], bf16, tag="term",
                                   name=f"t{g}_{lo}_{i}")
                nc.vector.tensor_scalar_mul(t[:, 0:n, :], src, scal)
                if i == 0:
                    acc = t
                elif i < 8:
                    nxt = term_pool.tile([128, 13, 96], bf16, tag="term",
                                         name=f"a{g}_{lo}_{i}")
                    nc.vector.tensor_add(nxt[:, 0:n, :], acc[:, 0:n, :], t[:, 0:n, :])
                    acc = nxt
                else:
                    nc.vector.tensor_add(tailb[g][:, lo:hi, :], acc[:, 0:n, :], t[:, 0:n, :])
            eng = (nc.sync, nc.gpsimd)[out_rr[0] % 2]
            out_rr[0] += 1
            eng.dma_start(out=out_d[g * 128:(g + 1) * 128, y0:y0 + n, :],
                          in_=tailb[g][:, lo:hi, :])

        # ---------------- PE warm-up ----------------
        # junk matmuls gated on successive DMA chunks so the PE stays busy
        # (HAM warm) across the whole stats prelude without running eagerly
        def emit_warmup(k, gate, drain=False):
            for j in range(k):
                nc.tensor.matmul(stpa[:, 384:512], lhsT=eye[:, :],
                                 rhs=gate, start=True, stop=True)
            if drain:
                wsc = scr.tile([128, 1], bf16, tag="wscr1", name="wscr")
                nc.scalar.activation(out=wsc[:, :], in_=stpa[:, 384:385], func=ACTF.Copy)
                nc.sync.dma_start(out=warm_d, in_=wsc[:, :])

        # ---------------- emission order ----------------
        NC8 = TAP_ORDER[:8]

        # prelude: window sums first (pooled -> w9 -> conv is the critical
        # path; image-max/theta only gates the post-drain center-tap add)
        for k in range(3):
            emit_band_windows(0, k, engine="vector")
        emit_band_windows(2, 0, engine="scalar")
        emit_band_windows(2, 1, engine="scalar")
        emit_band_windows(2, 2, engine="vector")
        emit_avg_fin(0)
        emit_avg_fin(2)
        emit_xB_dma(0)
        emit_xB_dma(1)
        emit_xB_dma(2)
        emit_warmup(30, padA[0][:, 65:67, 0:64])
        emit_warmup(30, padA[2][:, 65:67, 0:64])
        emit_sample(0, "pool")
        emit_fold_pool(0)
        emit_w9_mm(0)
        e0 = emit_w9_exp(0)
        emit_w9_fin(0, e0)
        emit_diag(0, NC8, "vector")
        for k in range(3):
            emit_band_max(0, k)
            emit_band_max(2, k)
        emit_mx_fin(0)
        emit_mx_fin(2)
        emit_warmup(24, padA[1][:, 10:12, 0:64])
        emit_warmup(16, padA[1][:, 40:42, 0:64], drain=True)
        emit_sample(0, "theta")
        emit_fold_theta(0)
        emit_theta_mm(0)
        emit_theta_fin(0)
        emit_w4p(0)

        conv_taps(0, 0, NC8)
        conv_taps(0, 1, NC8)

        # g1 stats (data lands mid-conv-g0); sums on ScalarE (slack there)
        for k in range(3):
            emit_band_windows(1, k, engine="scalar")
            emit_band_max(1, k)
        emit_mx_fin(1)
        emit_avg_fin(1)

        conv_drain(0, 0)
        conv_taps(0, 2, NC8)
        emit_sample(1, "pool")
        emit_fold_pool(1)
        emit_sample(1, "theta")
        emit_fold_theta(1)
        emit_theta_mm(1)
        emit_theta_fin(1)
        emit_w9_mm(1)
        e1 = emit_w9_exp(1)
        emit_w9_fin(1, e1)
        emit_w4p(1)
        emit_diag(1, NC8, "scalar")
        conv_drain(0, 1)
        conv_taps(0, 3, NC8)
        emit_theta_fin(2)
        emit_w9_mm(2)
        e2 = emit_w9_exp(2)
        emit_w9_fin(2, e2)
        emit_w4p(2)
        emit_diag(2, NC8, "scalar")
        conv_drain(0, 2)
        conv_taps(1, 0, NC8)
        conv_drain(0, 3)
        emit_conv_dve(0, 0, 13)
        conv_taps(1, 1, NC8)
        conv_drain(1, 0)
        emit_conv_dve(0, 13, 26)
        conv_taps(1, 2, NC8)
        conv_drain(1, 1)
        emit_conv_dve(1, 0, 13)
        conv_taps(1, 3, NC8)
        conv_drain(1, 2)
        emit_conv_dve(1, 13, 26)
        conv_taps(2, 0, NC8)
        conv_drain(1, 3)
        conv_taps(2, 1, NC8)
        conv_drain(2, 0)
        conv_taps(2, 2, NC8)
        conv_drain(2, 1)
        emit_conv_dve(2, 0, 6)
        conv_taps(2, 3, NC8)
        conv_drain(2, 2)
        conv_taps(2, 4, NC8)
        conv_drain(2, 3)
        conv_drain(2, 4)

    nc.compile()
    return nc


def _host_prep(inputs):
    x = np.ascontiguousarray(inputs["x"], dtype=np.float32)
    cam_w1 = np.asarray(inputs["cam_w1"], dtype=np.float32)
    cam_w2 = np.asarray(inputs["cam_w2"], dtype=np.float32)
    proj_w1 = np.asarray(inputs["proj_w1"], dtype=np.float32)
    bn_gamma = np.asarray(inputs["bn_gamma"], dtype=np.float32)
    bn_beta = np.asarray(inputs["bn_beta"], dtype=np.float32)
    proj_w2 = np.asarray(inputs["proj_w2"], dtype=np.float32)
    adk = np.asarray(inputs["adk_weight"], dtype=np.float32)

    xb16 = x.astype(BF16)
    xpA = np.zeros((B, C, HP, WP), dtype=BF16)
    xpA[:, :, 1:97, 2:98] = xb16
    # padB: x payload at col 3, rows = padded rows 50..97 (x rows 49..95)
    xpB = np.zeros((B, C, XB_NR, WP), dtype=BF16)
    xpB[:, :, 0:47, 3:99] = xb16[:, :, 49:96, :]

    in_maps = []
    w1t = cam_w1.T.astype(np.float32)
    p1t = (proj_w1.T / 1024.0).astype(np.float32)
    cmap = np.concatenate([np.arange(128), np.arange(128),
                           np.arange(128, 192), np.arange(128, 192)])
    consts = {
        "eye": np.eye(128, dtype=BF16),
        "w1avg_a": np.ascontiguousarray(w1t[0:128] / (H * W)),
        "w1avg_b": np.ascontiguousarray(np.concatenate([w1t[128:192] / (H * W)] * 2, axis=0)),
        "w1mx_a": np.ascontiguousarray(w1t[0:128]),
        "w1mx_b": np.ascontiguousarray(np.concatenate([w1t[128:192]] * 2, axis=0)),
        "w2t": np.ascontiguousarray(cam_w2.T.astype(np.float32)),
        "p1a": np.ascontiguousarray(p1t[0:128]),
        "p1b": np.ascontiguousarray(np.concatenate([p1t[128:192]] * 2, axis=0)),
        "bn_scale": np.ascontiguousarray((bn_gamma / np.sqrt(1.0 + BN_EPS)).reshape(R, 1)),
        "bn_beta": np.ascontiguousarray(bn_beta.reshape(R, 1)),
        "w2s": np.ascontiguousarray(proj_w2.T.astype(np.float32)),
        "adkT": np.ascontiguousarray(
            adk.transpose(1, 0, 2, 3).reshape(C, G * 9)[cmap].astype(np.float32)
        ),
    }
    for k in range(N_CORES):
        b0, b1 = 2 * k, 2 * k + 1
        shardA = np.ascontiguousarray(np.concatenate(
            [xpA[b0, 0:128], xpA[b1, 0:128], xpA[b0, 128:192], xpA[b1, 128:192]],
            axis=0))
        shardB = np.ascontiguousarray(np.concatenate(
            [xpB[b0, 0:128], xpB[b1, 0:128], xpB[b0, 128:192], xpB[b1, 128:192]],
            axis=0))
        m = {"xA": shardA, "xB": shardB}
        m.update(consts)
        in_maps.append(m)
    return in_maps


def kernel(**inputs) -> np.ndarray:
    global _COMPILED
    from concourse.bass_utils import run_bass_kernel_spmd

    in_maps = _host_prep(inputs)

    if _COMPILED is None:
        _COMPILED = _build()
    nc = _COMPILED

    res = run_bass_kernel_spmd(nc, in_maps, core_ids=list(range(N_CORES)))
    outs = [r["out"] for r in res.results]

    y = np.empty((B, C, H, W), np.float32)
    for k in range(N_CORES):
        o = np.asarray(outs[k]).reshape(384, H, W).astype(np.float32)
        b0, b1 = 2 * k, 2 * k + 1
        y[b0, 0:128] = o[0:128]
        y[b1, 0:128] = o[128:256]
        y[b0, 128:192] = o[256:320]
        y[b1, 128:192] = o[320:384]
    return y


if __name__ == "__main__":
    import reference

    inputs = {k: np.asarray(v) for k, v in reference.setup_inputs().items()}
    y = kernel(**inputs)
    print("kernel output:", y.shape, y.dtype)



# revision 15
# speedup vs baseline: 1.3002x; 1.0146x over previous
"""Trainium2 Bass kernel for nn_ADSCDConv (dense_cnn), 8-core data parallel.

Per core (2 samples = 384 (b,c) channel-images of 96x96), groups of 128
partitions: g0=(b0,c0:128), g1=(b1,c0:128), g2=(b0,c128:192)||(b1,c128:192).

v2 schedule (vs v1):
  - tap-outer conv matmuls: LDWEIGHTS amortized over a multi-bank PSUM
    window; PE runs at the 202ns/FD480 streaming roofline.
  - center tap (the only theta-dependent one) is emitted LAST per window
    and deferred for the first two windows of g0, so the conv starts on
    the pooled-only dependency chain while the image-max/theta chain
    finishes.
  - windows/bandmax stats are banded (32 rows) and pipelined with the
    input DMA; x lands padded to width 100 with the payload at col 2 so
    window sums hit the DVE 4x mode.
  - DVE conv tail uses fused scalar_tensor_tensor (mul+add in one op);
    a second x copy shifted by one column (padB) keeps all taps 4B
    aligned for the 2x bf16 mode.
  - PSUM: two conv window pools (4 banks + 3 banks) alternate A,B,A,...
    globally across groups; 1 stats bank.
"""

from contextlib import ExitStack

import numpy as np
import ml_dtypes

BF16 = ml_dtypes.bfloat16

B, C, H, W = 16, 192, 96, 96
G = 4
R = C // 4  # 48
BN_EPS = 1e-5
N_CORES = 8
HP, WP = H + 2, 100  # padded rows 98, padded cols 100 (x payload at col 2)
XB_R0 = 50           # padB covers padded rows 50..97
XB_NR = 48

# conv windows: banks per window, alternating pool A(4)/B(3) globally
# (sequence across groups must alternate 4,3,4,3,... for PSUM pool reuse)
WIN_SEQ = {0: [4, 3, 4, 3], 1: [4, 3, 4, 3], 2: [4, 3, 4, 3, 4]}
ROWS_PE = {g: 5 * sum(WIN_SEQ[g]) for g in range(3)}  # 70, 70, 90

# tap order: center (tap 4, the only theta-dependent tap) last
TAP_ORDER = [0, 1, 2, 3, 5, 6, 7, 8, 4]

_COMPILED = None


def _build():
    import concourse.tile as tile
    from concourse import bacc, mybir

    f32 = mybir.dt.float32
    bf16 = mybir.dt.bfloat16
    ALU = mybir.AluOpType
    ACTF = mybir.ActivationFunctionType

    nc = bacc.Bacc("TRN2", target_bir_lowering=False, debug=False, num_devices=N_CORES)

    # ---- DRAM tensors ----
    xA_d = nc.dram_tensor("xA", [384, HP, WP], bf16, kind="ExternalInput").ap()
    xB_d = nc.dram_tensor("xB", [384, XB_NR, WP], bf16, kind="ExternalInput").ap()
    out_d = nc.dram_tensor("out", [384, H, W], bf16, kind="ExternalOutput").ap()
    warm_d = nc.dram_tensor("warm", [128, 1], bf16, kind="ExternalOutput").ap()
    eye_d = nc.dram_tensor("eye", [128, 128], bf16, kind="ExternalInput").ap()
    w1avg_a_d = nc.dram_tensor("w1avg_a", [128, R], f32, kind="ExternalInput").ap()
    w1avg_b_d = nc.dram_tensor("w1avg_b", [128, R], f32, kind="ExternalInput").ap()
    w1mx_a_d = nc.dram_tensor("w1mx_a", [128, R], f32, kind="ExternalInput").ap()
    w1mx_b_d = nc.dram_tensor("w1mx_b", [128, R], f32, kind="ExternalInput").ap()
    w2t_d = nc.dram_tensor("w2t", [R, C], f32, kind="ExternalInput").ap()
    p1a_d = nc.dram_tensor("p1a", [128, R], f32, kind="ExternalInput").ap()
    p1b_d = nc.dram_tensor("p1b", [128, R], f32, kind="ExternalInput").ap()
    bns_d = nc.dram_tensor("bn_scale", [R, 1], f32, kind="ExternalInput").ap()
    bnb_d = nc.dram_tensor("bn_beta", [R, 1], f32, kind="ExternalInput").ap()
    w2s_d = nc.dram_tensor("w2s", [R, G * C], f32, kind="ExternalInput").ap()
    adkT_d = nc.dram_tensor("adkT", [384, 36], f32, kind="ExternalInput").ap()

    with tile.TileContext(nc) as tc, ExitStack() as ctx:
        def sb(name, shape, dt):
            return nc.alloc_sbuf_tensor(name, shape, dt).ap()

        padA = [sb(f"padA{g}", [128, HP, WP], bf16) for g in range(3)]
        padB = [sb(f"padB{g}", [128, XB_NR, WP], bf16) for g in range(3)]
        padAf = [p.rearrange("p a b -> p (a b)") for p in padA]
        padBf = [p.rearrange("p a b -> p (a b)") for p in padB]
        tailb = [sb(f"tail{g}", [128, 26, W], bf16) for g in range(3)]
        diag = [sb(f"diag{g}", [128, 9, 128], bf16) for g in range(3)]
        pooled = [sb(f"pooled{g}", [128, 9], f32) for g in range(3)]
        avgs = [sb(f"avgs{g}", [128, 1], f32) for g in range(3)]
        mx = [sb(f"mx{g}", [128, 1], f32) for g in range(3)]
        th = [sb(f"theta{g}", [128, 1], f32) for g in range(3)]
        w9 = [sb(f"w9_{g}", [128, 9], f32) for g in range(3)]
        w4p = [sb(f"w4p{g}", [128, 1], f32) for g in range(3)]
        wsum9 = [sb(f"wsum9_{g}", [128, 1], f32) for g in range(3)]
        adkT = [sb(f"adkT{g}_sb", [128, 36], f32) for g in range(3)]

        eye = sb("eye_sb", [128, 128], bf16)
        w1avg_a = sb("w1avg_a_sb", [128, R], f32)
        w1avg_b = sb("w1avg_b_sb", [128, R], f32)
        w1mx_a = sb("w1mx_a_sb", [128, R], f32)
        w1mx_b = sb("w1mx_b_sb", [128, R], f32)
        w2t = sb("w2t_sb", [R, C], f32)
        p1a = sb("p1a_sb", [128, R], f32)
        p1b = sb("p1b_sb", [128, R], f32)
        bns = sb("bns_sb", [R, 1], f32)
        bnb = sb("bnb_sb", [R, 1], f32)
        w2s = sb("w2s_sb", [R, G * C], f32)

        h_adk = [sb(f"h_adk{b}", [R, 9], f32) for b in range(2)]
        hsum = [sb(f"hsum{b}", [R, 1], f32) for b in range(2)]

        scr = ctx.enter_context(tc.tile_pool(name="scr", bufs=4))
        treep = ctx.enter_context(tc.tile_pool(name="treep", bufs=2))
        term_pool = ctx.enter_context(tc.tile_pool(name="terms", bufs=3))
        osb_pool = ctx.enter_context(tc.tile_pool(name="osbp", bufs=8))
        ct_pool = ctx.enter_context(tc.tile_pool(name="ctp", bufs=3))
        psA = ctx.enter_context(tc.tile_pool(name="psA", bufs=1, space="PSUM"))
        psB = ctx.enter_context(tc.tile_pool(name="psB", bufs=1, space="PSUM"))
        # stats bank: single-shot matmul groups only may share a bank
        stpa = nc.alloc_psum_tensor("statps", [128, 512], f32).ap()

        # ---------------- DMA emission ----------------
        row_chunks = [(0, 33), (33, 65), (65, HP)]

        def emit_xA_dma(g):
            for (r0, r1) in row_chunks:
                nc.sync.dma_start(
                    out=padA[g][:, r0:r1, :],
                    in_=xA_d[g * 128:(g + 1) * 128, r0:r1, :],
                )

        def emit_xB_dma(g):
            # WAW gate: tiny DVE write into padB dependent on g1's last xA
            # chunk keeps the xB transfer out of the critical input window
            nc.vector.tensor_copy(padB[g][:, 0:1, 0:2], padA[1][:, 97:98, 0:2])
            nc.scalar.dma_start(
                out=padB[g][:, :, :],
                in_=xB_d[g * 128:(g + 1) * 128, :, :],
            )

        emit_xA_dma(0)
        wloads = [
            (eye, eye_d), (w1avg_a, w1avg_a_d), (w1avg_b, w1avg_b_d),
            (w1mx_a, w1mx_a_d), (w1mx_b, w1mx_b_d), (w2t, w2t_d),
            (p1a, p1a_d), (p1b, p1b_d), (bns, bns_d), (bnb, bnb_d),
            (w2s, w2s_d),
            (adkT[0], adkT_d[0:128, :]), (adkT[1], adkT_d[128:256, :]),
            (adkT[2], adkT_d[256:384, :]),
        ]
        for (dst, src) in wloads:
            nc.gpsimd.dma_start(out=dst, in_=src)
        emit_xA_dma(2)
        emit_xA_dma(1)

        # ---------------- stats ----------------
        def emit_band_windows(g, k, engine="vector"):
            # 3 col-window sums of the 32-row band k -> pooled[g][:, 3k+j]
            if engine == "scalar":
                for j in range(3):
                    win = padA[g][:, 1 + 32 * k:33 + 32 * k, 2 + 32 * j:34 + 32 * j]
                    acc = pooled[g][:, 3 * k + j:3 * k + j + 1]
                    s = treep.tile([128, 32, 32], bf16, tag="wscr", name=f"w{g}_{k}_{j}")
                    nc.scalar.activation(out=s[:, :, :], in_=win,
                                         func=ACTF.Copy, accum_out=acc)
                return
            # DVE: bf16 TT add-tree 32->16->8->4 rows (2x mode), then 3
            # cache-reduce window sums over the 4 leaf rows
            p = padA[g]
            r0 = 1 + 32 * k
            t16 = treep.tile([128, 16, WP], bf16, tag="tr16", name=f"s16_{g}_{k}")
            nc.vector.tensor_add(t16[:, :, :], p[:, r0:r0 + 16, :], p[:, r0 + 16:r0 + 32, :])
            t8 = treep.tile([128, 8, WP], bf16, tag="tr8", name=f"s8_{g}_{k}")
            nc.vector.tensor_add(t8[:, :, :], t16[:, 0:8, :], t16[:, 8:16, :])
            t4 = treep.tile([128, 4, WP], bf16, tag="tr4", name=f"s4_{g}_{k}")
            nc.vector.tensor_add(t4[:, :, :], t8[:, 0:4, :], t8[:, 4:8, :])
            for j in range(3):
                acc = pooled[g][:, 3 * k + j:3 * k + j + 1]
                s = treep.tile([128, 4, 32], bf16, tag="wscr4", name=f"w{g}_{k}_{j}")
                nc.vector.tensor_scalar(s[:, :, :],
                                        t4[:, :, 2 + 32 * j:34 + 32 * j],
                                        1.0, None,
                                        op0=ALU.mult, op1=ALU.add, accum_out=acc)

        lvmax = [sb(f"lvmax{g}", [128, 12, WP], bf16) for g in range(3)]

        def emit_band_max(g, k):
            # bf16 TT max-tree 32->16->8->4 rows into lvmax[g][:, 4k:4k+4]
            p = padA[g]
            r0 = 1 + 32 * k
            t16 = treep.tile([128, 16, WP], bf16, tag="tr16", name=f"m16_{g}_{k}")
            nc.vector.tensor_tensor(out=t16[:, :, :], in0=p[:, r0:r0 + 16, :],
                                    in1=p[:, r0 + 16:r0 + 32, :], op=ALU.max)
            t8 = treep.tile([128, 8, WP], bf16, tag="tr8", name=f"m8_{g}_{k}")
            nc.vector.tensor_tensor(out=t8[:, :, :], in0=t16[:, 0:8, :],
                                    in1=t16[:, 8:16, :], op=ALU.max)
            nc.vector.tensor_tensor(out=lvmax[g][:, 4 * k:4 * k + 4, :],
                                    in0=t8[:, 0:4, :], in1=t8[:, 4:8, :], op=ALU.max)

        def emit_mx_fin(g):
            t6 = treep.tile([128, 6, WP], bf16, tag="tr6", name=f"mf6_{g}")
            nc.vector.tensor_tensor(out=t6[:, :, :], in0=lvmax[g][:, 0:6, :],
                                    in1=lvmax[g][:, 6:12, :], op=ALU.max)
            t3 = treep.tile([128, 3, WP], bf16, tag="tr3", name=f"mf3_{g}")
            nc.vector.tensor_tensor(out=t3[:, :, :], in0=t6[:, 0:3, :],
                                    in1=t6[:, 3:6, :], op=ALU.max)
            nc.vector.tensor_reduce(out=mx[g][:, :], in_=t3[:, :, :],
                                    axis=mybir.AxisListType.XY, op=ALU.max)

        def emit_avg_fin(g):
            asc = scr.tile([128, 9], bf16, tag="ascr", name=f"avg{g}")
            nc.scalar.activation(out=asc[:, :], in_=pooled[g][:, :],
                                 func=ACTF.Copy, accum_out=avgs[g][:, :])

        # ---------------- per-sample algebra ----------------
        def emit_sample(b, part):
            if b == 0:
                chunks = [
                    (w1avg_a[:, :], w1mx_a[:, :], p1a[:, :], (0, 0, 128)),
                    (w1avg_b[0:64, :], w1mx_b[0:64, :], p1b[0:64, :], (2, 0, 64)),
                ]
            else:
                chunks = [
                    (w1avg_a[:, :], w1mx_a[:, :], p1a[:, :], (1, 0, 128)),
                    (w1avg_b[64:128, :], w1mx_b[64:128, :], p1b[64:128, :], (2, 64, 128)),
                ]
            base = 8 + b * 22
            for i, (wa, wm, wp, (sg, q0, q1)) in enumerate(chunks):
                o = base + 11 * i
                if part == "pool":
                    nc.tensor.matmul(stpa[0:R, o + 2:o + 11], lhsT=wp, rhs=pooled[sg][q0:q1, :], start=True, stop=True)
                else:
                    nc.tensor.matmul(stpa[0:R, o:o + 1], lhsT=wa, rhs=avgs[sg][q0:q1, :], start=True, stop=True)
                    nc.tensor.matmul(stpa[0:R, o + 1:o + 2], lhsT=wm, rhs=mx[sg][q0:q1, :], start=True, stop=True)

        def emit_fold_pool(b):
            base = 8 + b * 22
            hc = scr.tile([R, 9], f32, tag="scr48", name=f"hc{b}")
            nc.vector.tensor_copy(hc[:, :], stpa[0:R, base + 2:base + 11])
            hs = scr.tile([R, 9], f32, tag="scr48", name=f"hs{b}")
            nc.vector.tensor_add(hs[:, :], hc[:, :],
                                 stpa[0:R, base + 13:base + 22])
            t1 = scr.tile([R, 9], f32, tag="scr48", name=f"bn{b}")
            nc.vector.tensor_scalar(t1[:, :], hs[:, :], bns[:, :], bnb[:, :],
                                    op0=ALU.mult, op1=ALU.add)
            nc.vector.tensor_scalar_max(h_adk[b][:, :], t1[:, :], 0.0)

        def emit_fold_theta(b):
            base = 8 + b * 22
            hg = scr.tile([R, 2], f32, tag="scr2", name=f"hg{b}")
            nc.vector.tensor_copy(hg[:, :], stpa[0:R, base:base + 2])
            hs = scr.tile([R, 2], f32, tag="scr2", name=f"ht{b}")
            nc.vector.tensor_add(hs[:, :], hg[:, :],
                                 stpa[0:R, base + 11:base + 13])
            ha = scr.tile([R, 1], f32, tag="scr1", name=f"ha{b}")
            hm = scr.tile([R, 1], f32, tag="scr1", name=f"hm{b}")
            nc.vector.tensor_scalar_max(ha[:, :], hs[:, 0:1], 0.0)
            nc.vector.tensor_scalar_max(hm[:, :], hs[:, 1:2], 0.0)
            nc.vector.tensor_add(hsum[b][:, :], ha[:, :], hm[:, :])

        # ---------------- theta ----------------
        ps_t = [stpa[:, i:i + 1] for i in range(3)]

        def emit_theta_mm(b):
            nc.tensor.matmul(ps_t[b], lhsT=w2t[:, 0:128], rhs=hsum[b][:, :], start=True, stop=True)
            q0, q1 = (0, 64) if b == 0 else (64, 128)
            nc.tensor.matmul(ps_t[2][q0:q1], lhsT=w2t[:, 128:192], rhs=hsum[b][:, :], start=True, stop=True)

        def emit_theta_fin(g):
            et = scr.tile([128, 1], f32, tag="scr1", name=f"et{g}")
            nc.scalar.activation(out=et[:, :], in_=ps_t[g], func=ACTF.Exp, scale=-1.0)
            d = scr.tile([128, 1], f32, tag="scr1", name=f"etd{g}")
            nc.vector.tensor_scalar_add(d[:, :], et[:, :], 1.0)
            nc.vector.reciprocal(th[g][:, :], d[:, :])

        # ---------------- dynamic kernels w9 ----------------
        def emit_w9_mm(g):
            ps_s = stpa[:, 64 + g * 36:64 + (g + 1) * 36]
            for gg in range(G):
                sl = slice(gg * 9, gg * 9 + 9)
                if g < 2:
                    nc.tensor.matmul(ps_s[:, sl], lhsT=w2s[:, gg * 192:gg * 192 + 128],
                                     rhs=h_adk[g][:, :], start=True, stop=True)
                else:
                    nc.tensor.matmul(ps_s[0:64, sl], lhsT=w2s[:, gg * 192 + 128:gg * 192 + 192],
                                     rhs=h_adk[0][:, :], start=True, stop=True)
                    nc.tensor.matmul(ps_s[64:128, sl], lhsT=w2s[:, gg * 192 + 128:gg * 192 + 192],
                                     rhs=h_adk[1][:, :], start=True, stop=True)

        def emit_w9_exp(g):
            ps_s = stpa[:, 64 + g * 36:64 + (g + 1) * 36]
            e = scr.tile([128, 36], f32, tag="scr36", name=f"e{g}")
            nc.scalar.activation(out=e[:, :], in_=ps_s, func=ACTF.Exp)
            return e

        def emit_w9_fin(g, e):
            d1 = scr.tile([128, 9], f32, tag="scr9", name=f"d1_{g}")
            d2 = scr.tile([128, 9], f32, tag="scr9", name=f"d2_{g}")
            nc.vector.tensor_add(d1[:, :], e[:, 0:9], e[:, 9:18])
            nc.vector.tensor_add(d2[:, :], e[:, 18:27], e[:, 27:36])
            nc.vector.tensor_add(d1[:, :], d1[:, :], d2[:, :])
            rec = scr.tile([128, 9], f32, tag="scr9", name=f"rec{g}")
            nc.vector.reciprocal(rec[:, :], d1[:, :])
            a = adkT[g]
            m1 = scr.tile([128, 9], f32, tag="scr9", name=f"m1_{g}")
            m2 = scr.tile([128, 9], f32, tag="scr9", name=f"m2_{g}")
            nc.vector.tensor_mul(m1[:, :], e[:, 0:9], a[:, 0:9])
            nc.vector.tensor_mul(m2[:, :], e[:, 9:18], a[:, 9:18])
            nc.vector.tensor_add(m1[:, :], m1[:, :], m2[:, :])
            nc.vector.tensor_mul(m2[:, :], e[:, 18:27], a[:, 18:27])
            nc.vector.tensor_add(m1[:, :], m1[:, :], m2[:, :])
            nc.vector.tensor_mul(m2[:, :], e[:, 27:36], a[:, 27:36])
            nc.vector.tensor_add(m1[:, :], m1[:, :], m2[:, :])
            nc.vector.tensor_mul(w9[g][:, :], m1[:, :], rec[:, :])
            nc.vector.tensor_reduce(out=wsum9[g][:, :], in_=w9[g][:, :],
                                    axis=mybir.AxisListType.X, op=ALU.add)

        def emit_w4p(g):
            t1 = scr.tile([128, 1], f32, tag="scr1", name=f"t1_{g}")
            nc.vector.tensor_mul(t1[:, :], w9[g][:, 4:5], th[g][:, :])
            nc.vector.tensor_add(t1[:, :], t1[:, :], w9[g][:, 4:5])
            nc.vector.tensor_sub(w4p[g][:, :], t1[:, :], wsum9[g][:, :])

        def emit_diag(g, taps, engine):
            for tap in taps:
                scal = w4p[g][:, 0:1] if tap == 4 else w9[g][:, tap:tap + 1]
                if engine == "vector":
                    nc.vector.tensor_scalar_mul(diag[g][:, tap, :], eye[:, :], scal)
                else:
                    nc.scalar.activation(out=diag[g][:, tap, :], in_=eye[:, :],
                                         func=ACTF.Copy, scale=scal)

        # ---------------- conv on PE ----------------
        # window w of group g covers rows win_r0 .. win_r0+5*banks
        win_r0 = {}
        for g in range(3):
            r = 0
            for w, nb in enumerate(WIN_SEQ[g]):
                win_r0[(g, w)] = r
                r += 5 * nb

        pools = {4: psA, 3: psB}
        win_tile = {}
        out_rr = [0]

        def conv_taps(g, w, taps):
            nb = WIN_SEQ[g][w]
            r0 = win_r0[(g, w)]
            key = (g, w)
            if key not in win_tile:
                win_tile[key] = pools[nb].tile(
                    [128, nb, 512], f32, tag=f"w{nb}", name=f"ps{g}_{w}")
            ps = win_tile[key]
            for tap in taps:
                dy, dx = divmod(tap, 3)
                for b in range(nb):
                    y0 = r0 + 5 * b + dy
                    nc.tensor.matmul(
                        ps[:, b, 0:480],
                        lhsT=diag[g][:, tap, :],
                        rhs=padA[g][:, y0:y0 + 5, dx + 1:dx + 97],
                        start=(tap == taps[0]), stop=(tap == taps[-1]),
                    )

        def conv_drain(g, w):
            nb = WIN_SEQ[g][w]
            r0 = win_r0[(g, w)]
            ps = win_tile.pop((g, w))
            nr = 5 * nb
            ot = osb_pool.tile([128, 20, W], bf16, tag="ow", name=f"ow{g}_{w}")
            nc.scalar.activation(
                out=ot[:, 0:nr, :],
                in_=ps[:, 0:nb, 0:480], func=ACTF.Copy)
            eng = (nc.sync, nc.gpsimd)[out_rr[0] % 2]
            out_rr[0] += 1
            eng.dma_start(
                out=out_d[g * 128:(g + 1) * 128, r0:r0 + nr, :],
                in_=ot[:, 0:nr, :])

        # ---------------- conv tail on DVE ----------------
        def emit_conv_dve(g, lo, hi):
            # DVE tail rows: flat contiguous strips (full padded rows, junk
            # at pad columns) so the muls hit the 4x DVE mode; only the
            # final add back into tailb is strided.
            y0 = ROWS_PE[g] + lo
            n = hi - lo
            L = (n - 1) * 100 + 96
            acc = None
            for i, tap in enumerate(TAP_ORDER):
                dy, dx = divmod(tap, 3)
                scal = w4p[g][:, 0:1] if tap == 4 else w9[g][:, tap:tap + 1]
                if dx == 1:
                    o0 = (y0 + dy) * 100 + 2
                    strip = padAf[g][:, o0:o0 + L]
                else:
                    o0 = (y0 + dy - XB_R0) * 100 + (2 if dx == 0 else 4)
                    strip = padBf[g][:, o0:o0 + L]
                t = term_pool.tile([128, 14, 100], bf16, tag="term",
                                   name=f"t{g}_{lo}_{i}")
                tf = t.rearrange("p a b -> p (a b)")
                nc.vector.tensor_scalar_mul(tf[:, 0:L], strip, scal)
                if i == 0:
                    acc = t
                elif i < 8:
                    nxt = term_pool.tile([128, 14, 100], bf16, tag="term",
                                         name=f"a{g}_{lo}_{i}")
                    nxf = nxt.rearrange("p a b -> p (a b)")
                    nc.vector.tensor_add(nxf[:, 0:L],
                                         acc.rearrange("p a b -> p (a b)")[:, 0:L],
                                         tf[:, 0:L])
                    acc = nxt
                else:
                    nc.vector.tensor_add(tailb[g][:, lo:hi, :],
                                         acc[:, 0:n, 0:96], t[:, 0:n, 0:96])
            eng = (nc.sync, nc.gpsimd)[out_rr[0] % 2]
            out_rr[0] += 1
            eng.dma_start(out=out_d[g * 128:(g + 1) * 128, y0:y0 + n, :],
                          in_=tailb[g][:, lo:hi, :])

        # ---------------- PE warm-up ----------------
        # junk matmuls gated on successive DMA chunks so the PE stays busy
        # (HAM warm) across the whole stats prelude without running eagerly
        def emit_warmup(k, gate, drain=False):
            for j in range(k):
                nc.tensor.matmul(stpa[:, 384:512], lhsT=eye[:, :],
                                 rhs=gate, start=True, stop=True)
            if drain:
                wsc = scr.tile([128, 1], bf16, tag="wscr1", name="wscr")
                nc.scalar.activation(out=wsc[:, :], in_=stpa[:, 384:385], func=ACTF.Copy)
                nc.sync.dma_start(out=warm_d, in_=wsc[:, :])

        # ---------------- emission order ----------------
        NC8 = TAP_ORDER[:8]

        # prelude: window sums first (pooled -> w9 -> conv is the critical
        # path; image-max/theta only gates the post-drain center-tap add)
        for k in range(3):
            emit_band_windows(0, k, engine="vector")
        emit_band_windows(2, 0, engine="scalar")
        emit_band_windows(2, 1, engine="scalar")
        emit_band_windows(2, 2, engine="vector")
        emit_avg_fin(0)
        emit_avg_fin(2)
        emit_xB_dma(0)
        emit_xB_dma(1)
        emit_xB_dma(2)
        emit_warmup(30, padA[0][:, 65:67, 0:64])
        emit_warmup(30, padA[2][:, 65:67, 0:64])
        emit_sample(0, "pool")
        emit_fold_pool(0)
        emit_w9_mm(0)
        e0 = emit_w9_exp(0)
        emit_w9_fin(0, e0)
        emit_diag(0, NC8, "vector")
        for k in range(3):
            emit_band_max(0, k)
            emit_band_max(2, k)
        emit_mx_fin(0)
        emit_mx_fin(2)
        emit_warmup(24, padA[1][:, 10:12, 0:64])
        emit_warmup(16, padA[1][:, 40:42, 0:64], drain=True)
        emit_sample(0, "theta")
        emit_fold_theta(0)
        emit_theta_mm(0)
        emit_theta_fin(0)
        emit_w4p(0)
        emit_diag(0, [4], "vector")

        conv_taps(0, 0, TAP_ORDER)
        conv_taps(0, 1, TAP_ORDER)

        # g1 stats (data lands mid-conv-g0); sums on ScalarE (slack there)
        for k in range(3):
            emit_band_windows(1, k, engine="scalar")
            emit_band_max(1, k)
        emit_mx_fin(1)
        emit_avg_fin(1)

        conv_drain(0, 0)
        conv_taps(0, 2, TAP_ORDER)
        emit_sample(1, "pool")
        emit_fold_pool(1)
        emit_sample(1, "theta")
        emit_fold_theta(1)
        emit_theta_mm(1)
        emit_theta_fin(1)
        emit_w9_mm(1)
        e1 = emit_w9_exp(1)
        emit_w9_fin(1, e1)
        emit_w4p(1)
        emit_diag(1, TAP_ORDER, "scalar")
        conv_drain(0, 1)
        conv_taps(0, 3, TAP_ORDER)
        emit_theta_fin(2)
        emit_w9_mm(2)
        e2 = emit_w9_exp(2)
        emit_w9_fin(2, e2)
        emit_w4p(2)
        emit_diag(2, TAP_ORDER, "scalar")
        conv_drain(0, 2)
        conv_taps(1, 0, TAP_ORDER)
        conv_drain(0, 3)
        emit_conv_dve(0, 0, 13)
        conv_taps(1, 1, TAP_ORDER)
        conv_drain(1, 0)
        emit_conv_dve(0, 13, 26)
        conv_taps(1, 2, TAP_ORDER)
        conv_drain(1, 1)
        emit_conv_dve(1, 0, 13)
        conv_taps(1, 3, TAP_ORDER)
        conv_drain(1, 2)
        emit_conv_dve(1, 13, 26)
        conv_taps(2, 0, TAP_ORDER)
        conv_drain(1, 3)
        conv_taps(2, 1, TAP_ORDER)
        conv_drain(2, 0)
        conv_taps(2, 2, TAP_ORDER)
        conv_drain(2, 1)
        emit_conv_dve(2, 0, 6)
        conv_taps(2, 3, TAP_ORDER)
        conv_drain(2, 2)
        conv_taps(2, 4, TAP_ORDER)
        conv_drain(2, 3)
        conv_drain(2, 4)

    nc.compile()
    return nc


def _host_prep(inputs):
    x = np.ascontiguousarray(inputs["x"], dtype=np.float32)
    cam_w1 = np.asarray(inputs["cam_w1"], dtype=np.float32)
    cam_w2 = np.asarray(inputs["cam_w2"], dtype=np.float32)
    proj_w1 = np.asarray(inputs["proj_w1"], dtype=np.float32)
    bn_gamma = np.asarray(inputs["bn_gamma"], dtype=np.float32)
    bn_beta = np.asarray(inputs["bn_beta"], dtype=np.float32)
    proj_w2 = np.asarray(inputs["proj_w2"], dtype=np.float32)
    adk = np.asarray(inputs["adk_weight"], dtype=np.float32)

    xb16 = x.astype(BF16)
    xpA = np.zeros((B, C, HP, WP), dtype=BF16)
    xpA[:, :, 1:97, 2:98] = xb16
    # padB: x payload at col 3, rows = padded rows 50..97 (x rows 49..95)
    xpB = np.zeros((B, C, XB_NR, WP), dtype=BF16)
    xpB[:, :, 0:47, 3:99] = xb16[:, :, 49:96, :]

    in_maps = []
    w1t = cam_w1.T.astype(np.float32)
    p1t = (proj_w1.T / 1024.0).astype(np.float32)
    cmap = np.concatenate([np.arange(128), np.arange(128),
                           np.arange(128, 192), np.arange(128, 192)])
    consts = {
        "eye": np.eye(128, dtype=BF16),
        "w1avg_a": np.ascontiguousarray(w1t[0:128] / (H * W)),
        "w1avg_b": np.ascontiguousarray(np.concatenate([w1t[128:192] / (H * W)] * 2, axis=0)),
        "w1mx_a": np.ascontiguousarray(w1t[0:128]),
        "w1mx_b": np.ascontiguousarray(np.concatenate([w1t[128:192]] * 2, axis=0)),
        "w2t": np.ascontiguousarray(cam_w2.T.astype(np.float32)),
        "p1a": np.ascontiguousarray(p1t[0:128]),
        "p1b": np.ascontiguousarray(np.concatenate([p1t[128:192]] * 2, axis=0)),
        "bn_scale": np.ascontiguousarray((bn_gamma / np.sqrt(1.0 + BN_EPS)).reshape(R, 1)),
        "bn_beta": np.ascontiguousarray(bn_beta.reshape(R, 1)),
        "w2s": np.ascontiguousarray(proj_w2.T.astype(np.float32)),
        "adkT": np.ascontiguousarray(
            adk.transpose(1, 0, 2, 3).reshape(C, G * 9)[cmap].astype(np.float32)
        ),
    }
    for k in range(N_CORES):
        b0, b1 = 2 * k, 2 * k + 1
        shardA = np.ascontiguousarray(np.concatenate(
            [xpA[b0, 0:128], xpA[b1, 0:128], xpA[b0, 128:192], xpA[b1, 128:192]],
            axis=0))
        shardB = np.ascontiguousarray(np.concatenate(
            [xpB[b0, 0:128], xpB[b1, 0:128], xpB[b0, 128:192], xpB[b1, 128:192]],
            axis=0))
        m = {"xA": shardA, "xB": shardB}
        m.update(consts)
        in_maps.append(m)
    return in_maps


def kernel(**inputs) -> np.ndarray:
    global _COMPILED
    from concourse.bass_utils import run_bass_kernel_spmd

    in_maps = _host_prep(inputs)

    if _COMPILED is None:
        _COMPILED = _build()
    nc = _COMPILED

    res = run_bass_kernel_spmd(nc, in_maps, core_ids=list(range(N_CORES)))
    outs = [r["out"] for r in res.results]

    y = np.empty((B, C, H, W), np.float32)
    for k in range(N_CORES):
        o = np.asarray(outs[k]).reshape(384, H, W).astype(np.float32)
        b0, b1 = 2 * k, 2 * k + 1
        y[b0, 0:128] = o[0:128]
        y[b1, 0:128] = o[128:256]
        y[b0, 128:192] = o[256:320]
        y[b1, 128:192] = o[320:384]
    return y


if __name__ == "__main__":
    import reference

    inputs = {k: np.asarray(v) for k, v in reference.setup_inputs().items()}
    y = kernel(**inputs)
    print("kernel output:", y.shape, y.dtype)



# revision 17
# speedup vs baseline: 1.3063x; 1.0047x over previous
"""Trainium2 Bass kernel for nn_ADSCDConv (dense_cnn), 8-core data parallel.

Per core (2 samples = 384 (b,c) channel-images of 96x96), groups of 128
partitions: g0=(b0,c0:128), g1=(b1,c0:128), g2=(b0,c128:192)||(b1,c128:192).

v2 schedule (vs v1):
  - tap-outer conv matmuls: LDWEIGHTS amortized over a multi-bank PSUM
    window; PE runs at the 202ns/FD480 streaming roofline.
  - center tap (the only theta-dependent one) is emitted LAST per window
    and deferred for the first two windows of g0, so the conv starts on
    the pooled-only dependency chain while the image-max/theta chain
    finishes.
  - windows/bandmax stats are banded (32 rows) and pipelined with the
    input DMA; x lands padded to width 100 with the payload at col 2 so
    window sums hit the DVE 4x mode.
  - DVE conv tail uses fused scalar_tensor_tensor (mul+add in one op);
    a second x copy shifted by one column (padB) keeps all taps 4B
    aligned for the 2x bf16 mode.
  - PSUM: two conv window pools (4 banks + 3 banks) alternate A,B,A,...
    globally across groups; 1 stats bank.
"""

from contextlib import ExitStack

import numpy as np
import ml_dtypes

BF16 = ml_dtypes.bfloat16

B, C, H, W = 16, 192, 96, 96
G = 4
R = C // 4  # 48
BN_EPS = 1e-5
N_CORES = 8
HP, WP = H + 2, 100  # padded rows 98, padded cols 100 (x payload at col 2)
XB_R0 = 50           # padB covers padded rows 50..97
XB_NR = 48

# conv windows: banks per window, alternating pool A(4)/B(3) globally
# (sequence across groups must alternate 4,3,4,3,... for PSUM pool reuse)
WIN_SEQ = {0: [4, 3, 4, 2], 1: [4, 3, 4, 2], 2: [4, 3, 4, 3]}
ROWS_PE = {g: 5 * sum(WIN_SEQ[g]) for g in range(3)}  # 65, 65, 70

# tap order: center (tap 4, the only theta-dependent tap) last
TAP_ORDER = [0, 1, 2, 3, 5, 6, 7, 8, 4]

_COMPILED = None


def _build():
    import concourse.tile as tile
    from concourse import bacc, mybir

    f32 = mybir.dt.float32
    bf16 = mybir.dt.bfloat16
    ALU = mybir.AluOpType
    ACTF = mybir.ActivationFunctionType

    nc = bacc.Bacc("TRN2", target_bir_lowering=False, debug=False, num_devices=N_CORES)

    # ---- DRAM tensors ----
    xA_d = nc.dram_tensor("xA", [384, HP, WP], bf16, kind="ExternalInput").ap()
    xB_d = nc.dram_tensor("xB", [384, XB_NR, WP], bf16, kind="ExternalInput").ap()
    out_d = nc.dram_tensor("out", [384, H, W], bf16, kind="ExternalOutput").ap()
    warm_d = nc.dram_tensor("warm", [128, 1], bf16, kind="ExternalOutput").ap()
    eye_d = nc.dram_tensor("eye", [128, 128], bf16, kind="ExternalInput").ap()
    w1avg_a_d = nc.dram_tensor("w1avg_a", [128, R], f32, kind="ExternalInput").ap()
    w1avg_b_d = nc.dram_tensor("w1avg_b", [128, R], f32, kind="ExternalInput").ap()
    w1mx_a_d = nc.dram_tensor("w1mx_a", [128, R], f32, kind="ExternalInput").ap()
    w1mx_b_d = nc.dram_tensor("w1mx_b", [128, R], f32, kind="ExternalInput").ap()
    w2t_d = nc.dram_tensor("w2t", [R, C], f32, kind="ExternalInput").ap()
    p1a_d = nc.dram_tensor("p1a", [128, R], f32, kind="ExternalInput").ap()
    p1b_d = nc.dram_tensor("p1b", [128, R], f32, kind="ExternalInput").ap()
    bns_d = nc.dram_tensor("bn_scale", [R, 1], f32, kind="ExternalInput").ap()
    bnb_d = nc.dram_tensor("bn_beta", [R, 1], f32, kind="ExternalInput").ap()
    w2s_d = nc.dram_tensor("w2s", [R, G * C], f32, kind="ExternalInput").ap()
    adkT_d = nc.dram_tensor("adkT", [384, 36], f32, kind="ExternalInput").ap()

    with tile.TileContext(nc) as tc, ExitStack() as ctx:
        def sb(name, shape, dt):
            return nc.alloc_sbuf_tensor(name, shape, dt).ap()

        padA = [sb(f"padA{g}", [128, HP, WP], bf16) for g in range(3)]
        padB = [sb(f"padB{g}", [128, XB_NR, WP], bf16) for g in range(3)]
        padAf = [p.rearrange("p a b -> p (a b)") for p in padA]
        padBf = [p.rearrange("p a b -> p (a b)") for p in padB]
        tailb = [sb(f"tail{g}", [128, 31, W], bf16) for g in range(3)]
        diag = [sb(f"diag{g}", [128, 9, 128], bf16) for g in range(3)]
        pooled = [sb(f"pooled{g}", [128, 9], f32) for g in range(3)]
        avgs = [sb(f"avgs{g}", [128, 1], f32) for g in range(3)]
        mx = [sb(f"mx{g}", [128, 1], f32) for g in range(3)]
        th = [sb(f"theta{g}", [128, 1], f32) for g in range(3)]
        w9 = [sb(f"w9_{g}", [128, 9], f32) for g in range(3)]
        w4p = [sb(f"w4p{g}", [128, 1], f32) for g in range(3)]
        wsum9 = [sb(f"wsum9_{g}", [128, 1], f32) for g in range(3)]
        adkT = [sb(f"adkT{g}_sb", [128, 36], f32) for g in range(3)]

        eye = sb("eye_sb", [128, 128], bf16)
        w1avg_a = sb("w1avg_a_sb", [128, R], f32)
        w1avg_b = sb("w1avg_b_sb", [128, R], f32)
        w1mx_a = sb("w1mx_a_sb", [128, R], f32)
        w1mx_b = sb("w1mx_b_sb", [128, R], f32)
        w2t = sb("w2t_sb", [R, C], f32)
        p1a = sb("p1a_sb", [128, R], f32)
        p1b = sb("p1b_sb", [128, R], f32)
        bns = sb("bns_sb", [R, 1], f32)
        bnb = sb("bnb_sb", [R, 1], f32)
        w2s = sb("w2s_sb", [R, G * C], f32)

        h_adk = [sb(f"h_adk{b}", [R, 9], f32) for b in range(2)]
        hsum = [sb(f"hsum{b}", [R, 1], f32) for b in range(2)]

        scr = ctx.enter_context(tc.tile_pool(name="scr", bufs=4))
        treep = ctx.enter_context(tc.tile_pool(name="treep", bufs=2))
        term_pool = ctx.enter_context(tc.tile_pool(name="terms", bufs=3))
        osb_pool = ctx.enter_context(tc.tile_pool(name="osbp", bufs=8))
        ct_pool = ctx.enter_context(tc.tile_pool(name="ctp", bufs=3))
        psA = ctx.enter_context(tc.tile_pool(name="psA", bufs=1, space="PSUM"))
        psB = ctx.enter_context(tc.tile_pool(name="psB", bufs=1, space="PSUM"))
        # stats bank: single-shot matmul groups only may share a bank
        stpa = nc.alloc_psum_tensor("statps", [128, 512], f32).ap()

        # ---------------- DMA emission ----------------
        row_chunks = [(0, 33), (33, 65), (65, HP)]

        def emit_xA_dma(g):
            for (r0, r1) in row_chunks:
                nc.sync.dma_start(
                    out=padA[g][:, r0:r1, :],
                    in_=xA_d[g * 128:(g + 1) * 128, r0:r1, :],
                )

        def emit_xB_dma(g):
            # WAW gate: tiny DVE write into padB dependent on g1's last xA
            # chunk keeps the xB transfer out of the critical input window
            nc.vector.tensor_copy(padB[g][:, 0:1, 0:2], padA[1][:, 97:98, 0:2])
            nc.scalar.dma_start(
                out=padB[g][:, :, :],
                in_=xB_d[g * 128:(g + 1) * 128, :, :],
            )

        emit_xA_dma(0)
        wloads = [
            (eye, eye_d), (w1avg_a, w1avg_a_d), (w1avg_b, w1avg_b_d),
            (w1mx_a, w1mx_a_d), (w1mx_b, w1mx_b_d), (w2t, w2t_d),
            (p1a, p1a_d), (p1b, p1b_d), (bns, bns_d), (bnb, bnb_d),
            (w2s, w2s_d),
            (adkT[0], adkT_d[0:128, :]), (adkT[1], adkT_d[128:256, :]),
            (adkT[2], adkT_d[256:384, :]),
        ]
        for (dst, src) in wloads:
            nc.gpsimd.dma_start(out=dst, in_=src)
        emit_xA_dma(2)
        emit_xA_dma(1)

        # ---------------- stats ----------------
        def emit_band_windows(g, k, engine="vector"):
            # 3 col-window sums of the 32-row band k -> pooled[g][:, 3k+j]
            if engine == "scalar":
                for j in range(3):
                    win = padA[g][:, 1 + 32 * k:33 + 32 * k, 2 + 32 * j:34 + 32 * j]
                    acc = pooled[g][:, 3 * k + j:3 * k + j + 1]
                    s = treep.tile([128, 32, 32], bf16, tag="wscr", name=f"w{g}_{k}_{j}")
                    nc.scalar.activation(out=s[:, :, :], in_=win,
                                         func=ACTF.Copy, accum_out=acc)
                return
            # DVE: bf16 TT add-tree 32->16->8->4 rows (2x mode), then 3
            # cache-reduce window sums over the 4 leaf rows
            p = padA[g]
            r0 = 1 + 32 * k
            t16 = treep.tile([128, 16, WP], bf16, tag="tr16", name=f"s16_{g}_{k}")
            nc.vector.tensor_add(t16[:, :, :], p[:, r0:r0 + 16, :], p[:, r0 + 16:r0 + 32, :])
            t8 = treep.tile([128, 8, WP], bf16, tag="tr8", name=f"s8_{g}_{k}")
            nc.vector.tensor_add(t8[:, :, :], t16[:, 0:8, :], t16[:, 8:16, :])
            t4 = treep.tile([128, 4, WP], bf16, tag="tr4", name=f"s4_{g}_{k}")
            nc.vector.tensor_add(t4[:, :, :], t8[:, 0:4, :], t8[:, 4:8, :])
            for j in range(3):
                acc = pooled[g][:, 3 * k + j:3 * k + j + 1]
                s = treep.tile([128, 4, 32], bf16, tag="wscr4", name=f"w{g}_{k}_{j}")
                nc.vector.tensor_scalar(s[:, :, :],
                                        t4[:, :, 2 + 32 * j:34 + 32 * j],
                                        1.0, None,
                                        op0=ALU.mult, op1=ALU.add, accum_out=acc)

        lvmax = [sb(f"lvmax{g}", [128, 12, WP], bf16) for g in range(3)]

        def emit_band_max(g, k):
            # bf16 TT max-tree 32->16->8->4 rows into lvmax[g][:, 4k:4k+4]
            p = padA[g]
            r0 = 1 + 32 * k
            t16 = treep.tile([128, 16, WP], bf16, tag="tr16", name=f"m16_{g}_{k}")
            nc.vector.tensor_tensor(out=t16[:, :, :], in0=p[:, r0:r0 + 16, :],
                                    in1=p[:, r0 + 16:r0 + 32, :], op=ALU.max)
            t8 = treep.tile([128, 8, WP], bf16, tag="tr8", name=f"m8_{g}_{k}")
            nc.vector.tensor_tensor(out=t8[:, :, :], in0=t16[:, 0:8, :],
                                    in1=t16[:, 8:16, :], op=ALU.max)
            nc.vector.tensor_tensor(out=lvmax[g][:, 4 * k:4 * k + 4, :],
                                    in0=t8[:, 0:4, :], in1=t8[:, 4:8, :], op=ALU.max)

        def emit_mx_fin(g):
            t6 = treep.tile([128, 6, WP], bf16, tag="tr6", name=f"mf6_{g}")
            nc.vector.tensor_tensor(out=t6[:, :, :], in0=lvmax[g][:, 0:6, :],
                                    in1=lvmax[g][:, 6:12, :], op=ALU.max)
            t3 = treep.tile([128, 3, WP], bf16, tag="tr3", name=f"mf3_{g}")
            nc.vector.tensor_tensor(out=t3[:, :, :], in0=t6[:, 0:3, :],
                                    in1=t6[:, 3:6, :], op=ALU.max)
            nc.vector.tensor_reduce(out=mx[g][:, :], in_=t3[:, :, :],
                                    axis=mybir.AxisListType.XY, op=ALU.max)

        def emit_avg_fin(g):
            asc = scr.tile([128, 9], bf16, tag="ascr", name=f"avg{g}")
            nc.scalar.activation(out=asc[:, :], in_=pooled[g][:, :],
                                 func=ACTF.Copy, accum_out=avgs[g][:, :])

        # ---------------- per-sample algebra ----------------
        def emit_sample(b, part):
            if b == 0:
                chunks = [
                    (w1avg_a[:, :], w1mx_a[:, :], p1a[:, :], (0, 0, 128)),
                    (w1avg_b[0:64, :], w1mx_b[0:64, :], p1b[0:64, :], (2, 0, 64)),
                ]
            else:
                chunks = [
                    (w1avg_a[:, :], w1mx_a[:, :], p1a[:, :], (1, 0, 128)),
                    (w1avg_b[64:128, :], w1mx_b[64:128, :], p1b[64:128, :], (2, 64, 128)),
                ]
            base = 8 + b * 22
            for i, (wa, wm, wp, (sg, q0, q1)) in enumerate(chunks):
                o = base + 11 * i
                if part == "pool":
                    nc.tensor.matmul(stpa[0:R, o + 2:o + 11], lhsT=wp, rhs=pooled[sg][q0:q1, :], start=True, stop=True)
                else:
                    nc.tensor.matmul(stpa[0:R, o:o + 1], lhsT=wa, rhs=avgs[sg][q0:q1, :], start=True, stop=True)
                    nc.tensor.matmul(stpa[0:R, o + 1:o + 2], lhsT=wm, rhs=mx[sg][q0:q1, :], start=True, stop=True)

        def emit_fold_pool(b):
            base = 8 + b * 22
            hc = scr.tile([R, 9], f32, tag="scr48", name=f"hc{b}")
            nc.vector.tensor_copy(hc[:, :], stpa[0:R, base + 2:base + 11])
            hs = scr.tile([R, 9], f32, tag="scr48", name=f"hs{b}")
            nc.vector.tensor_add(hs[:, :], hc[:, :],
                                 stpa[0:R, base + 13:base + 22])
            t1 = scr.tile([R, 9], f32, tag="scr48", name=f"bn{b}")
            nc.vector.tensor_scalar(t1[:, :], hs[:, :], bns[:, :], bnb[:, :],
                                    op0=ALU.mult, op1=ALU.add)
            nc.vector.tensor_scalar_max(h_adk[b][:, :], t1[:, :], 0.0)

        def emit_fold_theta(b):
            base = 8 + b * 22
            hg = scr.tile([R, 2], f32, tag="scr2", name=f"hg{b}")
            nc.vector.tensor_copy(hg[:, :], stpa[0:R, base:base + 2])
            hs = scr.tile([R, 2], f32, tag="scr2", name=f"ht{b}")
            nc.vector.tensor_add(hs[:, :], hg[:, :],
                                 stpa[0:R, base + 11:base + 13])
            ha = scr.tile([R, 1], f32, tag="scr1", name=f"ha{b}")
            hm = scr.tile([R, 1], f32, tag="scr1", name=f"hm{b}")
            nc.vector.tensor_scalar_max(ha[:, :], hs[:, 0:1], 0.0)
            nc.vector.tensor_scalar_max(hm[:, :], hs[:, 1:2], 0.0)
            nc.vector.tensor_add(hsum[b][:, :], ha[:, :], hm[:, :])

        # ---------------- theta ----------------
        ps_t = [stpa[:, i:i + 1] for i in range(3)]

        def emit_theta_mm(b):
            nc.tensor.matmul(ps_t[b], lhsT=w2t[:, 0:128], rhs=hsum[b][:, :], start=True, stop=True)
            q0, q1 = (0, 64) if b == 0 else (64, 128)
            nc.tensor.matmul(ps_t[2][q0:q1], lhsT=w2t[:, 128:192], rhs=hsum[b][:, :], start=True, stop=True)

        def emit_theta_fin(g):
            et = scr.tile([128, 1], f32, tag="scr1", name=f"et{g}")
            nc.scalar.activation(out=et[:, :], in_=ps_t[g], func=ACTF.Exp, scale=-1.0)
            d = scr.tile([128, 1], f32, tag="scr1", name=f"etd{g}")
            nc.vector.tensor_scalar_add(d[:, :], et[:, :], 1.0)
            nc.vector.reciprocal(th[g][:, :], d[:, :])

        # ---------------- dynamic kernels w9 ----------------
        def emit_w9_mm(g):
            ps_s = stpa[:, 64 + g * 36:64 + (g + 1) * 36]
            for gg in range(G):
                sl = slice(gg * 9, gg * 9 + 9)
                if g < 2:
                    nc.tensor.matmul(ps_s[:, sl], lhsT=w2s[:, gg * 192:gg * 192 + 128],
                                     rhs=h_adk[g][:, :], start=True, stop=True)
                else:
                    nc.tensor.matmul(ps_s[0:64, sl], lhsT=w2s[:, gg * 192 + 128:gg * 192 + 192],
                                     rhs=h_adk[0][:, :], start=True, stop=True)
                    nc.tensor.matmul(ps_s[64:128, sl], lhsT=w2s[:, gg * 192 + 128:gg * 192 + 192],
                                     rhs=h_adk[1][:, :], start=True, stop=True)

        def emit_w9_exp(g):
            ps_s = stpa[:, 64 + g * 36:64 + (g + 1) * 36]
            e = scr.tile([128, 36], f32, tag="scr36", name=f"e{g}")
            nc.scalar.activation(out=e[:, :], in_=ps_s, func=ACTF.Exp)
            return e

        def emit_w9_fin(g, e):
            d1 = scr.tile([128, 9], f32, tag="scr9", name=f"d1_{g}")
            d2 = scr.tile([128, 9], f32, tag="scr9", name=f"d2_{g}")
            nc.vector.tensor_add(d1[:, :], e[:, 0:9], e[:, 9:18])
            nc.vector.tensor_add(d2[:, :], e[:, 18:27], e[:, 27:36])
            nc.vector.tensor_add(d1[:, :], d1[:, :], d2[:, :])
            rec = scr.tile([128, 9], f32, tag="scr9", name=f"rec{g}")
            nc.vector.reciprocal(rec[:, :], d1[:, :])
            a = adkT[g]
            m1 = scr.tile([128, 9], f32, tag="scr9", name=f"m1_{g}")
            m2 = scr.tile([128, 9], f32, tag="scr9", name=f"m2_{g}")
            nc.vector.tensor_mul(m1[:, :], e[:, 0:9], a[:, 0:9])
            nc.vector.tensor_mul(m2[:, :], e[:, 9:18], a[:, 9:18])
            nc.vector.tensor_add(m1[:, :], m1[:, :], m2[:, :])
            nc.vector.tensor_mul(m2[:, :], e[:, 18:27], a[:, 18:27])
            nc.vector.tensor_add(m1[:, :], m1[:, :], m2[:, :])
            nc.vector.tensor_mul(m2[:, :], e[:, 27:36], a[:, 27:36])
            nc.vector.tensor_add(m1[:, :], m1[:, :], m2[:, :])
            nc.vector.tensor_mul(w9[g][:, :], m1[:, :], rec[:, :])
            nc.vector.tensor_reduce(out=wsum9[g][:, :], in_=w9[g][:, :],
                                    axis=mybir.AxisListType.X, op=ALU.add)

        def emit_w4p(g):
            t1 = scr.tile([128, 1], f32, tag="scr1", name=f"t1_{g}")
            nc.vector.tensor_mul(t1[:, :], w9[g][:, 4:5], th[g][:, :])
            nc.vector.tensor_add(t1[:, :], t1[:, :], w9[g][:, 4:5])
            nc.vector.tensor_sub(w4p[g][:, :], t1[:, :], wsum9[g][:, :])

        def emit_diag(g, taps, engine):
            for tap in taps:
                scal = w4p[g][:, 0:1] if tap == 4 else w9[g][:, tap:tap + 1]
                if engine == "vector":
                    nc.vector.tensor_scalar_mul(diag[g][:, tap, :], eye[:, :], scal)
                else:
                    nc.scalar.activation(out=diag[g][:, tap, :], in_=eye[:, :],
                                         func=ACTF.Copy, scale=scal)

        # ---------------- conv on PE ----------------
        # window w of group g covers rows win_r0 .. win_r0+5*banks
        win_r0 = {}
        for g in range(3):
            r = 0
            for w, nb in enumerate(WIN_SEQ[g]):
                win_r0[(g, w)] = r
                r += 5 * nb

        pools = {4: psA, 3: psB}
        win_tile = {}
        out_rr = [0]

        def conv_taps(g, w, taps):
            nb = WIN_SEQ[g][w]
            r0 = win_r0[(g, w)]
            key = (g, w)
            if key not in win_tile:
                pnb = 4 if w % 2 == 0 else 3
                win_tile[key] = pools[pnb].tile(
                    [128, pnb, 512], f32, tag=f"w{pnb}", name=f"ps{g}_{w}")
            ps = win_tile[key]
            for tap in taps:
                dy, dx = divmod(tap, 3)
                for b in range(nb):
                    y0 = r0 + 5 * b + dy
                    nc.tensor.matmul(
                        ps[:, b, 0:480],
                        lhsT=diag[g][:, tap, :],
                        rhs=padA[g][:, y0:y0 + 5, dx + 1:dx + 97],
                        start=(tap == taps[0]), stop=(tap == taps[-1]),
                    )

        def conv_drain(g, w):
            nb = WIN_SEQ[g][w]
            r0 = win_r0[(g, w)]
            ps = win_tile.pop((g, w))
            nr = 5 * nb
            ot = osb_pool.tile([128, 20, W], bf16, tag="ow", name=f"ow{g}_{w}")
            nc.scalar.activation(
                out=ot[:, 0:nr, :],
                in_=ps[:, 0:nb, 0:480], func=ACTF.Copy)
            eng = (nc.sync, nc.gpsimd)[out_rr[0] % 2]
            out_rr[0] += 1
            eng.dma_start(
                out=out_d[g * 128:(g + 1) * 128, r0:r0 + nr, :],
                in_=ot[:, 0:nr, :])

        # ---------------- conv tail on DVE ----------------
        def emit_conv_dve(g, lo, hi):
            # DVE tail rows: flat contiguous strips (full padded rows, junk
            # at pad columns) so the muls hit the 4x DVE mode; only the
            # final add back into tailb is strided.
            y0 = ROWS_PE[g] + lo
            n = hi - lo
            L = (n - 1) * 100 + 96
            acc = None
            for i, tap in enumerate(TAP_ORDER):
                dy, dx = divmod(tap, 3)
                scal = w4p[g][:, 0:1] if tap == 4 else w9[g][:, tap:tap + 1]
                if dx == 1:
                    o0 = (y0 + dy) * 100 + 2
                    strip = padAf[g][:, o0:o0 + L]
                else:
                    o0 = (y0 + dy - XB_R0) * 100 + (2 if dx == 0 else 4)
                    strip = padBf[g][:, o0:o0 + L]
                t = term_pool.tile([128, 14, 100], bf16, tag="term",
                                   name=f"t{g}_{lo}_{i}")
                tf = t.rearrange("p a b -> p (a b)")
                nc.vector.tensor_scalar_mul(tf[:, 0:L], strip, scal)
                if i == 0:
                    acc = t
                elif i < 8:
                    nxt = term_pool.tile([128, 14, 100], bf16, tag="term",
                                         name=f"a{g}_{lo}_{i}")
                    nxf = nxt.rearrange("p a b -> p (a b)")
                    nc.vector.tensor_add(nxf[:, 0:L],
                                         acc.rearrange("p a b -> p (a b)")[:, 0:L],
                                         tf[:, 0:L])
                    acc = nxt
                else:
                    nc.vector.tensor_add(tailb[g][:, lo:hi, :],
                                         acc[:, 0:n, 0:96], t[:, 0:n, 0:96])
            eng = (nc.sync, nc.gpsimd)[out_rr[0] % 2]
            out_rr[0] += 1
            eng.dma_start(out=out_d[g * 128:(g + 1) * 128, y0:y0 + n, :],
                          in_=tailb[g][:, lo:hi, :])

        # ---------------- PE warm-up ----------------
        # junk matmuls gated on successive DMA chunks so the PE stays busy
        # (HAM warm) across the whole stats prelude without running eagerly
        def emit_warmup(k, gate, drain=False):
            for j in range(k):
                nc.tensor.matmul(stpa[:, 384:512], lhsT=eye[:, :],
                                 rhs=gate, start=True, stop=True)
            if drain:
                wsc = scr.tile([128, 1], bf16, tag="wscr1", name="wscr")
                nc.scalar.activation(out=wsc[:, :], in_=stpa[:, 384:385], func=ACTF.Copy)
                nc.sync.dma_start(out=warm_d, in_=wsc[:, :])

        # ---------------- emission order ----------------
        NC8 = TAP_ORDER[:8]

        # prelude: window sums first (pooled -> w9 -> conv is the critical
        # path; image-max/theta only gates the post-drain center-tap add)
        for k in range(3):
            emit_band_windows(0, k, engine="vector")
        emit_band_windows(2, 0, engine="scalar")
        emit_band_windows(2, 1, engine="scalar")
        emit_band_windows(2, 2, engine="vector")
        emit_avg_fin(0)
        emit_avg_fin(2)
        emit_xB_dma(0)
        emit_xB_dma(1)
        emit_xB_dma(2)
        emit_warmup(30, padA[0][:, 65:67, 0:64])
        emit_warmup(30, padA[2][:, 65:67, 0:64])
        emit_sample(0, "pool")
        emit_fold_pool(0)
        emit_w9_mm(0)
        e0 = emit_w9_exp(0)
        emit_w9_fin(0, e0)
        emit_diag(0, NC8, "vector")
        for k in range(3):
            emit_band_max(0, k)
            emit_band_max(2, k)
        emit_mx_fin(0)
        emit_mx_fin(2)
        emit_warmup(24, padA[1][:, 10:12, 0:64])
        emit_warmup(16, padA[1][:, 40:42, 0:64], drain=True)
        emit_sample(0, "theta")
        emit_fold_theta(0)
        emit_theta_mm(0)
        emit_theta_fin(0)
        emit_w4p(0)
        emit_diag(0, [4], "vector")

        conv_taps(0, 0, TAP_ORDER)
        conv_taps(0, 1, TAP_ORDER)

        # g1 stats (data lands mid-conv-g0); sums on ScalarE (slack there)
        for k in range(3):
            emit_band_windows(1, k, engine="scalar")
            emit_band_max(1, k)
        emit_mx_fin(1)
        emit_avg_fin(1)

        conv_drain(0, 0)
        conv_taps(0, 2, TAP_ORDER)
        emit_sample(1, "pool")
        emit_fold_pool(1)
        emit_sample(1, "theta")
        emit_fold_theta(1)
        emit_theta_mm(1)
        emit_theta_fin(1)
        emit_w9_mm(1)
        e1 = emit_w9_exp(1)
        emit_w9_fin(1, e1)
        emit_w4p(1)
        emit_diag(1, TAP_ORDER, "scalar")
        conv_drain(0, 1)
        conv_taps(0, 3, TAP_ORDER)
        emit_theta_fin(2)
        emit_w9_mm(2)
        e2 = emit_w9_exp(2)
        emit_w9_fin(2, e2)
        emit_w4p(2)
        emit_diag(2, TAP_ORDER, "scalar")
        conv_drain(0, 2)
        conv_taps(1, 0, TAP_ORDER)
        conv_drain(0, 3)
        emit_conv_dve(0, 0, 13)
        conv_taps(1, 1, TAP_ORDER)
        conv_drain(1, 0)
        emit_conv_dve(0, 13, 26)
        conv_taps(1, 2, TAP_ORDER)
        conv_drain(1, 1)
        emit_conv_dve(0, 26, 31)
        emit_conv_dve(1, 0, 13)
        conv_taps(1, 3, TAP_ORDER)
        conv_drain(1, 2)
        emit_conv_dve(1, 13, 26)
        conv_taps(2, 0, TAP_ORDER)
        conv_drain(1, 3)
        emit_conv_dve(1, 26, 31)
        conv_taps(2, 1, TAP_ORDER)
        conv_drain(2, 0)
        emit_conv_dve(2, 0, 13)
        conv_taps(2, 2, TAP_ORDER)
        conv_drain(2, 1)
        emit_conv_dve(2, 13, 26)
        conv_taps(2, 3, TAP_ORDER)
        conv_drain(2, 2)
        conv_drain(2, 3)

    nc.compile()
    return nc


def _host_prep(inputs):
    x = np.ascontiguousarray(inputs["x"], dtype=np.float32)
    cam_w1 = np.asarray(inputs["cam_w1"], dtype=np.float32)
    cam_w2 = np.asarray(inputs["cam_w2"], dtype=np.float32)
    proj_w1 = np.asarray(inputs["proj_w1"], dtype=np.float32)
    bn_gamma = np.asarray(inputs["bn_gamma"], dtype=np.float32)
    bn_beta = np.asarray(inputs["bn_beta"], dtype=np.float32)
    proj_w2 = np.asarray(inputs["proj_w2"], dtype=np.float32)
    adk = np.asarray(inputs["adk_weight"], dtype=np.float32)

    xb16 = x.astype(BF16)
    xpA = np.zeros((B, C, HP, WP), dtype=BF16)
    xpA[:, :, 1:97, 2:98] = xb16
    # padB: x payload at col 3, rows = padded rows 50..97 (x rows 49..95)
    xpB = np.zeros((B, C, XB_NR, WP), dtype=BF16)
    xpB[:, :, 0:47, 3:99] = xb16[:, :, 49:96, :]

    in_maps = []
    w1t = cam_w1.T.astype(np.float32)
    p1t = (proj_w1.T / 1024.0).astype(np.float32)
    cmap = np.concatenate([np.arange(128), np.arange(128),
                           np.arange(128, 192), np.arange(128, 192)])
    consts = {
        "eye": np.eye(128, dtype=BF16),
        "w1avg_a": np.ascontiguousarray(w1t[0:128] / (H * W)),
        "w1avg_b": np.ascontiguousarray(np.concatenate([w1t[128:192] / (H * W)] * 2, axis=0)),
        "w1mx_a": np.ascontiguousarray(w1t[0:128]),
        "w1mx_b": np.ascontiguousarray(np.concatenate([w1t[128:192]] * 2, axis=0)),
        "w2t": np.ascontiguousarray(cam_w2.T.astype(np.float32)),
        "p1a": np.ascontiguousarray(p1t[0:128]),
        "p1b": np.ascontiguousarray(np.concatenate([p1t[128:192]] * 2, axis=0)),
        "bn_scale": np.ascontiguousarray((bn_gamma / np.sqrt(1.0 + BN_EPS)).reshape(R, 1)),
        "bn_beta": np.ascontiguousarray(bn_beta.reshape(R, 1)),
        "w2s": np.ascontiguousarray(proj_w2.T.astype(np.float32)),
        "adkT": np.ascontiguousarray(
            adk.transpose(1, 0, 2, 3).reshape(C, G * 9)[cmap].astype(np.float32)
        ),
    }
    for k in range(N_CORES):
        b0, b1 = 2 * k, 2 * k + 1
        shardA = np.ascontiguousarray(np.concatenate(
            [xpA[b0, 0:128], xpA[b1, 0:128], xpA[b0, 128:192], xpA[b1, 128:192]],
            axis=0))
        shardB = np.ascontiguousarray(np.concatenate(
            [xpB[b0, 0:128], xpB[b1, 0:128], xpB[b0, 128:192], xpB[b1, 128:192]],
            axis=0))
        m = {"xA": shardA, "xB": shardB}
        m.update(consts)
        in_maps.append(m)
    return in_maps


def kernel(**inputs) -> np.ndarray:
    global _COMPILED
    from concourse.bass_utils import run_bass_kernel_spmd

    in_maps = _host_prep(inputs)

    if _COMPILED is None:
        _COMPILED = _build()
    nc = _COMPILED

    res = run_bass_kernel_spmd(nc, in_maps, core_ids=list(range(N_CORES)))
    outs = [r["out"] for r in res.results]

    y = np.empty((B, C, H, W), np.float32)
    for k in range(N_CORES):
        o = np.asarray(outs[k]).reshape(384, H, W).astype(np.float32)
        b0, b1 = 2 * k, 2 * k + 1
        y[b0, 0:128] = o[0:128]
        y[b1, 0:128] = o[128:256]
        y[b0, 128:192] = o[256:320]
        y[b1, 128:192] = o[320:384]
    return y


if __name__ == "__main__":
    import reference

    inputs = {k: np.asarray(v) for k, v in reference.setup_inputs().items()}
    y = kernel(**inputs)
    print("kernel output:", y.shape, y.dtype)



# revision 18
# speedup vs baseline: 1.3637x; 1.0439x over previous
"""Trainium2 Bass kernel for nn_ADSCDConv (dense_cnn), 8-core data parallel.

Per core (2 samples = 384 (b,c) channel-images of 96x96), groups of 128
partitions: g0=(b0,c0:128), g1=(b1,c0:128), g2=(b0,c128:192)||(b1,c128:192).

v2 schedule (vs v1):
  - tap-outer conv matmuls: LDWEIGHTS amortized over a multi-bank PSUM
    window; PE runs at the 202ns/FD480 streaming roofline.
  - center tap (the only theta-dependent one) is emitted LAST per window
    and deferred for the first two windows of g0, so the conv starts on
    the pooled-only dependency chain while the image-max/theta chain
    finishes.
  - windows/bandmax stats are banded (32 rows) and pipelined with the
    input DMA; x lands padded to width 100 with the payload at col 2 so
    window sums hit the DVE 4x mode.
  - DVE conv tail uses fused scalar_tensor_tensor (mul+add in one op);
    a second x copy shifted by one column (padB) keeps all taps 4B
    aligned for the 2x bf16 mode.
  - PSUM: two conv window pools (4 banks + 3 banks) alternate A,B,A,...
    globally across groups; 1 stats bank.
"""

from contextlib import ExitStack

import numpy as np
import ml_dtypes

BF16 = ml_dtypes.bfloat16

B, C, H, W = 16, 192, 96, 96
G = 4
R = C // 4  # 48
BN_EPS = 1e-5
N_CORES = 8
HP, WP = H + 2, 100  # padded rows 98, padded cols 100 (x payload at col 2)
XB_R0 = 50           # padB covers padded rows 50..97
XB_NR = 48

# conv windows: banks per window, alternating pool A(4)/B(3) globally
# (sequence across groups must alternate 4,3,4,3,... for PSUM pool reuse)
WIN_SEQ = {0: [4, 3, 4, 3], 1: [4, 3, 4, 2], 2: [4, 3, 4, 3]}
ROWS_PE = {g: 5 * sum(WIN_SEQ[g]) for g in range(3)}  # 70, 65, 70

# tap order: center (tap 4, the only theta-dependent tap) last
TAP_ORDER = [0, 1, 2, 3, 5, 6, 7, 8, 4]

_COMPILED = None


def _build():
    import concourse.tile as tile
    from concourse import bacc, mybir

    f32 = mybir.dt.float32
    bf16 = mybir.dt.bfloat16
    ALU = mybir.AluOpType
    ACTF = mybir.ActivationFunctionType

    nc = bacc.Bacc("TRN2", target_bir_lowering=False, debug=False, num_devices=N_CORES)

    # ---- DRAM tensors ----
    xA_d = nc.dram_tensor("xA", [384, HP, WP], bf16, kind="ExternalInput").ap()
    xB_d = nc.dram_tensor("xB", [384, XB_NR, WP], bf16, kind="ExternalInput").ap()
    out_d = nc.dram_tensor("out", [384, H, W], bf16, kind="ExternalOutput").ap()
    warm_d = nc.dram_tensor("warm", [128, 1], bf16, kind="ExternalOutput").ap()
    eye_d = nc.dram_tensor("eye", [128, 128], bf16, kind="ExternalInput").ap()
    w1avg_a_d = nc.dram_tensor("w1avg_a", [128, R], f32, kind="ExternalInput").ap()
    w1avg_b_d = nc.dram_tensor("w1avg_b", [128, R], f32, kind="ExternalInput").ap()
    w1mx_a_d = nc.dram_tensor("w1mx_a", [128, R], f32, kind="ExternalInput").ap()
    w1mx_b_d = nc.dram_tensor("w1mx_b", [128, R], f32, kind="ExternalInput").ap()
    w2t_d = nc.dram_tensor("w2t", [R, C], f32, kind="ExternalInput").ap()
    p1a_d = nc.dram_tensor("p1a", [128, R], f32, kind="ExternalInput").ap()
    p1b_d = nc.dram_tensor("p1b", [128, R], f32, kind="ExternalInput").ap()
    bns_d = nc.dram_tensor("bn_scale", [R, 1], f32, kind="ExternalInput").ap()
    bnb_d = nc.dram_tensor("bn_beta", [R, 1], f32, kind="ExternalInput").ap()
    w2s_d = nc.dram_tensor("w2s", [R, G * C], f32, kind="ExternalInput").ap()
    adkT_d = nc.dram_tensor("adkT", [384, 36], f32, kind="ExternalInput").ap()

    with tile.TileContext(nc) as tc, ExitStack() as ctx:
        def sb(name, shape, dt):
            return nc.alloc_sbuf_tensor(name, shape, dt).ap()

        padA = [sb(f"padA{g}", [128, HP, WP], bf16) for g in range(3)]
        padB = [sb(f"padB{g}", [128, XB_NR, WP], bf16) for g in range(3)]
        padAf = [p.rearrange("p a b -> p (a b)") for p in padA]
        padBf = [p.rearrange("p a b -> p (a b)") for p in padB]
        tailb = [sb(f"tail{g}", [128, 31, W], bf16) for g in range(3)]
        diag = [sb(f"diag{g}", [128, 9, 128], bf16) for g in range(3)]
        pooled = [sb(f"pooled{g}", [128, 9], f32) for g in range(3)]
        avgs = [sb(f"avgs{g}", [128, 1], f32) for g in range(3)]
        mx = [sb(f"mx{g}", [128, 1], f32) for g in range(3)]
        th = [sb(f"theta{g}", [128, 1], f32) for g in range(3)]
        w9 = [sb(f"w9_{g}", [128, 9], f32) for g in range(3)]
        w4p = [sb(f"w4p{g}", [128, 1], f32) for g in range(3)]
        wsum9 = [sb(f"wsum9_{g}", [128, 1], f32) for g in range(3)]
        adkT = [sb(f"adkT{g}_sb", [128, 36], f32) for g in range(3)]

        eye = sb("eye_sb", [128, 128], bf16)
        w1avg_a = sb("w1avg_a_sb", [128, R], f32)
        w1avg_b = sb("w1avg_b_sb", [128, R], f32)
        w1mx_a = sb("w1mx_a_sb", [128, R], f32)
        w1mx_b = sb("w1mx_b_sb", [128, R], f32)
        w2t = sb("w2t_sb", [R, C], f32)
        p1a = sb("p1a_sb", [128, R], f32)
        p1b = sb("p1b_sb", [128, R], f32)
        bns = sb("bns_sb", [R, 1], f32)
        bnb = sb("bnb_sb", [R, 1], f32)
        w2s = sb("w2s_sb", [R, G * C], f32)

        h_adk = [sb(f"h_adk{b}", [R, 9], f32) for b in range(2)]
        hsum = [sb(f"hsum{b}", [R, 1], f32) for b in range(2)]

        scr = ctx.enter_context(tc.tile_pool(name="scr", bufs=4))
        treep = ctx.enter_context(tc.tile_pool(name="treep", bufs=2))
        term_pool = ctx.enter_context(tc.tile_pool(name="terms", bufs=3))
        osb_pool = ctx.enter_context(tc.tile_pool(name="osbp", bufs=8))
        ct_pool = ctx.enter_context(tc.tile_pool(name="ctp", bufs=3))
        psA = ctx.enter_context(tc.tile_pool(name="psA", bufs=1, space="PSUM"))
        psB = ctx.enter_context(tc.tile_pool(name="psB", bufs=1, space="PSUM"))
        # stats bank: single-shot matmul groups only may share a bank
        stpa = nc.alloc_psum_tensor("statps", [128, 512], f32).ap()

        # ---------------- DMA emission ----------------
        row_chunks = [(0, 33), (33, 65), (65, HP)]

        def emit_xA_dma(g):
            for (r0, r1) in row_chunks:
                nc.sync.dma_start(
                    out=padA[g][:, r0:r1, :],
                    in_=xA_d[g * 128:(g + 1) * 128, r0:r1, :],
                )

        def emit_xB_dma(g):
            # WAW gate: tiny DVE write into padB dependent on g1's last xA
            # chunk keeps the xB transfer out of the critical input window
            nc.vector.tensor_copy(padB[g][:, 0:1, 0:2], padA[1][:, 97:98, 0:2])
            nc.scalar.dma_start(
                out=padB[g][:, :, :],
                in_=xB_d[g * 128:(g + 1) * 128, :, :],
            )

        emit_xA_dma(0)
        wloads = [
            (eye, eye_d), (w1avg_a, w1avg_a_d), (w1avg_b, w1avg_b_d),
            (w1mx_a, w1mx_a_d), (w1mx_b, w1mx_b_d), (w2t, w2t_d),
            (p1a, p1a_d), (p1b, p1b_d), (bns, bns_d), (bnb, bnb_d),
            (w2s, w2s_d),
            (adkT[0], adkT_d[0:128, :]), (adkT[1], adkT_d[128:256, :]),
            (adkT[2], adkT_d[256:384, :]),
        ]
        for (dst, src) in wloads:
            nc.gpsimd.dma_start(out=dst, in_=src)
        emit_xA_dma(2)
        emit_xA_dma(1)

        # ---------------- stats ----------------
        def emit_band_windows(g, k, engine="vector"):
            # 3 col-window sums of the 32-row band k -> pooled[g][:, 3k+j]
            if engine == "scalar":
                for j in range(3):
                    win = padA[g][:, 1 + 32 * k:33 + 32 * k, 2 + 32 * j:34 + 32 * j]
                    acc = pooled[g][:, 3 * k + j:3 * k + j + 1]
                    s = treep.tile([128, 32, 32], bf16, tag="wscr", name=f"w{g}_{k}_{j}")
                    nc.scalar.activation(out=s[:, :, :], in_=win,
                                         func=ACTF.Copy, accum_out=acc)
                return
            # DVE: bf16 TT add-tree 32->16->8->4 rows (2x mode), then 3
            # cache-reduce window sums over the 4 leaf rows
            p = padA[g]
            r0 = 1 + 32 * k
            t16 = treep.tile([128, 16, WP], bf16, tag="tr16", name=f"s16_{g}_{k}")
            nc.vector.tensor_add(t16[:, :, :], p[:, r0:r0 + 16, :], p[:, r0 + 16:r0 + 32, :])
            t8 = treep.tile([128, 8, WP], bf16, tag="tr8", name=f"s8_{g}_{k}")
            nc.vector.tensor_add(t8[:, :, :], t16[:, 0:8, :], t16[:, 8:16, :])
            t4 = treep.tile([128, 4, WP], bf16, tag="tr4", name=f"s4_{g}_{k}")
            nc.vector.tensor_add(t4[:, :, :], t8[:, 0:4, :], t8[:, 4:8, :])
            for j in range(3):
                acc = pooled[g][:, 3 * k + j:3 * k + j + 1]
                s = treep.tile([128, 4, 32], bf16, tag="wscr4", name=f"w{g}_{k}_{j}")
                nc.vector.tensor_scalar(s[:, :, :],
                                        t4[:, :, 2 + 32 * j:34 + 32 * j],
                                        1.0, None,
                                        op0=ALU.mult, op1=ALU.add, accum_out=acc)

        lvmax = [sb(f"lvmax{g}", [128, 12, WP], bf16) for g in range(3)]

        def emit_band_max(g, k):
            # bf16 TT max-tree 32->16->8->4 rows into lvmax[g][:, 4k:4k+4]
            p = padA[g]
            r0 = 1 + 32 * k
            t16 = treep.tile([128, 16, WP], bf16, tag="tr16", name=f"m16_{g}_{k}")
            nc.vector.tensor_tensor(out=t16[:, :, :], in0=p[:, r0:r0 + 16, :],
                                    in1=p[:, r0 + 16:r0 + 32, :], op=ALU.max)
            t8 = treep.tile([128, 8, WP], bf16, tag="tr8", name=f"m8_{g}_{k}")
            nc.vector.tensor_tensor(out=t8[:, :, :], in0=t16[:, 0:8, :],
                                    in1=t16[:, 8:16, :], op=ALU.max)
            nc.vector.tensor_tensor(out=lvmax[g][:, 4 * k:4 * k + 4, :],
                                    in0=t8[:, 0:4, :], in1=t8[:, 4:8, :], op=ALU.max)

        def emit_mx_fin(g):
            t6 = treep.tile([128, 6, WP], bf16, tag="tr6", name=f"mf6_{g}")
            nc.vector.tensor_tensor(out=t6[:, :, :], in0=lvmax[g][:, 0:6, :],
                                    in1=lvmax[g][:, 6:12, :], op=ALU.max)
            t3 = treep.tile([128, 3, WP], bf16, tag="tr3", name=f"mf3_{g}")
            nc.vector.tensor_tensor(out=t3[:, :, :], in0=t6[:, 0:3, :],
                                    in1=t6[:, 3:6, :], op=ALU.max)
            nc.vector.tensor_reduce(out=mx[g][:, :], in_=t3[:, :, :],
                                    axis=mybir.AxisListType.XY, op=ALU.max)

        def emit_avg_fin(g):
            asc = scr.tile([128, 9], bf16, tag="ascr", name=f"avg{g}")
            nc.scalar.activation(out=asc[:, :], in_=pooled[g][:, :],
                                 func=ACTF.Copy, accum_out=avgs[g][:, :])

        # ---------------- per-sample algebra ----------------
        def emit_sample(b, part):
            if b == 0:
                chunks = [
                    (w1avg_a[:, :], w1mx_a[:, :], p1a[:, :], (0, 0, 128)),
                    (w1avg_b[0:64, :], w1mx_b[0:64, :], p1b[0:64, :], (2, 0, 64)),
                ]
            else:
                chunks = [
                    (w1avg_a[:, :], w1mx_a[:, :], p1a[:, :], (1, 0, 128)),
                    (w1avg_b[64:128, :], w1mx_b[64:128, :], p1b[64:128, :], (2, 64, 128)),
                ]
            base = 8 + b * 22
            for i, (wa, wm, wp, (sg, q0, q1)) in enumerate(chunks):
                o = base + 11 * i
                if part == "pool":
                    nc.tensor.matmul(stpa[0:R, o + 2:o + 11], lhsT=wp, rhs=pooled[sg][q0:q1, :], start=True, stop=True)
                else:
                    nc.tensor.matmul(stpa[0:R, o:o + 1], lhsT=wa, rhs=avgs[sg][q0:q1, :], start=True, stop=True)
                    nc.tensor.matmul(stpa[0:R, o + 1:o + 2], lhsT=wm, rhs=mx[sg][q0:q1, :], start=True, stop=True)

        def emit_fold_pool(b):
            base = 8 + b * 22
            hc = scr.tile([R, 9], f32, tag="scr48", name=f"hc{b}")
            nc.vector.tensor_copy(hc[:, :], stpa[0:R, base + 2:base + 11])
            hs = scr.tile([R, 9], f32, tag="scr48", name=f"hs{b}")
            nc.vector.tensor_add(hs[:, :], hc[:, :],
                                 stpa[0:R, base + 13:base + 22])
            t1 = scr.tile([R, 9], f32, tag="scr48", name=f"bn{b}")
            nc.vector.tensor_scalar(t1[:, :], hs[:, :], bns[:, :], bnb[:, :],
                                    op0=ALU.mult, op1=ALU.add)
            nc.vector.tensor_scalar_max(h_adk[b][:, :], t1[:, :], 0.0)

        def emit_fold_theta(b):
            base = 8 + b * 22
            hg = scr.tile([R, 2], f32, tag="scr2", name=f"hg{b}")
            nc.vector.tensor_copy(hg[:, :], stpa[0:R, base:base + 2])
            hs = scr.tile([R, 2], f32, tag="scr2", name=f"ht{b}")
            nc.vector.tensor_add(hs[:, :], hg[:, :],
                                 stpa[0:R, base + 11:base + 13])
            ha = scr.tile([R, 1], f32, tag="scr1", name=f"ha{b}")
            hm = scr.tile([R, 1], f32, tag="scr1", name=f"hm{b}")
            nc.vector.tensor_scalar_max(ha[:, :], hs[:, 0:1], 0.0)
            nc.vector.tensor_scalar_max(hm[:, :], hs[:, 1:2], 0.0)
            nc.vector.tensor_add(hsum[b][:, :], ha[:, :], hm[:, :])

        # ---------------- theta ----------------
        ps_t = [stpa[:, i:i + 1] for i in range(3)]

        def emit_theta_mm(b):
            nc.tensor.matmul(ps_t[b], lhsT=w2t[:, 0:128], rhs=hsum[b][:, :], start=True, stop=True)
            q0, q1 = (0, 64) if b == 0 else (64, 128)
            nc.tensor.matmul(ps_t[2][q0:q1], lhsT=w2t[:, 128:192], rhs=hsum[b][:, :], start=True, stop=True)

        def emit_theta_fin(g):
            et = scr.tile([128, 1], f32, tag="scr1", name=f"et{g}")
            nc.scalar.activation(out=et[:, :], in_=ps_t[g], func=ACTF.Exp, scale=-1.0)
            d = scr.tile([128, 1], f32, tag="scr1", name=f"etd{g}")
            nc.vector.tensor_scalar_add(d[:, :], et[:, :], 1.0)
            nc.vector.reciprocal(th[g][:, :], d[:, :])

        # ---------------- dynamic kernels w9 ----------------
        def emit_w9_mm(g):
            ps_s = stpa[:, 64 + g * 36:64 + (g + 1) * 36]
            for gg in range(G):
                sl = slice(gg * 9, gg * 9 + 9)
                if g < 2:
                    nc.tensor.matmul(ps_s[:, sl], lhsT=w2s[:, gg * 192:gg * 192 + 128],
                                     rhs=h_adk[g][:, :], start=True, stop=True)
                else:
                    nc.tensor.matmul(ps_s[0:64, sl], lhsT=w2s[:, gg * 192 + 128:gg * 192 + 192],
                                     rhs=h_adk[0][:, :], start=True, stop=True)
                    nc.tensor.matmul(ps_s[64:128, sl], lhsT=w2s[:, gg * 192 + 128:gg * 192 + 192],
                                     rhs=h_adk[1][:, :], start=True, stop=True)

        def emit_w9_exp(g):
            ps_s = stpa[:, 64 + g * 36:64 + (g + 1) * 36]
            e = scr.tile([128, 36], f32, tag="scr36", name=f"e{g}")
            nc.scalar.activation(out=e[:, :], in_=ps_s, func=ACTF.Exp)
            return e

        def emit_w9_fin(g, e):
            d1 = scr.tile([128, 9], f32, tag="scr9", name=f"d1_{g}")
            d2 = scr.tile([128, 9], f32, tag="scr9", name=f"d2_{g}")
            nc.vector.tensor_add(d1[:, :], e[:, 0:9], e[:, 9:18])
            nc.vector.tensor_add(d2[:, :], e[:, 18:27], e[:, 27:36])
            nc.vector.tensor_add(d1[:, :], d1[:, :], d2[:, :])
            rec = scr.tile([128, 9], f32, tag="scr9", name=f"rec{g}")
            nc.vector.reciprocal(rec[:, :], d1[:, :])
            a = adkT[g]
            m1 = scr.tile([128, 9], f32, tag="scr9", name=f"m1_{g}")
            m2 = scr.tile([128, 9], f32, tag="scr9", name=f"m2_{g}")
            nc.vector.tensor_mul(m1[:, :], e[:, 0:9], a[:, 0:9])
            nc.vector.tensor_mul(m2[:, :], e[:, 9:18], a[:, 9:18])
            nc.vector.tensor_add(m1[:, :], m1[:, :], m2[:, :])
            nc.vector.tensor_mul(m2[:, :], e[:, 18:27], a[:, 18:27])
            nc.vector.tensor_add(m1[:, :], m1[:, :], m2[:, :])
            nc.vector.tensor_mul(m2[:, :], e[:, 27:36], a[:, 27:36])
            nc.vector.tensor_add(m1[:, :], m1[:, :], m2[:, :])
            nc.vector.tensor_mul(w9[g][:, :], m1[:, :], rec[:, :])
            nc.vector.tensor_reduce(out=wsum9[g][:, :], in_=w9[g][:, :],
                                    axis=mybir.AxisListType.X, op=ALU.add)

        def emit_w4p(g):
            t1 = scr.tile([128, 1], f32, tag="scr1", name=f"t1_{g}")
            nc.vector.tensor_mul(t1[:, :], w9[g][:, 4:5], th[g][:, :])
            nc.vector.tensor_add(t1[:, :], t1[:, :], w9[g][:, 4:5])
            nc.vector.tensor_sub(w4p[g][:, :], t1[:, :], wsum9[g][:, :])

        def emit_diag(g, taps, engine):
            for tap in taps:
                scal = w4p[g][:, 0:1] if tap == 4 else w9[g][:, tap:tap + 1]
                if engine == "vector":
                    nc.vector.tensor_scalar_mul(diag[g][:, tap, :], eye[:, :], scal)
                else:
                    nc.scalar.activation(out=diag[g][:, tap, :], in_=eye[:, :],
                                         func=ACTF.Copy, scale=scal)

        # ---------------- conv on PE ----------------
        # window w of group g covers rows win_r0 .. win_r0+5*banks
        win_r0 = {}
        for g in range(3):
            r = 0
            for w, nb in enumerate(WIN_SEQ[g]):
                win_r0[(g, w)] = r
                r += 5 * nb

        pools = {4: psA, 3: psB}
        win_tile = {}
        out_rr = [0]

        def conv_taps(g, w, taps):
            nb = WIN_SEQ[g][w]
            r0 = win_r0[(g, w)]
            key = (g, w)
            if key not in win_tile:
                pnb = 4 if w % 2 == 0 else 3
                win_tile[key] = pools[pnb].tile(
                    [128, pnb, 512], f32, tag=f"w{pnb}", name=f"ps{g}_{w}")
            ps = win_tile[key]
            for tap in taps:
                dy, dx = divmod(tap, 3)
                for b in range(nb):
                    y0 = r0 + 5 * b + dy
                    nc.tensor.matmul(
                        ps[:, b, 0:480],
                        lhsT=diag[g][:, tap, :],
                        rhs=padA[g][:, y0:y0 + 5, dx + 1:dx + 97],
                        start=(tap == taps[0]), stop=(tap == taps[-1]),
                    )

        def conv_drain(g, w):
            nb = WIN_SEQ[g][w]
            r0 = win_r0[(g, w)]
            ps = win_tile.pop((g, w))
            nr = 5 * nb
            ot = osb_pool.tile([128, 20, W], bf16, tag="ow", name=f"ow{g}_{w}")
            nc.scalar.activation(
                out=ot[:, 0:nr, :],
                in_=ps[:, 0:nb, 0:480], func=ACTF.Copy)
            eng = (nc.sync, nc.gpsimd)[out_rr[0] % 2]
            out_rr[0] += 1
            eng.dma_start(
                out=out_d[g * 128:(g + 1) * 128, r0:r0 + nr, :],
                in_=ot[:, 0:nr, :])

        # ---------------- conv tail on DVE ----------------
        def emit_conv_dve(g, lo, hi):
            # DVE tail rows: flat contiguous strips (full padded rows, junk
            # at pad columns) so the muls hit the 4x DVE mode; only the
            # final add back into tailb is strided.
            y0 = ROWS_PE[g] + lo
            n = hi - lo
            L = (n - 1) * 100 + 96
            acc = None
            for i, tap in enumerate(TAP_ORDER):
                dy, dx = divmod(tap, 3)
                scal = w4p[g][:, 0:1] if tap == 4 else w9[g][:, tap:tap + 1]
                if dx == 1:
                    o0 = (y0 + dy) * 100 + 2
                    strip = padAf[g][:, o0:o0 + L]
                else:
                    o0 = (y0 + dy - XB_R0) * 100 + (2 if dx == 0 else 4)
                    strip = padBf[g][:, o0:o0 + L]
                t = term_pool.tile([128, 14, 100], bf16, tag="term",
                                   name=f"t{g}_{lo}_{i}")
                tf = t.rearrange("p a b -> p (a b)")
                nc.vector.tensor_scalar_mul(tf[:, 0:L], strip, scal)
                if i == 0:
                    acc = t
                elif i < 8:
                    nxt = term_pool.tile([128, 14, 100], bf16, tag="term",
                                         name=f"a{g}_{lo}_{i}")
                    nxf = nxt.rearrange("p a b -> p (a b)")
                    nc.vector.tensor_add(nxf[:, 0:L],
                                         acc.rearrange("p a b -> p (a b)")[:, 0:L],
                                         tf[:, 0:L])
                    acc = nxt
                else:
                    nc.vector.tensor_add(tailb[g][:, lo:hi, :],
                                         acc[:, 0:n, 0:96], t[:, 0:n, 0:96])
            eng = (nc.sync, nc.gpsimd)[out_rr[0] % 2]
            out_rr[0] += 1
            eng.dma_start(out=out_d[g * 128:(g + 1) * 128, y0:y0 + n, :],
                          in_=tailb[g][:, lo:hi, :])

        # ---------------- PE warm-up ----------------
        # junk matmuls gated on successive DMA chunks so the PE stays busy
        # (HAM warm) across the whole stats prelude without running eagerly
        def emit_warmup(k, gate, drain=False):
            for j in range(k):
                nc.tensor.matmul(stpa[:, 384:512], lhsT=eye[:, :],
                                 rhs=gate, start=True, stop=True)
            if drain:
                wsc = scr.tile([128, 1], bf16, tag="wscr1", name="wscr")
                nc.scalar.activation(out=wsc[:, :], in_=stpa[:, 384:385], func=ACTF.Copy)
                nc.sync.dma_start(out=warm_d, in_=wsc[:, :])

        # ---------------- emission order ----------------
        NC8 = TAP_ORDER[:8]

        # prelude: window sums first (pooled -> w9 -> conv is the critical
        # path; image-max/theta only gates the post-drain center-tap add)
        for k in range(3):
            emit_band_windows(0, k, engine="vector")
        emit_band_windows(2, 0, engine="scalar")
        emit_band_windows(2, 1, engine="scalar")
        emit_band_windows(2, 2, engine="vector")
        emit_avg_fin(0)
        emit_avg_fin(2)
        emit_xB_dma(0)
        emit_xB_dma(1)
        emit_xB_dma(2)
        emit_warmup(30, padA[0][:, 65:67, 0:64])
        emit_warmup(30, padA[2][:, 65:67, 0:64])
        emit_sample(0, "pool")
        emit_fold_pool(0)
        emit_w9_mm(0)
        e0 = emit_w9_exp(0)
        emit_w9_fin(0, e0)
        emit_diag(0, NC8, "vector")
        for k in range(3):
            emit_band_max(0, k)
            emit_band_max(2, k)
        emit_mx_fin(0)
        emit_mx_fin(2)
        emit_warmup(24, padA[1][:, 10:12, 0:64])
        emit_warmup(16, padA[1][:, 40:42, 0:64], drain=True)
        emit_sample(0, "theta")
        emit_fold_theta(0)
        emit_theta_mm(0)
        emit_theta_fin(0)
        emit_w4p(0)
        emit_diag(0, [4], "vector")

        conv_taps(0, 0, TAP_ORDER)
        conv_taps(0, 1, TAP_ORDER)

        # g1 stats (data lands mid-conv-g0); sums on ScalarE (slack there)
        for k in range(3):
            emit_band_windows(1, k, engine="scalar")
            emit_band_max(1, k)
        emit_mx_fin(1)
        emit_avg_fin(1)

        conv_drain(0, 0)
        conv_taps(0, 2, TAP_ORDER)
        emit_sample(1, "pool")
        emit_fold_pool(1)
        emit_sample(1, "theta")
        emit_fold_theta(1)
        emit_theta_mm(1)
        emit_theta_fin(1)
        emit_w9_mm(1)
        e1 = emit_w9_exp(1)
        emit_w9_fin(1, e1)
        emit_w4p(1)
        emit_diag(1, TAP_ORDER, "scalar")
        conv_drain(0, 1)
        conv_taps(0, 3, TAP_ORDER)
        emit_theta_fin(2)
        emit_w9_mm(2)
        e2 = emit_w9_exp(2)
        emit_w9_fin(2, e2)
        emit_w4p(2)
        emit_diag(2, TAP_ORDER, "scalar")
        conv_drain(0, 2)
        conv_taps(1, 0, TAP_ORDER)
        conv_drain(0, 3)
        emit_conv_dve(0, 0, 13)
        conv_taps(1, 1, TAP_ORDER)
        conv_drain(1, 0)
        emit_conv_dve(0, 13, 26)
        conv_taps(1, 2, TAP_ORDER)
        conv_drain(1, 1)
        emit_conv_dve(1, 0, 13)
        conv_taps(1, 3, TAP_ORDER)
        conv_drain(1, 2)
        emit_conv_dve(1, 13, 26)
        conv_taps(2, 0, TAP_ORDER)
        conv_drain(1, 3)
        emit_conv_dve(1, 26, 31)
        conv_taps(2, 1, TAP_ORDER)
        conv_drain(2, 0)
        emit_conv_dve(2, 0, 13)
        conv_taps(2, 2, TAP_ORDER)
        conv_drain(2, 1)
        emit_conv_dve(2, 13, 26)
        conv_taps(2, 3, TAP_ORDER)
        conv_drain(2, 2)
        conv_drain(2, 3)

    nc.compile()
    return nc


def _host_prep(inputs):
    x = np.ascontiguousarray(inputs["x"], dtype=np.float32)
    cam_w1 = np.asarray(inputs["cam_w1"], dtype=np.float32)
    cam_w2 = np.asarray(inputs["cam_w2"], dtype=np.float32)
    proj_w1 = np.asarray(inputs["proj_w1"], dtype=np.float32)
    bn_gamma = np.asarray(inputs["bn_gamma"], dtype=np.float32)
    bn_beta = np.asarray(inputs["bn_beta"], dtype=np.float32)
    proj_w2 = np.asarray(inputs["proj_w2"], dtype=np.float32)
    adk = np.asarray(inputs["adk_weight"], dtype=np.float32)

    xb16 = x.astype(BF16)
    xpA = np.zeros((B, C, HP, WP), dtype=BF16)
    xpA[:, :, 1:97, 2:98] = xb16
    # padB: x payload at col 3, rows = padded rows 50..97 (x rows 49..95)
    xpB = np.zeros((B, C, XB_NR, WP), dtype=BF16)
    xpB[:, :, 0:47, 3:99] = xb16[:, :, 49:96, :]

    in_maps = []
    w1t = cam_w1.T.astype(np.float32)
    p1t = (proj_w1.T / 1024.0).astype(np.float32)
    cmap = np.concatenate([np.arange(128), np.arange(128),
                           np.arange(128, 192), np.arange(128, 192)])
    consts = {
        "eye": np.eye(128, dtype=BF16),
        "w1avg_a": np.ascontiguousarray(w1t[0:128] / (H * W)),
        "w1avg_b": np.ascontiguousarray(np.concatenate([w1t[128:192] / (H * W)] * 2, axis=0)),
        "w1mx_a": np.ascontiguousarray(w1t[0:128]),
        "w1mx_b": np.ascontiguousarray(np.concatenate([w1t[128:192]] * 2, axis=0)),
        "w2t": np.ascontiguousarray(cam_w2.T.astype(np.float32)),
        "p1a": np.ascontiguousarray(p1t[0:128]),
        "p1b": np.ascontiguousarray(np.concatenate([p1t[128:192]] * 2, axis=0)),
        "bn_scale": np.ascontiguousarray((bn_gamma / np.sqrt(1.0 + BN_EPS)).reshape(R, 1)),
        "bn_beta": np.ascontiguousarray(bn_beta.reshape(R, 1)),
        "w2s": np.ascontiguousarray(proj_w2.T.astype(np.float32)),
        "adkT": np.ascontiguousarray(
            adk.transpose(1, 0, 2, 3).reshape(C, G * 9)[cmap].astype(np.float32)
        ),
    }
    for k in range(N_CORES):
        b0, b1 = 2 * k, 2 * k + 1
        shardA = np.ascontiguousarray(np.concatenate(
            [xpA[b0, 0:128], xpA[b1, 0:128], xpA[b0, 128:192], xpA[b1, 128:192]],
            axis=0))
        shardB = np.ascontiguousarray(np.concatenate(
            [xpB[b0, 0:128], xpB[b1, 0:128], xpB[b0, 128:192], xpB[b1, 128:192]],
            axis=0))
        m = {"xA": shardA, "xB": shardB}
        m.update(consts)
        in_maps.append(m)
    return in_maps


def kernel(**inputs) -> np.ndarray:
    global _COMPILED
    from concourse.bass_utils import run_bass_kernel_spmd

    in_maps = _host_prep(inputs)

    if _COMPILED is None:
        _COMPILED = _build()
    nc = _COMPILED

    res = run_bass_kernel_spmd(nc, in_maps, core_ids=list(range(N_CORES)))
    outs = [r["out"] for r in res.results]

    y = np.empty((B, C, H, W), np.float32)
    for k in range(N_CORES):
        o = np.asarray(outs[k]).reshape(384, H, W).astype(np.float32)
        b0, b1 = 2 * k, 2 * k + 1
        y[b0, 0:128] = o[0:128]
        y[b1, 0:128] = o[128:256]
        y[b0, 128:192] = o[256:320]
        y[b1, 128:192] = o[320:384]
    return y


if __name__ == "__main__":
    import reference

    inputs = {k: np.asarray(v) for k, v in reference.setup_inputs().items()}
    y = kernel(**inputs)
    print("kernel output:", y.shape, y.dtype)

